# revision 27
# baseline (speedup 1.0000x reference)
"""DPCA block (dual-pruned cross-attention) Trainium2 kernel.

Sharding: data-parallel over batch. B=8 -> 8 NeuronCores, one batch per core,
weights replicated, zero collectives.

Per-core dataflow (channel-major: channels on partitions, positions free):
 - chan-LN: gains folded into weights on host; per-position mu/rstd from PE
   ones-matmul broadcast-sums; x'' = (x-mu)*rstd stored bf16.
 - projections bf16 (f32 PSUM accumulate).
 - l2norm factors per head row; khat = k*rstd_b interleaved with v into a
   (khat,v) bf16 pair tensor so one gpsimd.ap_gather pulls both.
 - top-8 rows/cols via vector.max + max_index on f32 scores; 64 gathered
   (row,col) positions per head.
 - attention with head PAIRS block-diag packed on 128 partitions, keys on
   partitions, softmax Z via half-ones matvec, exp needs no max-subtract
   (|sim| <= 1 since khat,qhat l2-normalized).
 - out-proj bf16 + out-LN (same stats trick) + gamma*.. + residual in f32.
"""

import numpy as np

import concourse.bass as bass
import concourse.bacc as bacc
import concourse.mybir as mybir
from concourse.tile import TileContext
from concourse.bass_utils import run_bass_kernel_spmd

F32 = mybir.dt.float32
F32R = mybir.dt.float32r
BF16 = mybir.dt.bfloat16
F16 = mybir.dt.float16
I16 = mybir.dt.int16
I32 = mybir.dt.int32
U32 = mybir.dt.uint32
AX = mybir.AxisListType
OP = mybir.AluOpType
AF = mybir.ActivationFunctionType

C = 256
N = 4096
HEADS = 8
D = 64
PAIRS = 4
INNER = HEADS * D        # 512
NCH = 512
CH = N // NCH            # 8
KEYS = 64                # 8 rows x 8 cols kept per head
EPS = 1e-5


def build_program(stop_stage=99, sub=99):
    nc = bacc.Bacc()

    ctx_d = nc.declare_dram_parameter("ctx", [C, N], F32, False)
    qs_d = nc.declare_dram_parameter("qsrc", [C, N], F32, False)
    wkvT_d = nc.declare_dram_parameter("wkvT", [C, 2 * INNER], F16, False)
    wqT_d = nc.declare_dram_parameter("wqT", [C, INNER], F16, False)
    woutT_d = nc.declare_dram_parameter("woutT", [INNER, C], BF16, False)
    gg_d = nc.declare_dram_parameter("gg", [C, 1], F32, False)
    ident_d = nc.declare_dram_parameter("identc", [128, 64], F16, False)
    onehot8_d = nc.declare_dram_parameter("onehot8c", [128, 8], F32, False)
    m8i_d = nc.declare_dram_parameter("m8ic", [128, 1], I32, False)
    m8f_d = nc.declare_dram_parameter("m8fc", [128, 1], F32, False)
    zsel2_d = nc.declare_dram_parameter("zsel2c", [2, 128], F32, False)
    out_d = nc.declare_dram_parameter("out", [C, N], F32, True)

    with TileContext(nc) as tc:
        with (
            tc.tile_pool(name="const", bufs=1) as constp,
            tc.tile_pool(name="wpool", bufs=1) as wpool,
            tc.tile_pool(name="xin", bufs=2) as xin,
            tc.tile_pool(name="stat", bufs=1) as statp,
            tc.tile_pool(name="xpp", bufs=1) as xpp,
            tc.tile_pool(name="kvq", bufs=1) as kvqp,
            tc.tile_pool(name="pairs", bufs=2) as pairp,
            tc.tile_pool(name="sel", bufs=1) as selp,
            tc.tile_pool(name="attn", bufs=1) as attnp,
            tc.tile_pool(name="ptile", bufs=2) as ptp,
            tc.tile_pool(name="fin", bufs=1) as finp,
            tc.tile_pool(name="psStat", bufs=2, space="PSUM") as psStat,
            tc.tile_pool(name="psMain", bufs=4, space="PSUM") as psMain,
            tc.tile_pool(name="psSmall", bufs=2, space="PSUM") as psSmall,
        ):
            # ------------- constants -------------
            ones128 = constp.tile([128, 128], F32, tag="ones128")
            nc.vector.memset(ones128[:], 1.0)
            ones128_16 = constp.tile([128, 128], BF16, tag="ones128_16")
            nc.vector.memset(ones128_16[:], 1.0)
            ones128_f16 = constp.tile([128, 128], F16, tag="ones128_f16")
            nc.vector.memset(ones128_f16[:], 1.0)
            halves2 = constp.tile([128, 2], F32, tag="halves2")
            nc.vector.memset(halves2[:], 0.0)
            nc.vector.memset(halves2[0:64, 0:1], 1.0)
            nc.vector.memset(halves2[64:128, 1:2], 1.0)
            eps_c = constp.tile([128, 1], F32, tag="eps_c")
            nc.vector.memset(eps_c[:], EPS)
            halves2f = constp.tile([128, 2], F16, tag="halves2f")
            nc.vector.memset(halves2f[:], 0.0)
            nc.vector.memset(halves2f[0:64, 0:1], 1.0)
            nc.vector.memset(halves2f[64:128, 1:2], 1.0)
            ident16 = constp.tile([128, 64], F16, tag="ident16")
            nc.sync.dma_start(out=ident16[:], in_=ident_d[:])
            # block-diag ones: half-broadcast-sum stationary
            halvesbc16 = constp.tile([128, 128], F16, tag="halvesbc16")
            nc.vector.memset(halvesbc16[:], 0.0)
            nc.vector.memset(halvesbc16[0:64, 0:64], 1.0)
            nc.vector.memset(halvesbc16[64:128, 64:128], 1.0)
            zsel2 = constp.tile([2, 128], F32, tag="zsel2")
            nc.sync.dma_start(out=zsel2[:], in_=zsel2_d[:])
            onehot8 = constp.tile([128, 8], F32, tag="onehot8")
            nc.sync.dma_start(out=onehot8[:], in_=onehot8_d[:])
            m8f = constp.tile([128, 1], F32, tag="m8f")
            nc.sync.dma_start(out=m8f[:], in_=m8f_d[:])

            # ------------- weights -------------
            wkvT = [wpool.tile([128, 2 * INNER], F16, tag=f"wkvT{i}", name=f"wkvT{i}") for i in range(2)]
            wqT = [wpool.tile([128, INNER], F16, tag=f"wqT{i}", name=f"wqT{i}") for i in range(2)]
            for i in range(2):
                nc.sync.dma_start(out=wkvT[i][:], in_=wkvT_d[128 * i:128 * (i + 1), :])
                nc.sync.dma_start(out=wqT[i][:], in_=wqT_d[128 * i:128 * (i + 1), :])
            woutT = [wpool.tile([128, C], BF16, tag=f"woutT{i}", name=f"woutT{i}") for i in range(4)]
            for i in range(4):
                nc.sync.dma_start(out=woutT[i][:], in_=woutT_d[128 * i:128 * (i + 1), :])
            gg = [wpool.tile([128, 1], F32, tag=f"gg{i}", name=f"gg{i}") for i in range(2)]
            for i in range(2):
                nc.sync.dma_start(out=gg[i][:], in_=gg_d[128 * i:128 * (i + 1), :])

            # ------------- phase A: chan-LN -> x'' (bf16) -------------
            xpp_t = {}
            for name, src in (("ctx", ctx_d), ("qs", qs_d)):
                xpp_t[name] = [xpp.tile([128, N], F16, tag=f"xpp_{name}{i}", name=f"xpp_{name}{i}")
                               for i in range(2)]
                for ch in range(CH):
                    sl = slice(ch * NCH, (ch + 1) * NCH)
                    xt = [xin.tile([128, NCH], F32, tag="xt", name="xt") for _ in range(2)]
                    for i in range(2):
                        nc.sync.dma_start(out=xt[i][:],
                                          in_=src[128 * i:128 * (i + 1), sl])
                    xsq = [xin.tile([128, NCH], F16, tag="xsq", name="xsq") for _ in range(2)]
                    for i in range(2):
                        nc.scalar.activation(xsq[i][:], xt[i][:], AF.Square)
                    S_ps = psStat.tile([128, NCH], F32, tag="st")
                    nc.tensor.matmul(S_ps[:], lhsT=ones128[:], rhs=xt[0][:],
                                     start=True, stop=False)
                    nc.tensor.matmul(S_ps[:], lhsT=ones128[:], rhs=xt[1][:],
                                     start=False, stop=True)
                    Q_ps = psStat.tile([128, NCH], F32, tag="st")
                    nc.tensor.matmul(Q_ps[:], lhsT=ones128_f16[:], rhs=xsq[0][:],
                                     start=True, stop=False)
                    nc.tensor.matmul(Q_ps[:], lhsT=ones128_f16[:], rhs=xsq[1][:],
                                     start=False, stop=True)
                    t_mu = statp.tile([128, NCH], F32, tag="t_mu")
                    nc.vector.tensor_scalar(t_mu[:], S_ps[:], 1.0 / C,
                                            scalar2=None, op0=OP.mult)
                    t1 = statp.tile([128, NCH], F32, tag="se", name="t1")
                    nc.scalar.activation(t1[:], t_mu[:], AF.Square)
                    varb = statp.tile([128, NCH], F32, tag="varb")
                    nc.vector.scalar_tensor_tensor(out=varb[:], in0=Q_ps[:],
                                                   scalar=1.0 / C, in1=t1[:],
                                                   op0=OP.mult, op1=OP.subtract)
                    se = statp.tile([128, NCH], F32, tag="se")
                    nc.scalar.activation(se[:], varb[:], AF.Sqrt, bias=eps_c[:])
                    rstd_b = statp.tile([128, NCH], F32, tag="rstd_b")
                    nc.vector.reciprocal_approx_fast(out=rstd_b[:], in_=se[:])
                    mu_b = statp.tile([128, NCH], F32, tag="mu_b")
                    nc.vector.tensor_mul(mu_b[:], t_mu[:], rstd_b[:])
                    for i in range(2):
                        tt = statp.tile([128, NCH], F32, tag="xr", bufs=2)
                        nc.vector.tensor_mul(tt[:], xt[i][:], rstd_b[:])
                        nc.gpsimd.tensor_sub(xpp_t[name][i][:, sl], tt[:], mu_b[:])

            # ------------- phase B: software-pipelined per head-pair -------
            # Emission order interleaves pairs so each pair's attention (which
            # waits on its gather) is emitted after the next pair's
            # projections: the in-order PE queue then never stalls on a
            # gather.
            ao16 = [attnp.tile([128, N], BF16, tag=f"ao{p}", name=f"ao{p}")
                    for p in range(PAIRS)]
            il_t, qh_t, ksel_t, kbd_t, vbd_t = {}, {}, {}, {}, {}

            def do_b1(p):
                il = kvqp.tile([128, 2 * N], F16, tag="il", bufs=2, name=f"il{p}")
                qh = kvqp.tile([128, N], F16, tag="qh", bufs=2, name=f"qh{p}")
                il_t[p], qh_t[p] = il, qh
                for ch in range(CH):
                    sl = slice(ch * NCH, (ch + 1) * NCH)
                    # --- projections (k, v, q) for this chunk ---
                    kps = psMain.tile([128, NCH], F32, tag="m")
                    nc.tensor.matmul(kps[:], lhsT=wkvT[0][:, 128 * p:128 * (p + 1)],
                                     rhs=xpp_t["ctx"][0][:, sl], start=True, stop=False)
                    nc.tensor.matmul(kps[:], lhsT=wkvT[1][:, 128 * p:128 * (p + 1)],
                                     rhs=xpp_t["ctx"][1][:, sl], start=False, stop=True)
                    k16c = kvqp.tile([128, NCH], F16, tag="k16c", bufs=2)
                    nc.scalar.copy(k16c[:], kps[:])
                    vps = psMain.tile([128, NCH], F32, tag="m")
                    vo = INNER + 128 * p
                    nc.tensor.matmul(vps[:], lhsT=wkvT[0][:, vo:vo + 128],
                                     rhs=xpp_t["ctx"][0][:, sl], start=True, stop=False)
                    nc.tensor.matmul(vps[:], lhsT=wkvT[1][:, vo:vo + 128],
                                     rhs=xpp_t["ctx"][1][:, sl], start=False, stop=True)
                    nc.scalar.copy(il[:, 2 * sl.start + 1:2 * sl.stop:2], vps[:])
                    qps = psMain.tile([128, NCH], F32, tag="m")
                    nc.tensor.matmul(qps[:], lhsT=wqT[0][:, 128 * p:128 * (p + 1)],
                                     rhs=xpp_t["qs"][0][:, sl], start=True, stop=False)
                    nc.tensor.matmul(qps[:], lhsT=wqT[1][:, 128 * p:128 * (p + 1)],
                                     rhs=xpp_t["qs"][1][:, sl], start=False, stop=True)
                    q16c = kvqp.tile([128, NCH], F16, tag="q16c", bufs=2)
                    nc.scalar.copy(q16c[:], qps[:])
                    # --- l2 factors + khat/qhat ---
                    k2c = kvqp.tile([128, NCH], F16, tag="k2c", bufs=2)
                    nc.vector.tensor_mul(k2c[:], k16c[:], k16c[:])
                    q2c = kvqp.tile([128, NCH], F16, tag="q2c", bufs=2)
                    nc.vector.tensor_mul(q2c[:], q16c[:], q16c[:])
                    rkps = psMain.tile([128, NCH], F32, tag="m")
                    nc.tensor.matmul(rkps[:], lhsT=halvesbc16[:], rhs=k2c[:],
                                     start=True, stop=True)
                    sek = statp.tile([128, NCH], F32, tag="se_", bufs=2)
                    nc.scalar.activation(sek[:], rkps[:], AF.Sqrt)
                    rbk = statp.tile([128, NCH], F32, tag="rb_", bufs=2)
                    nc.vector.reciprocal_approx_fast(out=rbk[:], in_=sek[:])
                    nc.gpsimd.tensor_tensor(out=il[:, 2 * sl.start:2 * sl.stop:2],
                                            in0=k16c[:], in1=rbk[:], op=OP.mult)
                    rqps = psMain.tile([128, NCH], F32, tag="m")
                    nc.tensor.matmul(rqps[:], lhsT=halvesbc16[:], rhs=q2c[:],
                                     start=True, stop=True)
                    seq2 = statp.tile([128, NCH], F32, tag="se_", bufs=2)
                    nc.scalar.activation(seq2[:], rqps[:], AF.Sqrt)
                    rbq = statp.tile([128, NCH], F32, tag="rb_", bufs=2)
                    nc.vector.reciprocal_approx_fast(out=rbq[:], in_=seq2[:])
                    nc.vector.tensor_tensor(out=qh[:, sl], in0=q16c[:],
                                            in1=rbq[:], op=OP.mult)

            def do_b2(p):
                il, qh = il_t[p], qh_t[p]
                # --- segmented |khat| sums + q_probe + scores + topk ---
                il4 = il[:].rearrange("p (h w d) -> p h w d", h=64, w=64, d=2)
                kabs_r = pairp.tile([128, 64], F32, tag="kabsr")
                nc.vector.tensor_reduce(out=kabs_r[:], in_=il4[:, :, :, 0],
                                        axis=AX.X, op=OP.add, apply_absolute_value=True)
                il4c = il[:].rearrange("p (h w d) -> p w h d", h=64, w=64, d=2)
                kabs_c = pairp.tile([128, 64], F32, tag="kabsc")
                nc.vector.tensor_reduce(out=kabs_c[:], in_=il4c[:, :, :, 0],
                                        axis=AX.X, op=OP.add, apply_absolute_value=True)
                qp = pairp.tile([128, 1], F32, tag="qp")
                nc.vector.tensor_reduce(out=qp[:], in_=qh[:], axis=AX.X, op=OP.add)
                qp2 = pairp.tile([128, 2], F32, tag="qp2")
                nc.vector.memset(qp2[:], 0.0)
                nc.vector.tensor_copy(out=qp2[0:64, 0:1], in_=qp[0:64, :])
                nc.vector.tensor_copy(out=qp2[64:128, 1:2], in_=qp[64:128, :])
                sc_r = pairp.tile([2, 64], F32, tag="scr")
                sc_ps = psSmall.tile([2, 64], F32, tag="s")
                nc.tensor.matmul(sc_ps[:], lhsT=qp2[:], rhs=kabs_r[:],
                                 start=True, stop=True)
                nc.scalar.copy(sc_r[:], sc_ps[:])
                sc_c = pairp.tile([2, 64], F32, tag="scc")
                sc_ps2 = psSmall.tile([2, 64], F32, tag="s")
                nc.tensor.matmul(sc_ps2[:], lhsT=qp2[:], rhs=kabs_c[:],
                                 start=True, stop=True)
                nc.scalar.copy(sc_c[:], sc_ps2[:])
                mx = pairp.tile([2, 8], F32, tag="mx")
                idx_r = pairp.tile([2, 8], U32, tag="idxr")
                nc.vector.max(out=mx[:], in_=sc_r[:])
                nc.vector.max_index(out=idx_r[:], in_max=mx[:], in_values=sc_r[:])
                mxc = pairp.tile([2, 8], F32, tag="mxc")
                idx_c = pairp.tile([2, 8], U32, tag="idxc")
                nc.vector.max(out=mxc[:], in_=sc_c[:])
                nc.vector.max_index(out=idx_c[:], in_max=mxc[:], in_values=sc_c[:])
                idxr_f = pairp.tile([2, 8], F32, tag="idxrf")
                nc.vector.tensor_copy(out=idxr_f[:], in_=idx_r[:])
                idxc_f = pairp.tile([2, 8], F32, tag="idxcf")
                nc.vector.tensor_copy(out=idxc_f[:], in_=idx_c[:])
                # broadcast idx rows to all partitions by head half
                rbc_ps = psSmall.tile([128, 8], F32, tag="s")
                nc.tensor.matmul(rbc_ps[:], lhsT=zsel2[:], rhs=idxr_f[:],
                                 start=True, stop=True)
                rbc = pairp.tile([128, 8], F32, tag="rbc")
                nc.scalar.copy(rbc[:], rbc_ps[:])
                cbc_ps = psSmall.tile([128, 8], F32, tag="s")
                nc.tensor.matmul(cbc_ps[:], lhsT=zsel2[:], rhs=idxc_f[:],
                                 start=True, stop=True)
                cbc = pairp.tile([128, 8], F32, tag="cbc")
                nc.scalar.copy(cbc[:], cbc_ps[:])
                # Bcol[p] = idx_c[h(p), p%8]
                junk8 = pairp.tile([128, 8], F32, tag="junk8")
                nc.vector.tensor_mul(junk8[:], cbc[:], onehot8[:])
                Bcol = pairp.tile([128, 1], F32, tag="Bcol")
                nc.vector.tensor_reduce(out=Bcol[:], in_=junk8[:], axis=AX.X,
                                        op=OP.add)
                # wr[p, s] = idx_r[h(p), 2s + ((p>>3)&1)]
                wdiff = pairp.tile([128, 4], F32, tag="wdiff")
                nc.vector.tensor_sub(wdiff[:], rbc[:, 1:8:2], rbc[:, 0:8:2])
                wsel = pairp.tile([128, 4], F32, tag="wsel")
                nc.vector.tensor_scalar(wsel[:], wdiff[:], m8f[:], scalar2=None,
                                        op0=OP.mult)
                wr = pairp.tile([128, 4], F32, tag="wr")
                nc.vector.tensor_add(wr[:], wsel[:], rbc[:, 0:8:2])
                posfw = pairp.tile([128, 4], F32, tag="posfw")
                nc.vector.scalar_tensor_tensor(out=posfw[:], in0=wr[:], scalar=64.0,
                                               in1=Bcol[:].to_broadcast([128, 4]),
                                               op0=OP.mult, op1=OP.add)
                widx32 = pairp.tile([128, 4], I32, tag="widx32")
                nc.vector.tensor_copy(out=widx32[:], in_=posfw[:])
                widx = pairp.tile([128, 4], I16, tag="widx")
                nc.vector.tensor_copy(out=widx[:], in_=widx32[:])
                # --- gather ---
                ksel_il = selp.tile([128, 128], F16, tag="kselil", bufs=2,
                                    name=f"ksel{p}")
                nc.gpsimd.ap_gather(
                    out_ap=ksel_il[:].rearrange("p (k d) -> p k d", d=2),
                    in_ap=il[:].rearrange("p (n d) -> p n d", d=2),
                    idxs_ap=widx[:],
                    channels=128, num_elems=N, d=2, num_idxs=KEYS)
                ksel_t[p] = ksel_il

            def do_extract(p):
                ksel_il = ksel_t[p]
                kbd = selp.tile([128, 128], F16, tag="kbd", bufs=2, name=f"kbd{p}")
                nc.vector.memset(kbd[:], 0.0)
                nc.vector.tensor_copy(out=kbd[0:64, 0:64], in_=ksel_il[0:64, 0:128:2])
                nc.vector.tensor_copy(out=kbd[64:128, 64:128],
                                      in_=ksel_il[64:128, 0:128:2])
                vbd = selp.tile([128, 128], F16, tag="vbd", bufs=2, name=f"vbd{p}")
                nc.vector.memset(vbd[:], 0.0)
                for h in range(2):
                    o = 64 * h
                    tps = psSmall.tile([64, 64], F16, tag="s")
                    nc.tensor.transpose(out=tps[:], in_=ksel_il[o:o + 64, 1:128:2],
                                        identity=ident16[o:o + 64, :])
                    nc.scalar.copy(vbd[o:o + 64, o:o + 64], tps[:])
                kbd_t[p], vbd_t[p] = kbd, vbd

            def do_b3(p):
                kbd, vbd, qh = kbd_t[p], vbd_t[p], qh_t[p]
                # --- attention for this pair ---
                for ch in range(CH):
                    sl = slice(ch * NCH, (ch + 1) * NCH)
                    sps = psMain.tile([128, NCH], F32, tag="m")
                    nc.tensor.matmul(sps[:], lhsT=kbd[:], rhs=qh[:, sl],
                                     start=True, stop=True)
                    pt = ptp.tile([128, NCH], F16, tag="pT")
                    nc.scalar.activation(pt[:], sps[:], AF.Exp)
                    zps = psSmall.tile([2, NCH], F32, tag="s")
                    nc.tensor.matmul(zps[:], lhsT=halves2f[:], rhs=pt[:],
                                     start=True, stop=True)
                    zinv = ptp.tile([2, NCH], F32, tag="zinv")
                    nc.vector.reciprocal_approx_fast(out=zinv[:], in_=zps[:])
                    zb = psMain.tile([128, NCH], F32, tag="m")
                    nc.tensor.matmul(zb[:], lhsT=zsel2[:], rhs=zinv[:],
                                     start=True, stop=True)
                    ph16 = ptp.tile([128, NCH], F16, tag="ph16")
                    nc.vector.tensor_tensor(out=ph16[:], in0=pt[:], in1=zb[:],
                                            op=OP.mult)
                    pvs = psMain.tile([128, NCH], F32, tag="m")
                    nc.tensor.matmul(pvs[:], lhsT=vbd[:], rhs=ph16[:],
                                     start=True, stop=True)
                    nc.gpsimd.tensor_copy(out=ao16[p][:, sl], in_=pvs[:])

            if stop_stage >= 2:
                do_b1(0); do_b2(0)
                do_b1(1); do_b2(1)
                do_extract(0)
                if stop_stage >= 3:
                    do_b3(0)
                do_b1(2); do_b2(2)
                do_extract(1)
                if stop_stage >= 3:
                    do_b3(1)
                do_b1(3); do_b2(3)
                do_extract(2)
                if stop_stage >= 3:
                    do_b3(2)
                do_extract(3)
                if stop_stage >= 3:
                    do_b3(3)

            # ------------- out-proj + out-LN + residual -------------
            y16 = [attnp.tile([128, N], BF16, tag=f"y16_{i}", name=f"y16_{i}") for i in range(2)]
            for ch in range(CH if stop_stage >= 4 else 0):
                sl = slice(ch * NCH, (ch + 1) * NCH)
                for i in range(2):
                    yps = psStat.tile([128, NCH], F32, tag="st")
                    for p in range(PAIRS):
                        nc.tensor.matmul(yps[:],
                                         lhsT=woutT[p][:, 128 * i:128 * (i + 1)],
                                         rhs=ao16[p][:, sl], start=(p == 0),
                                         stop=(p == 3))
                    nc.scalar.copy(y16[i][:, sl], yps[:])
            for ch in range(CH if stop_stage >= 4 else 0):
                sl = slice(ch * NCH, (ch + 1) * NCH)
                y2 = [finp.tile([128, NCH], BF16, tag="y2", name="y2", bufs=2) for _ in range(2)]
                for i in range(2):
                    nc.vector.tensor_mul(y2[i][:], y16[i][:, sl], y16[i][:, sl])
                S_ps = psStat.tile([128, NCH], F32, tag="st")
                nc.tensor.matmul(S_ps[:], lhsT=ones128_16[:], rhs=y16[0][:, sl],
                                 start=True, stop=False)
                nc.tensor.matmul(S_ps[:], lhsT=ones128_16[:], rhs=y16[1][:, sl],
                                 start=False, stop=True)
                Q_ps = psStat.tile([128, NCH], F32, tag="st")
                nc.tensor.matmul(Q_ps[:], lhsT=ones128_16[:], rhs=y2[0][:],
                                 start=True, stop=False)
                nc.tensor.matmul(Q_ps[:], lhsT=ones128_16[:], rhs=y2[1][:],
                                 start=False, stop=True)
                t_mu = finp.tile([128, NCH], F32, tag="ft_mu")
                nc.vector.tensor_scalar(t_mu[:], S_ps[:], 1.0 / C,
                                        scalar2=None, op0=OP.mult)
                t1 = finp.tile([128, NCH], F32, tag="fse", name="ft1")
                nc.scalar.activation(t1[:], t_mu[:], AF.Square)
                varb = finp.tile([128, NCH], F32, tag="fvarb")
                nc.vector.scalar_tensor_tensor(out=varb[:], in0=Q_ps[:], scalar=1.0 / C,
                                               in1=t1[:], op0=OP.mult, op1=OP.subtract)
                se = finp.tile([128, NCH], F32, tag="fse")
                nc.scalar.activation(se[:], varb[:], AF.Sqrt, bias=eps_c[:])
                rstd_b = finp.tile([128, NCH], F32, tag="frstd")
                nc.vector.reciprocal_approx_fast(out=rstd_b[:], in_=se[:])
                mu_b = finp.tile([128, NCH], F32, tag="fmu")
                nc.vector.tensor_mul(mu_b[:], t_mu[:], rstd_b[:])
                for i in range(2):
                    qs_t = finp.tile([128, NCH], F32, tag="qs_t")
                    nc.sync.dma_start(out=qs_t[:], in_=qs_d[128 * i:128 * (i + 1), sl])
                    t = finp.tile([128, NCH], F32, tag="fabc", name="fa", bufs=2)
                    nc.vector.tensor_tensor(out=t[:], in0=y16[i][:, sl],
                                            in1=rstd_b[:], op=OP.mult)
                    t2 = finp.tile([128, NCH], F32, tag="fabc", name="fb", bufs=2)
                    nc.vector.tensor_sub(t2[:], t[:], mu_b[:])
                    t3 = finp.tile([128, NCH], F32, tag="fabc", name="fc", bufs=2)
                    nc.scalar.activation(t3[:], t2[:], AF.Copy, scale=gg[i][:])
                    ot = finp.tile([128, NCH], F32, tag="fo")
                    nc.gpsimd.tensor_add(ot[:], t3[:], qs_t[:])
                    nc.sync.dma_start(out=out_d[128 * i:128 * (i + 1), sl], in_=ot[:])

            if stop_stage < 4:
                for i in range(2):
                    dummy = finp.tile([128, N], F32, tag="dummy")
                    nc.vector.memset(dummy[:], 0.0)
                    nc.sync.dma_start(out=out_d[128 * i:128 * (i + 1), :],
                                      in_=dummy[:])
    nc.finalize()
    return nc


_CACHE = {}


def kernel(**inputs):
    qsrc = np.asarray(inputs["query_source"], np.float32)
    ctx = np.asarray(inputs["context"], np.float32)
    cn_g = np.asarray(inputs["cn_g"], np.float32).reshape(C)
    cn_b = np.asarray(inputs["cn_b"], np.float32).reshape(C)
    qn_g = np.asarray(inputs["qn_g"], np.float32).reshape(C)
    qn_b = np.asarray(inputs["qn_b"], np.float32).reshape(C)
    on_g = np.asarray(inputs["on_g"], np.float32).reshape(C)
    on_b = np.asarray(inputs["on_b"], np.float32).reshape(C)
    w_kv = np.asarray(inputs["w_kv"], np.float32)
    w_q = np.asarray(inputs["w_q"], np.float32)
    w_out = np.asarray(inputs["w_out"], np.float32)
    gamma = float(np.asarray(inputs["gamma"], np.float32).reshape(()))

    assert np.abs(cn_b).max() == 0 and np.abs(qn_b).max() == 0 and \
        np.abs(on_b).max() == 0, "nonzero LN bias not implemented"

    import ml_dtypes
    bf16 = ml_dtypes.bfloat16
    wkvT = np.ascontiguousarray((w_kv * cn_g[None, :]).T).astype(np.float16)
    wqT = np.ascontiguousarray((w_q * qn_g[None, :]).T).astype(np.float16)
    woutT = np.ascontiguousarray(w_out.T).astype(bf16)
    gg = np.ascontiguousarray((gamma * on_g).reshape(C, 1), np.float32)

    p_idx = np.arange(128)
    identc = np.zeros((128, 64), np.float16)
    identc[p_idx, p_idx % 64] = 1.0
    onehot8c = (p_idx[:, None] % 8 == np.arange(8)[None, :]).astype(np.float32)
    m8ic = (((p_idx >> 3) & 1).astype(np.int32)).reshape(128, 1)
    zsel2c = (np.arange(128)[None, :] // 64 ==
              np.arange(2)[:, None]).astype(np.float32)

    if "nc" not in _CACHE:
        _CACHE["nc"] = build_program()
    nc = _CACHE["nc"]

    B = qsrc.shape[0]
    in_maps = []
    for b in range(B):
        in_maps.append({
            "ctx": np.ascontiguousarray(ctx[b].reshape(C, N)),
            "qsrc": np.ascontiguousarray(qsrc[b].reshape(C, N)),
            "wkvT": wkvT,
            "wqT": wqT,
            "woutT": woutT,
            "gg": gg,
            "identc": identc,
            "onehot8c": onehot8c,
            "m8ic": m8ic,
            "m8fc": m8ic.astype(np.float32),
            "zsel2c": zsel2c,
        })
    res = run_bass_kernel_spmd(nc, in_maps, core_ids=list(range(8)))
    outs = [np.asarray(r["out"], np.float32).reshape(1, C, 64, 64)
            for r in res.results]
    return np.concatenate(outs, axis=0)



# revision 28
# speedup vs baseline: 1.0020x; 1.0020x over previous
"""DPCA block (dual-pruned cross-attention) Trainium2 kernel.

Sharding: data-parallel over batch. B=8 -> 8 NeuronCores, one batch per core,
weights replicated, zero collectives.

Per-core dataflow (channel-major: channels on partitions, positions free):
 - chan-LN: gains folded into weights on host; per-position mu/rstd from PE
   ones-matmul broadcast-sums; x'' = (x-mu)*rstd stored bf16.
 - projections bf16 (f32 PSUM accumulate).
 - l2norm factors per head row; khat = k*rstd_b interleaved with v into a
   (khat,v) bf16 pair tensor so one gpsimd.ap_gather pulls both.
 - top-8 rows/cols via vector.max + max_index on f32 scores; 64 gathered
   (row,col) positions per head.
 - attention with head PAIRS block-diag packed on 128 partitions, keys on
   partitions, softmax Z via half-ones matvec, exp needs no max-subtract
   (|sim| <= 1 since khat,qhat l2-normalized).
 - out-proj bf16 + out-LN (same stats trick) + gamma*.. + residual in f32.
"""

import numpy as np

import concourse.bass as bass
import concourse.bacc as bacc
import concourse.mybir as mybir
from concourse.tile import TileContext
from concourse.bass_utils import run_bass_kernel_spmd

F32 = mybir.dt.float32
F32R = mybir.dt.float32r
BF16 = mybir.dt.bfloat16
F16 = mybir.dt.float16
I16 = mybir.dt.int16
I32 = mybir.dt.int32
U32 = mybir.dt.uint32
AX = mybir.AxisListType
OP = mybir.AluOpType
AF = mybir.ActivationFunctionType

C = 256
N = 4096
HEADS = 8
D = 64
PAIRS = 4
INNER = HEADS * D        # 512
NCH = 512
CH = N // NCH            # 8
KEYS = 64                # 8 rows x 8 cols kept per head
EPS = 1e-5


def build_program(stop_stage=99, sub=99):
    nc = bacc.Bacc()

    ctx_d = nc.declare_dram_parameter("ctx", [C, N], F32, False)
    qs_d = nc.declare_dram_parameter("qsrc", [C, N], F32, False)
    wkvT_d = nc.declare_dram_parameter("wkvT", [C, 2 * INNER], F16, False)
    wqT_d = nc.declare_dram_parameter("wqT", [C, INNER], F16, False)
    woutT_d = nc.declare_dram_parameter("woutT", [INNER, C], BF16, False)
    gg_d = nc.declare_dram_parameter("gg", [C, 1], F32, False)
    ident_d = nc.declare_dram_parameter("identc", [128, 64], F16, False)
    onehot8_d = nc.declare_dram_parameter("onehot8c", [128, 8], F32, False)
    m8i_d = nc.declare_dram_parameter("m8ic", [128, 1], I32, False)
    m8f_d = nc.declare_dram_parameter("m8fc", [128, 1], F32, False)
    zsel2_d = nc.declare_dram_parameter("zsel2c", [2, 128], F32, False)
    out_d = nc.declare_dram_parameter("out", [C, N], F32, True)

    with TileContext(nc) as tc:
        with (
            tc.tile_pool(name="const", bufs=1) as constp,
            tc.tile_pool(name="wpool", bufs=1) as wpool,
            tc.tile_pool(name="xin", bufs=2) as xin,
            tc.tile_pool(name="stat", bufs=1) as statp,
            tc.tile_pool(name="xpp", bufs=1) as xpp,
            tc.tile_pool(name="kvq", bufs=1) as kvqp,
            tc.tile_pool(name="pairs", bufs=2) as pairp,
            tc.tile_pool(name="sel", bufs=1) as selp,
            tc.tile_pool(name="attn", bufs=1) as attnp,
            tc.tile_pool(name="ptile", bufs=2) as ptp,
            tc.tile_pool(name="fin", bufs=1) as finp,
            tc.tile_pool(name="psStat", bufs=3, space="PSUM") as psStat,
            tc.tile_pool(name="psMain", bufs=4, space="PSUM") as psMain,
            tc.tile_pool(name="psSmall", bufs=1, space="PSUM") as psSmall,
        ):
            # ------------- constants -------------
            ones128 = constp.tile([128, 128], F32, tag="ones128")
            nc.vector.memset(ones128[:], 1.0)
            ones128_16 = constp.tile([128, 128], BF16, tag="ones128_16")
            nc.vector.memset(ones128_16[:], 1.0)
            ones128_f16 = constp.tile([128, 128], F16, tag="ones128_f16")
            nc.vector.memset(ones128_f16[:], 1.0)
            halves2 = constp.tile([128, 2], F32, tag="halves2")
            nc.vector.memset(halves2[:], 0.0)
            nc.vector.memset(halves2[0:64, 0:1], 1.0)
            nc.vector.memset(halves2[64:128, 1:2], 1.0)
            eps_c = constp.tile([128, 1], F32, tag="eps_c")
            nc.vector.memset(eps_c[:], EPS)
            halves2f = constp.tile([128, 2], F16, tag="halves2f")
            nc.vector.memset(halves2f[:], 0.0)
            nc.vector.memset(halves2f[0:64, 0:1], 1.0)
            nc.vector.memset(halves2f[64:128, 1:2], 1.0)
            ident16 = constp.tile([128, 64], F16, tag="ident16")
            nc.sync.dma_start(out=ident16[:], in_=ident_d[:])
            # block-diag ones: half-broadcast-sum stationary
            halvesbc16 = constp.tile([128, 128], F16, tag="halvesbc16")
            nc.vector.memset(halvesbc16[:], 0.0)
            nc.vector.memset(halvesbc16[0:64, 0:64], 1.0)
            nc.vector.memset(halvesbc16[64:128, 64:128], 1.0)
            zsel2 = constp.tile([2, 128], F32, tag="zsel2")
            nc.sync.dma_start(out=zsel2[:], in_=zsel2_d[:])
            onehot8 = constp.tile([128, 8], F32, tag="onehot8")
            nc.sync.dma_start(out=onehot8[:], in_=onehot8_d[:])
            m8f = constp.tile([128, 1], F32, tag="m8f")
            nc.sync.dma_start(out=m8f[:], in_=m8f_d[:])

            # ------------- weights -------------
            wkvT = [wpool.tile([128, 2 * INNER], F16, tag=f"wkvT{i}", name=f"wkvT{i}") for i in range(2)]
            wqT = [wpool.tile([128, INNER], F16, tag=f"wqT{i}", name=f"wqT{i}") for i in range(2)]
            for i in range(2):
                nc.sync.dma_start(out=wkvT[i][:], in_=wkvT_d[128 * i:128 * (i + 1), :])
                nc.sync.dma_start(out=wqT[i][:], in_=wqT_d[128 * i:128 * (i + 1), :])
            woutT = [wpool.tile([128, C], BF16, tag=f"woutT{i}", name=f"woutT{i}") for i in range(4)]
            for i in range(4):
                nc.sync.dma_start(out=woutT[i][:], in_=woutT_d[128 * i:128 * (i + 1), :])
            gg = [wpool.tile([128, 1], F32, tag=f"gg{i}", name=f"gg{i}") for i in range(2)]
            for i in range(2):
                nc.sync.dma_start(out=gg[i][:], in_=gg_d[128 * i:128 * (i + 1), :])

            # ------------- phase A: chan-LN -> x'' (bf16) -------------
            xpp_t = {}
            for name, src in (("ctx", ctx_d), ("qs", qs_d)):
                xpp_t[name] = [xpp.tile([128, N], F16, tag=f"xpp_{name}{i}", name=f"xpp_{name}{i}")
                               for i in range(2)]
                for ch in range(CH):
                    sl = slice(ch * NCH, (ch + 1) * NCH)
                    xt = [xin.tile([128, NCH], F32, tag="xt", name="xt") for _ in range(2)]
                    for i in range(2):
                        nc.sync.dma_start(out=xt[i][:],
                                          in_=src[128 * i:128 * (i + 1), sl])
                    xsq = [xin.tile([128, NCH], F16, tag="xsq", name="xsq") for _ in range(2)]
                    for i in range(2):
                        nc.scalar.activation(xsq[i][:], xt[i][:], AF.Square)
                    S_ps = psStat.tile([128, NCH], F32, tag="st")
                    nc.tensor.matmul(S_ps[:], lhsT=ones128[:], rhs=xt[0][:],
                                     start=True, stop=False)
                    nc.tensor.matmul(S_ps[:], lhsT=ones128[:], rhs=xt[1][:],
                                     start=False, stop=True)
                    Q_ps = psStat.tile([128, NCH], F32, tag="st")
                    nc.tensor.matmul(Q_ps[:], lhsT=ones128_f16[:], rhs=xsq[0][:],
                                     start=True, stop=False)
                    nc.tensor.matmul(Q_ps[:], lhsT=ones128_f16[:], rhs=xsq[1][:],
                                     start=False, stop=True)
                    t_mu = statp.tile([128, NCH], F32, tag="t_mu")
                    nc.vector.tensor_scalar(t_mu[:], S_ps[:], 1.0 / C,
                                            scalar2=None, op0=OP.mult)
                    t1 = statp.tile([128, NCH], F32, tag="se", name="t1")
                    nc.scalar.activation(t1[:], t_mu[:], AF.Square)
                    varb = statp.tile([128, NCH], F32, tag="varb")
                    nc.vector.scalar_tensor_tensor(out=varb[:], in0=Q_ps[:],
                                                   scalar=1.0 / C, in1=t1[:],
                                                   op0=OP.mult, op1=OP.subtract)
                    se = statp.tile([128, NCH], F32, tag="se")
                    nc.scalar.activation(se[:], varb[:], AF.Sqrt, bias=eps_c[:])
                    rstd_b = statp.tile([128, NCH], F32, tag="rstd_b")
                    nc.vector.reciprocal_approx_fast(out=rstd_b[:], in_=se[:])
                    mu_b = statp.tile([128, NCH], F32, tag="mu_b")
                    nc.vector.tensor_mul(mu_b[:], t_mu[:], rstd_b[:])
                    for i in range(2):
                        tt = statp.tile([128, NCH], F32, tag="xr", bufs=2)
                        nc.vector.tensor_mul(tt[:], xt[i][:], rstd_b[:])
                        nc.gpsimd.tensor_sub(xpp_t[name][i][:, sl], tt[:], mu_b[:])

            # ------------- phase B: software-pipelined per head-pair -------
            # Emission order interleaves pairs so each pair's attention (which
            # waits on its gather) is emitted after the next pair's
            # projections: the in-order PE queue then never stalls on a
            # gather.
            ao16 = [attnp.tile([128, N], BF16, tag=f"ao{p}", name=f"ao{p}")
                    for p in range(PAIRS)]
            il_t, qh_t, ksel_t, kbd_t, vbd_t = {}, {}, {}, {}, {}

            def do_b1(p):
                il = kvqp.tile([128, 2 * N], F16, tag="il", bufs=2, name=f"il{p}")
                qh = kvqp.tile([128, N], F16, tag="qh", bufs=2, name=f"qh{p}")
                il_t[p], qh_t[p] = il, qh
                for ch in range(CH):
                    sl = slice(ch * NCH, (ch + 1) * NCH)
                    # --- projections (k, v, q) for this chunk ---
                    kps = psMain.tile([128, NCH], F32, tag="m")
                    nc.tensor.matmul(kps[:], lhsT=wkvT[0][:, 128 * p:128 * (p + 1)],
                                     rhs=xpp_t["ctx"][0][:, sl], start=True, stop=False)
                    nc.tensor.matmul(kps[:], lhsT=wkvT[1][:, 128 * p:128 * (p + 1)],
                                     rhs=xpp_t["ctx"][1][:, sl], start=False, stop=True)
                    k16c = kvqp.tile([128, NCH], F16, tag="k16c", bufs=2)
                    nc.scalar.copy(k16c[:], kps[:])
                    vps = psMain.tile([128, NCH], F32, tag="m")
                    vo = INNER + 128 * p
                    nc.tensor.matmul(vps[:], lhsT=wkvT[0][:, vo:vo + 128],
                                     rhs=xpp_t["ctx"][0][:, sl], start=True, stop=False)
                    nc.tensor.matmul(vps[:], lhsT=wkvT[1][:, vo:vo + 128],
                                     rhs=xpp_t["ctx"][1][:, sl], start=False, stop=True)
                    nc.scalar.copy(il[:, 2 * sl.start + 1:2 * sl.stop:2], vps[:])
                    qps = psMain.tile([128, NCH], F32, tag="m")
                    nc.tensor.matmul(qps[:], lhsT=wqT[0][:, 128 * p:128 * (p + 1)],
                                     rhs=xpp_t["qs"][0][:, sl], start=True, stop=False)
                    nc.tensor.matmul(qps[:], lhsT=wqT[1][:, 128 * p:128 * (p + 1)],
                                     rhs=xpp_t["qs"][1][:, sl], start=False, stop=True)
                    q16c = kvqp.tile([128, NCH], F16, tag="q16c", bufs=2)
                    nc.scalar.copy(q16c[:], qps[:])
                    # --- l2 factors + khat/qhat ---
                    k2c = kvqp.tile([128, NCH], F16, tag="k2c", bufs=2)
                    nc.vector.tensor_mul(k2c[:], k16c[:], k16c[:])
                    q2c = kvqp.tile([128, NCH], F16, tag="q2c", bufs=2)
                    nc.vector.tensor_mul(q2c[:], q16c[:], q16c[:])
                    rkps = psMain.tile([128, NCH], F32, tag="m")
                    nc.tensor.matmul(rkps[:], lhsT=halvesbc16[:], rhs=k2c[:],
                                     start=True, stop=True)
                    sek = statp.tile([128, NCH], F32, tag="se_", bufs=2)
                    nc.scalar.activation(sek[:], rkps[:], AF.Sqrt)
                    rbk = statp.tile([128, NCH], F32, tag="rb_", bufs=2)
                    nc.vector.reciprocal_approx_fast(out=rbk[:], in_=sek[:])
                    nc.gpsimd.tensor_tensor(out=il[:, 2 * sl.start:2 * sl.stop:2],
                                            in0=k16c[:], in1=rbk[:], op=OP.mult)
                    rqps = psMain.tile([128, NCH], F32, tag="m")
                    nc.tensor.matmul(rqps[:], lhsT=halvesbc16[:], rhs=q2c[:],
                                     start=True, stop=True)
                    seq2 = statp.tile([128, NCH], F32, tag="se_", bufs=2)
                    nc.scalar.activation(seq2[:], rqps[:], AF.Sqrt)
                    rbq = statp.tile([128, NCH], F32, tag="rb_", bufs=2)
                    nc.vector.reciprocal_approx_fast(out=rbq[:], in_=seq2[:])
                    nc.vector.tensor_tensor(out=qh[:, sl], in0=q16c[:],
                                            in1=rbq[:], op=OP.mult)

            def do_b2(p):
                il, qh = il_t[p], qh_t[p]
                # --- segmented |khat| sums + q_probe + scores + topk ---
                il4 = il[:].rearrange("p (h w d) -> p h w d", h=64, w=64, d=2)
                kabs_r = pairp.tile([128, 64], F32, tag="kabsr")
                nc.vector.tensor_reduce(out=kabs_r[:], in_=il4[:, :, :, 0],
                                        axis=AX.X, op=OP.add, apply_absolute_value=True)
                il4c = il[:].rearrange("p (h w d) -> p w h d", h=64, w=64, d=2)
                kabs_c = pairp.tile([128, 64], F32, tag="kabsc")
                nc.vector.tensor_reduce(out=kabs_c[:], in_=il4c[:, :, :, 0],
                                        axis=AX.X, op=OP.add, apply_absolute_value=True)
                qp = pairp.tile([128, 1], F32, tag="qp")
                nc.vector.tensor_reduce(out=qp[:], in_=qh[:], axis=AX.X, op=OP.add)
                qp2 = pairp.tile([128, 2], F32, tag="qp2")
                nc.vector.memset(qp2[:], 0.0)
                nc.vector.tensor_copy(out=qp2[0:64, 0:1], in_=qp[0:64, :])
                nc.vector.tensor_copy(out=qp2[64:128, 1:2], in_=qp[64:128, :])
                sc_r = pairp.tile([2, 64], F32, tag="scr")
                sc_ps = psSmall.tile([2, 64], F32, tag="s")
                nc.tensor.matmul(sc_ps[:], lhsT=qp2[:], rhs=kabs_r[:],
                                 start=True, stop=True)
                nc.scalar.copy(sc_r[:], sc_ps[:])
                sc_c = pairp.tile([2, 64], F32, tag="scc")
                sc_ps2 = psSmall.tile([2, 64], F32, tag="s")
                nc.tensor.matmul(sc_ps2[:], lhsT=qp2[:], rhs=kabs_c[:],
                                 start=True, stop=True)
                nc.scalar.copy(sc_c[:], sc_ps2[:])
                mx = pairp.tile([2, 8], F32, tag="mx")
                idx_r = pairp.tile([2, 8], U32, tag="idxr")
                nc.vector.max(out=mx[:], in_=sc_r[:])
                nc.vector.max_index(out=idx_r[:], in_max=mx[:], in_values=sc_r[:])
                mxc = pairp.tile([2, 8], F32, tag="mxc")
                idx_c = pairp.tile([2, 8], U32, tag="idxc")
                nc.vector.max(out=mxc[:], in_=sc_c[:])
                nc.vector.max_index(out=idx_c[:], in_max=mxc[:], in_values=sc_c[:])
                idxr_f = pairp.tile([2, 8], F32, tag="idxrf")
                nc.vector.tensor_copy(out=idxr_f[:], in_=idx_r[:])
                idxc_f = pairp.tile([2, 8], F32, tag="idxcf")
                nc.vector.tensor_copy(out=idxc_f[:], in_=idx_c[:])
                # broadcast idx rows to all partitions by head half
                rbc_ps = psSmall.tile([128, 8], F32, tag="s")
                nc.tensor.matmul(rbc_ps[:], lhsT=zsel2[:], rhs=idxr_f[:],
                                 start=True, stop=True)
                rbc = pairp.tile([128, 8], F32, tag="rbc")
                nc.scalar.copy(rbc[:], rbc_ps[:])
                cbc_ps = psSmall.tile([128, 8], F32, tag="s")
                nc.tensor.matmul(cbc_ps[:], lhsT=zsel2[:], rhs=idxc_f[:],
                                 start=True, stop=True)
                cbc = pairp.tile([128, 8], F32, tag="cbc")
                nc.scalar.copy(cbc[:], cbc_ps[:])
                # Bcol[p] = idx_c[h(p), p%8]
                junk8 = pairp.tile([128, 8], F32, tag="junk8")
                nc.vector.tensor_mul(junk8[:], cbc[:], onehot8[:])
                Bcol = pairp.tile([128, 1], F32, tag="Bcol")
                nc.vector.tensor_reduce(out=Bcol[:], in_=junk8[:], axis=AX.X,
                                        op=OP.add)
                # wr[p, s] = idx_r[h(p), 2s + ((p>>3)&1)]
                wdiff = pairp.tile([128, 4], F32, tag="wdiff")
                nc.vector.tensor_sub(wdiff[:], rbc[:, 1:8:2], rbc[:, 0:8:2])
                wsel = pairp.tile([128, 4], F32, tag="wsel")
                nc.vector.tensor_scalar(wsel[:], wdiff[:], m8f[:], scalar2=None,
                                        op0=OP.mult)
                wr = pairp.tile([128, 4], F32, tag="wr")
                nc.vector.tensor_add(wr[:], wsel[:], rbc[:, 0:8:2])
                posfw = pairp.tile([128, 4], F32, tag="posfw")
                nc.vector.scalar_tensor_tensor(out=posfw[:], in0=wr[:], scalar=64.0,
                                               in1=Bcol[:].to_broadcast([128, 4]),
                                               op0=OP.mult, op1=OP.add)
                widx32 = pairp.tile([128, 4], I32, tag="widx32")
                nc.vector.tensor_copy(out=widx32[:], in_=posfw[:])
                widx = pairp.tile([128, 4], I16, tag="widx")
                nc.vector.tensor_copy(out=widx[:], in_=widx32[:])
                # --- gather ---
                ksel_il = selp.tile([128, 128], F16, tag="kselil", bufs=2,
                                    name=f"ksel{p}")
                nc.gpsimd.ap_gather(
                    out_ap=ksel_il[:].rearrange("p (k d) -> p k d", d=2),
                    in_ap=il[:].rearrange("p (n d) -> p n d", d=2),
                    idxs_ap=widx[:],
                    channels=128, num_elems=N, d=2, num_idxs=KEYS)
                ksel_t[p] = ksel_il

            def do_extract(p):
                ksel_il = ksel_t[p]
                kbd = selp.tile([128, 128], F16, tag="kbd", bufs=2, name=f"kbd{p}")
                nc.vector.memset(kbd[:], 0.0)
                nc.vector.tensor_copy(out=kbd[0:64, 0:64], in_=ksel_il[0:64, 0:128:2])
                nc.vector.tensor_copy(out=kbd[64:128, 64:128],
                                      in_=ksel_il[64:128, 0:128:2])
                vbd = selp.tile([128, 128], F16, tag="vbd", bufs=2, name=f"vbd{p}")
                nc.vector.memset(vbd[:], 0.0)
                for h in range(2):
                    o = 64 * h
                    tps = psSmall.tile([64, 64], F16, tag="s")
                    nc.tensor.transpose(out=tps[:], in_=ksel_il[o:o + 64, 1:128:2],
                                        identity=ident16[o:o + 64, :])
                    nc.scalar.copy(vbd[o:o + 64, o:o + 64], tps[:])
                kbd_t[p], vbd_t[p] = kbd, vbd

            def do_b3(p):
                kbd, vbd, qh = kbd_t[p], vbd_t[p], qh_t[p]
                # --- attention for this pair ---
                for ch in range(CH):
                    sl = slice(ch * NCH, (ch + 1) * NCH)
                    sps = psMain.tile([128, NCH], F32, tag="m")
                    nc.tensor.matmul(sps[:], lhsT=kbd[:], rhs=qh[:, sl],
                                     start=True, stop=True)
                    pt = ptp.tile([128, NCH], F16, tag="pT")
                    nc.scalar.activation(pt[:], sps[:], AF.Exp)
                    zps = psSmall.tile([2, NCH], F32, tag="s")
                    nc.tensor.matmul(zps[:], lhsT=halves2f[:], rhs=pt[:],
                                     start=True, stop=True)
                    zinv = ptp.tile([2, NCH], F32, tag="zinv")
                    nc.vector.reciprocal_approx_fast(out=zinv[:], in_=zps[:])
                    zb = psMain.tile([128, NCH], F32, tag="m")
                    nc.tensor.matmul(zb[:], lhsT=zsel2[:], rhs=zinv[:],
                                     start=True, stop=True)
                    ph16 = ptp.tile([128, NCH], F16, tag="ph16")
                    nc.vector.tensor_tensor(out=ph16[:], in0=pt[:], in1=zb[:],
                                            op=OP.mult)
                    pvs = psMain.tile([128, NCH], F32, tag="m")
                    nc.tensor.matmul(pvs[:], lhsT=vbd[:], rhs=ph16[:],
                                     start=True, stop=True)
                    nc.gpsimd.tensor_copy(out=ao16[p][:, sl], in_=pvs[:])

            if stop_stage >= 2:
                do_b1(0); do_b2(0)
                do_b1(1); do_b2(1)
                do_extract(0)
                if stop_stage >= 3:
                    do_b3(0)
                do_b1(2); do_b2(2)
                do_extract(1)
                if stop_stage >= 3:
                    do_b3(1)
                do_b1(3); do_b2(3)
                do_extract(2)
                if stop_stage >= 3:
                    do_b3(2)
                do_extract(3)
                if stop_stage >= 3:
                    do_b3(3)

            # ------------- out-proj + out-LN + residual -------------
            y16 = [attnp.tile([128, N], BF16, tag=f"y16_{i}", name=f"y16_{i}") for i in range(2)]
            for ch in range(CH if stop_stage >= 4 else 0):
                sl = slice(ch * NCH, (ch + 1) * NCH)
                for i in range(2):
                    yps = psStat.tile([128, NCH], F32, tag="st")
                    for p in range(PAIRS):
                        nc.tensor.matmul(yps[:],
                                         lhsT=woutT[p][:, 128 * i:128 * (i + 1)],
                                         rhs=ao16[p][:, sl], start=(p == 0),
                                         stop=(p == 3))
                    nc.scalar.copy(y16[i][:, sl], yps[:])
            for ch in range(CH if stop_stage >= 4 else 0):
                sl = slice(ch * NCH, (ch + 1) * NCH)
                y2 = [finp.tile([128, NCH], BF16, tag="y2", name="y2", bufs=2) for _ in range(2)]
                for i in range(2):
                    nc.vector.tensor_mul(y2[i][:], y16[i][:, sl], y16[i][:, sl])
                S_ps = psStat.tile([128, NCH], F32, tag="st")
                nc.tensor.matmul(S_ps[:], lhsT=ones128_16[:], rhs=y16[0][:, sl],
                                 start=True, stop=False)
                nc.tensor.matmul(S_ps[:], lhsT=ones128_16[:], rhs=y16[1][:, sl],
                                 start=False, stop=True)
                Q_ps = psStat.tile([128, NCH], F32, tag="st")
                nc.tensor.matmul(Q_ps[:], lhsT=ones128_16[:], rhs=y2[0][:],
                                 start=True, stop=False)
                nc.tensor.matmul(Q_ps[:], lhsT=ones128_16[:], rhs=y2[1][:],
                                 start=False, stop=True)
                t_mu = finp.tile([128, NCH], F32, tag="ft_mu")
                nc.vector.tensor_scalar(t_mu[:], S_ps[:], 1.0 / C,
                                        scalar2=None, op0=OP.mult)
                t1 = finp.tile([128, NCH], F32, tag="fse", name="ft1")
                nc.scalar.activation(t1[:], t_mu[:], AF.Square)
                varb = finp.tile([128, NCH], F32, tag="fvarb")
                nc.vector.scalar_tensor_tensor(out=varb[:], in0=Q_ps[:], scalar=1.0 / C,
                                               in1=t1[:], op0=OP.mult, op1=OP.subtract)
                se = finp.tile([128, NCH], F32, tag="fse")
                nc.scalar.activation(se[:], varb[:], AF.Sqrt, bias=eps_c[:])
                rstd_b = finp.tile([128, NCH], F32, tag="frstd")
                nc.vector.reciprocal_approx_fast(out=rstd_b[:], in_=se[:])
                mu_b = finp.tile([128, NCH], F32, tag="fmu")
                nc.vector.tensor_mul(mu_b[:], t_mu[:], rstd_b[:])
                for i in range(2):
                    qs_t = finp.tile([128, NCH], F32, tag="qs_t")
                    nc.sync.dma_start(out=qs_t[:], in_=qs_d[128 * i:128 * (i + 1), sl])
                    t = finp.tile([128, NCH], F32, tag="fabc", name="fa", bufs=2)
                    nc.vector.tensor_tensor(out=t[:], in0=y16[i][:, sl],
                                            in1=rstd_b[:], op=OP.mult)
                    t2 = finp.tile([128, NCH], F32, tag="fabc", name="fb", bufs=2)
                    nc.vector.tensor_sub(t2[:], t[:], mu_b[:])
                    t3 = finp.tile([128, NCH], F32, tag="fabc", name="fc", bufs=2)
                    nc.scalar.activation(t3[:], t2[:], AF.Copy, scale=gg[i][:])
                    ot = finp.tile([128, NCH], F32, tag="fo")
                    nc.gpsimd.tensor_add(ot[:], t3[:], qs_t[:])
                    nc.sync.dma_start(out=out_d[128 * i:128 * (i + 1), sl], in_=ot[:])

            if stop_stage < 4:
                for i in range(2):
                    dummy = finp.tile([128, N], F32, tag="dummy")
                    nc.vector.memset(dummy[:], 0.0)
                    nc.sync.dma_start(out=out_d[128 * i:128 * (i + 1), :],
                                      in_=dummy[:])
    nc.finalize()
    return nc


_CACHE = {}


def kernel(**inputs):
    qsrc = np.asarray(inputs["query_source"], np.float32)
    ctx = np.asarray(inputs["context"], np.float32)
    cn_g = np.asarray(inputs["cn_g"], np.float32).reshape(C)
    cn_b = np.asarray(inputs["cn_b"], np.float32).reshape(C)
    qn_g = np.asarray(inputs["qn_g"], np.float32).reshape(C)
    qn_b = np.asarray(inputs["qn_b"], np.float32).reshape(C)
    on_g = np.asarray(inputs["on_g"], np.float32).reshape(C)
    on_b = np.asarray(inputs["on_b"], np.float32).reshape(C)
    w_kv = np.asarray(inputs["w_kv"], np.float32)
    w_q = np.asarray(inputs["w_q"], np.float32)
    w_out = np.asarray(inputs["w_out"], np.float32)
    gamma = float(np.asarray(inputs["gamma"], np.float32).reshape(()))

    assert np.abs(cn_b).max() == 0 and np.abs(qn_b).max() == 0 and \
        np.abs(on_b).max() == 0, "nonzero LN bias not implemented"

    import ml_dtypes
    bf16 = ml_dtypes.bfloat16
    wkvT = np.ascontiguousarray((w_kv * cn_g[None, :]).T).astype(np.float16)
    wqT = np.ascontiguousarray((w_q * qn_g[None, :]).T).astype(np.float16)
    woutT = np.ascontiguousarray(w_out.T).astype(bf16)
    gg = np.ascontiguousarray((gamma * on_g).reshape(C, 1), np.float32)

    p_idx = np.arange(128)
    identc = np.zeros((128, 64), np.float16)
    identc[p_idx, p_idx % 64] = 1.0
    onehot8c = (p_idx[:, None] % 8 == np.arange(8)[None, :]).astype(np.float32)
    m8ic = (((p_idx >> 3) & 1).astype(np.int32)).reshape(128, 1)
    zsel2c = (np.arange(128)[None, :] // 64 ==
              np.arange(2)[:, None]).astype(np.float32)

    if "nc" not in _CACHE:
        _CACHE["nc"] = build_program()
    nc = _CACHE["nc"]

    B = qsrc.shape[0]
    in_maps = []
    for b in range(B):
        in_maps.append({
            "ctx": np.ascontiguousarray(ctx[b].reshape(C, N)),
            "qsrc": np.ascontiguousarray(qsrc[b].reshape(C, N)),
            "wkvT": wkvT,
            "wqT": wqT,
            "woutT": woutT,
            "gg": gg,
            "identc": identc,
            "onehot8c": onehot8c,
            "m8ic": m8ic,
            "m8fc": m8ic.astype(np.float32),
            "zsel2c": zsel2c,
        })
    res = run_bass_kernel_spmd(nc, in_maps, core_ids=list(range(8)))
    outs = [np.asarray(r["out"], np.float32).reshape(1, C, 64, 64)
            for r in res.results]
    return np.concatenate(outs, axis=0)



# revision 30
# speedup vs baseline: 1.0709x; 1.0688x over previous
"""DPCA block (dual-pruned cross-attention) Trainium2 kernel.

Sharding: data-parallel over batch. B=8 -> 8 NeuronCores, one batch per core,
weights replicated, zero collectives.

Per-core dataflow (channel-major: channels on partitions, positions free):
 - chan-LN: gains folded into weights on host; per-position mu/rstd from PE
   ones-matmul broadcast-sums; x'' = (x-mu)*rstd stored bf16.
 - projections bf16 (f32 PSUM accumulate).
 - l2norm factors per head row; khat = k*rstd_b interleaved with v into a
   (khat,v) bf16 pair tensor so one gpsimd.ap_gather pulls both.
 - top-8 rows/cols via vector.max + max_index on f32 scores; 64 gathered
   (row,col) positions per head.
 - attention with head PAIRS block-diag packed on 128 partitions, keys on
   partitions, softmax Z via half-ones matvec, exp needs no max-subtract
   (|sim| <= 1 since khat,qhat l2-normalized).
 - out-proj bf16 + out-LN (same stats trick) + gamma*.. + residual in f32.
"""

import numpy as np

import concourse.bass as bass
import concourse.bacc as bacc
import concourse.mybir as mybir
from concourse.tile import TileContext
from concourse.bass_utils import run_bass_kernel_spmd

F32 = mybir.dt.float32
F32R = mybir.dt.float32r
BF16 = mybir.dt.bfloat16
F16 = mybir.dt.float16
I16 = mybir.dt.int16
I32 = mybir.dt.int32
U32 = mybir.dt.uint32
AX = mybir.AxisListType
OP = mybir.AluOpType
AF = mybir.ActivationFunctionType

C = 256
N = 4096
HEADS = 8
D = 64
PAIRS = 4
INNER = HEADS * D        # 512
NCH = 512
CH = N // NCH            # 8
KEYS = 64                # 8 rows x 8 cols kept per head
EPS = 1e-5


def build_program(stop_stage=99, sub=99):
    nc = bacc.Bacc()

    ctx_d = nc.declare_dram_parameter("ctx", [C, N], F32, False)
    qs_d = nc.declare_dram_parameter("qsrc", [C, N], F32, False)
    wkvT_d = nc.declare_dram_parameter("wkvT", [C, 2 * INNER], F16, False)
    wqT_d = nc.declare_dram_parameter("wqT", [C, INNER], F16, False)
    woutT_d = nc.declare_dram_parameter("woutT", [INNER, C], BF16, False)
    gg_d = nc.declare_dram_parameter("gg", [C, 1], F32, False)
    ident_d = nc.declare_dram_parameter("identc", [128, 64], F16, False)
    onehot8_d = nc.declare_dram_parameter("onehot8c", [128, 8], F32, False)
    m8i_d = nc.declare_dram_parameter("m8ic", [128, 1], I32, False)
    m8f_d = nc.declare_dram_parameter("m8fc", [128, 1], F32, False)
    zsel2_d = nc.declare_dram_parameter("zsel2c", [2, 128], F32, False)
    out_d = nc.declare_dram_parameter("out", [C, N], F32, True)

    with TileContext(nc) as tc:
        with (
            tc.tile_pool(name="const", bufs=1) as constp,
            tc.tile_pool(name="wpool", bufs=1) as wpool,
            tc.tile_pool(name="xin", bufs=2) as xin,
            tc.tile_pool(name="stat", bufs=1) as statp,
            tc.tile_pool(name="xpp", bufs=1) as xpp,
            tc.tile_pool(name="kvq", bufs=1) as kvqp,
            tc.tile_pool(name="pairs", bufs=2) as pairp,
            tc.tile_pool(name="sel", bufs=1) as selp,
            tc.tile_pool(name="attn", bufs=1) as attnp,
            tc.tile_pool(name="ptile", bufs=2) as ptp,
            tc.tile_pool(name="fin", bufs=1) as finp,
            tc.tile_pool(name="psStat", bufs=3, space="PSUM") as psStat,
            tc.tile_pool(name="psMain", bufs=4, space="PSUM") as psMain,
            tc.tile_pool(name="psSmall", bufs=1, space="PSUM") as psSmall,
        ):
            # ------------- constants -------------
            ones128 = constp.tile([128, 128], F32, tag="ones128")
            nc.vector.memset(ones128[:], 1.0)
            ones128_16 = constp.tile([128, 128], BF16, tag="ones128_16")
            nc.vector.memset(ones128_16[:], 1.0)
            ones128_f16 = constp.tile([128, 128], F16, tag="ones128_f16")
            nc.vector.memset(ones128_f16[:], 1.0)
            halves2 = constp.tile([128, 2], F32, tag="halves2")
            nc.vector.memset(halves2[:], 0.0)
            nc.vector.memset(halves2[0:64, 0:1], 1.0)
            nc.vector.memset(halves2[64:128, 1:2], 1.0)
            eps_c = constp.tile([128, 1], F32, tag="eps_c")
            nc.vector.memset(eps_c[:], EPS)
            halves2f = constp.tile([128, 2], F16, tag="halves2f")
            nc.vector.memset(halves2f[:], 0.0)
            nc.vector.memset(halves2f[0:64, 0:1], 1.0)
            nc.vector.memset(halves2f[64:128, 1:2], 1.0)
            ident16 = constp.tile([128, 64], F16, tag="ident16")
            nc.sync.dma_start(out=ident16[:], in_=ident_d[:])
            # block-diag ones: half-broadcast-sum stationary
            halvesbc16 = constp.tile([128, 128], F16, tag="halvesbc16")
            nc.vector.memset(halvesbc16[:], 0.0)
            nc.vector.memset(halvesbc16[0:64, 0:64], 1.0)
            nc.vector.memset(halvesbc16[64:128, 64:128], 1.0)
            zsel2 = constp.tile([2, 128], F32, tag="zsel2")
            nc.sync.dma_start(out=zsel2[:], in_=zsel2_d[:])
            onehot8 = constp.tile([128, 8], F32, tag="onehot8")
            nc.sync.dma_start(out=onehot8[:], in_=onehot8_d[:])
            m8f = constp.tile([128, 1], F32, tag="m8f")
            nc.sync.dma_start(out=m8f[:], in_=m8f_d[:])

            # ------------- weights -------------
            wkvT = [wpool.tile([128, 2 * INNER], F16, tag=f"wkvT{i}", name=f"wkvT{i}") for i in range(2)]
            wqT = [wpool.tile([128, INNER], F16, tag=f"wqT{i}", name=f"wqT{i}") for i in range(2)]
            for i in range(2):
                nc.sync.dma_start(out=wkvT[i][:], in_=wkvT_d[128 * i:128 * (i + 1), :])
                nc.sync.dma_start(out=wqT[i][:], in_=wqT_d[128 * i:128 * (i + 1), :])
            woutT = [wpool.tile([128, C], BF16, tag=f"woutT{i}", name=f"woutT{i}") for i in range(4)]
            for i in range(4):
                nc.sync.dma_start(out=woutT[i][:], in_=woutT_d[128 * i:128 * (i + 1), :])
            gg = [wpool.tile([128, 1], F32, tag=f"gg{i}", name=f"gg{i}") for i in range(2)]
            for i in range(2):
                nc.sync.dma_start(out=gg[i][:], in_=gg_d[128 * i:128 * (i + 1), :])

            # ------------- phase A: chan-LN -> x'' (f16) -------------
            # ctx gets the full (x-mu)*rstd (v needs it); qs gets only x-mu:
            # the q l2norm cancels any per-position scale, so rstd_qs (and the
            # whole variance pipeline) is unnecessary for the q path.
            xpp_t = {}
            for name, src in (("ctx", ctx_d), ("qs", qs_d)):
                xpp_t[name] = [xpp.tile([128, N], F16, tag=f"xpp_{name}{i}", name=f"xpp_{name}{i}")
                               for i in range(2)]
                for ch in range(CH):
                    sl = slice(ch * NCH, (ch + 1) * NCH)
                    xt = [xin.tile([128, NCH], F32, tag="xt", name="xt") for _ in range(2)]
                    for i in range(2):
                        nc.sync.dma_start(out=xt[i][:],
                                          in_=src[128 * i:128 * (i + 1), sl])
                    S_ps = psStat.tile([128, NCH], F32, tag="st")
                    nc.tensor.matmul(S_ps[:], lhsT=ones128[:], rhs=xt[0][:],
                                     start=True, stop=False)
                    nc.tensor.matmul(S_ps[:], lhsT=ones128[:], rhs=xt[1][:],
                                     start=False, stop=True)
                    t_mu = statp.tile([128, NCH], F32, tag="t_mu", bufs=2)
                    nc.vector.tensor_scalar(t_mu[:], S_ps[:], 1.0 / C,
                                            scalar2=None, op0=OP.mult)
                    if name == "qs":
                        for i in range(2):
                            nc.gpsimd.tensor_sub(xpp_t[name][i][:, sl],
                                                 xt[i][:], t_mu[:])
                        continue
                    xsq = [xin.tile([128, NCH], F16, tag="xsq", name="xsq") for _ in range(2)]
                    for i in range(2):
                        nc.scalar.activation(xsq[i][:], xt[i][:], AF.Square)
                    Q_ps = psStat.tile([128, NCH], F32, tag="st")
                    nc.tensor.matmul(Q_ps[:], lhsT=ones128_f16[:], rhs=xsq[0][:],
                                     start=True, stop=False)
                    nc.tensor.matmul(Q_ps[:], lhsT=ones128_f16[:], rhs=xsq[1][:],
                                     start=False, stop=True)
                    t1 = statp.tile([128, NCH], F32, tag="se", name="t1")
                    nc.scalar.activation(t1[:], t_mu[:], AF.Square)
                    varb = statp.tile([128, NCH], F32, tag="varb")
                    nc.vector.scalar_tensor_tensor(out=varb[:], in0=Q_ps[:],
                                                   scalar=1.0 / C, in1=t1[:],
                                                   op0=OP.mult, op1=OP.subtract)
                    se = statp.tile([128, NCH], F32, tag="se")
                    nc.scalar.activation(se[:], varb[:], AF.Sqrt, bias=eps_c[:])
                    rstd_b = statp.tile([128, NCH], F32, tag="rstd_b")
                    nc.vector.reciprocal_approx_fast(out=rstd_b[:], in_=se[:])
                    mu_b = statp.tile([128, NCH], F32, tag="mu_b")
                    nc.vector.tensor_mul(mu_b[:], t_mu[:], rstd_b[:])
                    for i in range(2):
                        tt = statp.tile([128, NCH], F32, tag="xr", bufs=2)
                        nc.vector.tensor_mul(tt[:], xt[i][:], rstd_b[:])
                        nc.gpsimd.tensor_sub(xpp_t[name][i][:, sl], tt[:], mu_b[:])

            # ------------- phase B: software-pipelined per head-pair -------
            # Emission order interleaves pairs so each pair's attention (which
            # waits on its gather) is emitted after the next pair's
            # projections: the in-order PE queue then never stalls on a
            # gather.
            ao16 = [attnp.tile([128, N], BF16, tag=f"ao{p}", name=f"ao{p}")
                    for p in range(PAIRS)]
            il_t, qh_t, ksel_t, kbd_t, vbd_t = {}, {}, {}, {}, {}

            def do_b1(p):
                il = kvqp.tile([128, 2 * N], F16, tag="il", bufs=2, name=f"il{p}")
                qh = kvqp.tile([128, N], F16, tag="qh", bufs=2, name=f"qh{p}")
                il_t[p], qh_t[p] = il, qh
                for ch in range(CH):
                    sl = slice(ch * NCH, (ch + 1) * NCH)
                    # --- projections (k, v, q) for this chunk ---
                    kps = psMain.tile([128, NCH], F32, tag="m")
                    nc.tensor.matmul(kps[:], lhsT=wkvT[0][:, 128 * p:128 * (p + 1)],
                                     rhs=xpp_t["ctx"][0][:, sl], start=True, stop=False)
                    nc.tensor.matmul(kps[:], lhsT=wkvT[1][:, 128 * p:128 * (p + 1)],
                                     rhs=xpp_t["ctx"][1][:, sl], start=False, stop=True)
                    k16c = kvqp.tile([128, NCH], F16, tag="k16c", bufs=2)
                    nc.scalar.copy(k16c[:], kps[:])
                    vps = psMain.tile([128, NCH], F32, tag="m")
                    vo = INNER + 128 * p
                    nc.tensor.matmul(vps[:], lhsT=wkvT[0][:, vo:vo + 128],
                                     rhs=xpp_t["ctx"][0][:, sl], start=True, stop=False)
                    nc.tensor.matmul(vps[:], lhsT=wkvT[1][:, vo:vo + 128],
                                     rhs=xpp_t["ctx"][1][:, sl], start=False, stop=True)
                    nc.scalar.copy(il[:, 2 * sl.start + 1:2 * sl.stop:2], vps[:])
                    qps = psMain.tile([128, NCH], F32, tag="m")
                    nc.tensor.matmul(qps[:], lhsT=wqT[0][:, 128 * p:128 * (p + 1)],
                                     rhs=xpp_t["qs"][0][:, sl], start=True, stop=False)
                    nc.tensor.matmul(qps[:], lhsT=wqT[1][:, 128 * p:128 * (p + 1)],
                                     rhs=xpp_t["qs"][1][:, sl], start=False, stop=True)
                    q16c = kvqp.tile([128, NCH], F16, tag="q16c", bufs=2)
                    nc.scalar.copy(q16c[:], qps[:])
                    # --- l2 factors + khat/qhat ---
                    k2c = kvqp.tile([128, NCH], F16, tag="k2c", bufs=2)
                    nc.vector.tensor_mul(k2c[:], k16c[:], k16c[:])
                    q2c = kvqp.tile([128, NCH], F16, tag="q2c", bufs=2)
                    nc.vector.tensor_mul(q2c[:], q16c[:], q16c[:])
                    rkps = psMain.tile([128, NCH], F32, tag="m")
                    nc.tensor.matmul(rkps[:], lhsT=halvesbc16[:], rhs=k2c[:],
                                     start=True, stop=True)
                    sek = statp.tile([128, NCH], F32, tag="se_", bufs=2)
                    nc.scalar.activation(sek[:], rkps[:], AF.Sqrt)
                    rbk = statp.tile([128, NCH], F32, tag="rb_", bufs=2)
                    nc.vector.reciprocal_approx_fast(out=rbk[:], in_=sek[:])
                    nc.gpsimd.tensor_tensor(out=il[:, 2 * sl.start:2 * sl.stop:2],
                                            in0=k16c[:], in1=rbk[:], op=OP.mult)
                    rqps = psMain.tile([128, NCH], F32, tag="m")
                    nc.tensor.matmul(rqps[:], lhsT=halvesbc16[:], rhs=q2c[:],
                                     start=True, stop=True)
                    seq2 = statp.tile([128, NCH], F32, tag="se_", bufs=2)
                    nc.scalar.activation(seq2[:], rqps[:], AF.Sqrt)
                    rbq = statp.tile([128, NCH], F32, tag="rb_", bufs=2)
                    nc.vector.reciprocal_approx_fast(out=rbq[:], in_=seq2[:])
                    nc.vector.tensor_tensor(out=qh[:, sl], in0=q16c[:],
                                            in1=rbq[:], op=OP.mult)

            def do_b2(p):
                il, qh = il_t[p], qh_t[p]
                # --- segmented |khat| sums + q_probe + scores + topk ---
                il4 = il[:].rearrange("p (h w d) -> p h w d", h=64, w=64, d=2)
                kabs_r = pairp.tile([128, 64], F32, tag="kabsr")
                nc.vector.tensor_reduce(out=kabs_r[:], in_=il4[:, :, :, 0],
                                        axis=AX.X, op=OP.add, apply_absolute_value=True)
                il4c = il[:].rearrange("p (h w d) -> p w h d", h=64, w=64, d=2)
                kabs_c = pairp.tile([128, 64], F32, tag="kabsc")
                nc.vector.tensor_reduce(out=kabs_c[:], in_=il4c[:, :, :, 0],
                                        axis=AX.X, op=OP.add, apply_absolute_value=True)
                qp = pairp.tile([128, 1], F32, tag="qp")
                nc.vector.tensor_reduce(out=qp[:], in_=qh[:], axis=AX.X, op=OP.add)
                qp2 = pairp.tile([128, 2], F32, tag="qp2")
                nc.vector.memset(qp2[:], 0.0)
                nc.vector.tensor_copy(out=qp2[0:64, 0:1], in_=qp[0:64, :])
                nc.vector.tensor_copy(out=qp2[64:128, 1:2], in_=qp[64:128, :])
                sc_r = pairp.tile([2, 64], F32, tag="scr")
                sc_ps = psSmall.tile([2, 64], F32, tag="s")
                nc.tensor.matmul(sc_ps[:], lhsT=qp2[:], rhs=kabs_r[:],
                                 start=True, stop=True)
                nc.scalar.copy(sc_r[:], sc_ps[:])
                sc_c = pairp.tile([2, 64], F32, tag="scc")
                sc_ps2 = psSmall.tile([2, 64], F32, tag="s")
                nc.tensor.matmul(sc_ps2[:], lhsT=qp2[:], rhs=kabs_c[:],
                                 start=True, stop=True)
                nc.scalar.copy(sc_c[:], sc_ps2[:])
                mx = pairp.tile([2, 8], F32, tag="mx")
                idx_r = pairp.tile([2, 8], U32, tag="idxr")
                nc.vector.max(out=mx[:], in_=sc_r[:])
                nc.vector.max_index(out=idx_r[:], in_max=mx[:], in_values=sc_r[:])
                mxc = pairp.tile([2, 8], F32, tag="mxc")
                idx_c = pairp.tile([2, 8], U32, tag="idxc")
                nc.vector.max(out=mxc[:], in_=sc_c[:])
                nc.vector.max_index(out=idx_c[:], in_max=mxc[:], in_values=sc_c[:])
                idxr_f = pairp.tile([2, 8], F32, tag="idxrf")
                nc.vector.tensor_copy(out=idxr_f[:], in_=idx_r[:])
                idxc_f = pairp.tile([2, 8], F32, tag="idxcf")
                nc.vector.tensor_copy(out=idxc_f[:], in_=idx_c[:])
                # broadcast idx rows to all partitions by head half
                rbc_ps = psSmall.tile([128, 8], F32, tag="s")
                nc.tensor.matmul(rbc_ps[:], lhsT=zsel2[:], rhs=idxr_f[:],
                                 start=True, stop=True)
                rbc = pairp.tile([128, 8], F32, tag="rbc")
                nc.scalar.copy(rbc[:], rbc_ps[:])
                cbc_ps = psSmall.tile([128, 8], F32, tag="s")
                nc.tensor.matmul(cbc_ps[:], lhsT=zsel2[:], rhs=idxc_f[:],
                                 start=True, stop=True)
                cbc = pairp.tile([128, 8], F32, tag="cbc")
                nc.scalar.copy(cbc[:], cbc_ps[:])
                # Bcol[p] = idx_c[h(p), p%8]
                junk8 = pairp.tile([128, 8], F32, tag="junk8")
                nc.vector.tensor_mul(junk8[:], cbc[:], onehot8[:])
                Bcol = pairp.tile([128, 1], F32, tag="Bcol")
                nc.vector.tensor_reduce(out=Bcol[:], in_=junk8[:], axis=AX.X,
                                        op=OP.add)
                # wr[p, s] = idx_r[h(p), 2s + ((p>>3)&1)]
                wdiff = pairp.tile([128, 4], F32, tag="wdiff")
                nc.vector.tensor_sub(wdiff[:], rbc[:, 1:8:2], rbc[:, 0:8:2])
                wsel = pairp.tile([128, 4], F32, tag="wsel")
                nc.vector.tensor_scalar(wsel[:], wdiff[:], m8f[:], scalar2=None,
                                        op0=OP.mult)
                wr = pairp.tile([128, 4], F32, tag="wr")
                nc.vector.tensor_add(wr[:], wsel[:], rbc[:, 0:8:2])
                posfw = pairp.tile([128, 4], F32, tag="posfw")
                nc.vector.scalar_tensor_tensor(out=posfw[:], in0=wr[:], scalar=64.0,
                                               in1=Bcol[:].to_broadcast([128, 4]),
                                               op0=OP.mult, op1=OP.add)
                widx32 = pairp.tile([128, 4], I32, tag="widx32")
                nc.vector.tensor_copy(out=widx32[:], in_=posfw[:])
                widx = pairp.tile([128, 4], I16, tag="widx")
                nc.vector.tensor_copy(out=widx[:], in_=widx32[:])
                # --- gather ---
                ksel_il = selp.tile([128, 128], F16, tag="kselil", bufs=2,
                                    name=f"ksel{p}")
                nc.gpsimd.ap_gather(
                    out_ap=ksel_il[:].rearrange("p (k d) -> p k d", d=2),
                    in_ap=il[:].rearrange("p (n d) -> p n d", d=2),
                    idxs_ap=widx[:],
                    channels=128, num_elems=N, d=2, num_idxs=KEYS)
                ksel_t[p] = ksel_il

            def do_extract(p):
                ksel_il = ksel_t[p]
                kbd = selp.tile([128, 128], F16, tag="kbd", bufs=2, name=f"kbd{p}")
                nc.vector.memset(kbd[:], 0.0)
                nc.vector.tensor_copy(out=kbd[0:64, 0:64], in_=ksel_il[0:64, 0:128:2])
                nc.vector.tensor_copy(out=kbd[64:128, 64:128],
                                      in_=ksel_il[64:128, 0:128:2])
                vbd = selp.tile([128, 128], F16, tag="vbd", bufs=2, name=f"vbd{p}")
                nc.vector.memset(vbd[:], 0.0)
                for h in range(2):
                    o = 64 * h
                    tps = psSmall.tile([64, 64], F16, tag="s")
                    nc.tensor.transpose(out=tps[:], in_=ksel_il[o:o + 64, 1:128:2],
                                        identity=ident16[o:o + 64, :])
                    nc.scalar.copy(vbd[o:o + 64, o:o + 64], tps[:])
                kbd_t[p], vbd_t[p] = kbd, vbd

            def do_b3(p):
                kbd, vbd, qh = kbd_t[p], vbd_t[p], qh_t[p]
                # --- attention for this pair ---
                for ch in range(CH):
                    sl = slice(ch * NCH, (ch + 1) * NCH)
                    sps = psMain.tile([128, NCH], F32, tag="m")
                    nc.tensor.matmul(sps[:], lhsT=kbd[:], rhs=qh[:, sl],
                                     start=True, stop=True)
                    pt = ptp.tile([128, NCH], F16, tag="pT")
                    nc.scalar.activation(pt[:], sps[:], AF.Exp)
                    zps = psSmall.tile([2, NCH], F32, tag="s")
                    nc.tensor.matmul(zps[:], lhsT=halves2f[:], rhs=pt[:],
                                     start=True, stop=True)
                    zinv = ptp.tile([2, NCH], F32, tag="zinv")
                    nc.vector.reciprocal_approx_fast(out=zinv[:], in_=zps[:])
                    zb = psMain.tile([128, NCH], F32, tag="m")
                    nc.tensor.matmul(zb[:], lhsT=zsel2[:], rhs=zinv[:],
                                     start=True, stop=True)
                    ph16 = ptp.tile([128, NCH], F16, tag="ph16")
                    nc.vector.tensor_tensor(out=ph16[:], in0=pt[:], in1=zb[:],
                                            op=OP.mult)
                    pvs = psMain.tile([128, NCH], F32, tag="m")
                    nc.tensor.matmul(pvs[:], lhsT=vbd[:], rhs=ph16[:],
                                     start=True, stop=True)
                    nc.gpsimd.tensor_copy(out=ao16[p][:, sl], in_=pvs[:])

            if stop_stage >= 2:
                do_b1(0); do_b2(0)
                do_b1(1); do_b2(1)
                do_extract(0)
                if stop_stage >= 3:
                    do_b3(0)
                do_b1(2); do_b2(2)
                do_extract(1)
                if stop_stage >= 3:
                    do_b3(1)
                do_b1(3); do_b2(3)
                do_extract(2)
                if stop_stage >= 3:
                    do_b3(2)
                do_extract(3)
                if stop_stage >= 3:
                    do_b3(3)

            # ------------- out-proj + out-LN + residual -------------
            y16 = [attnp.tile([128, N], BF16, tag=f"y16_{i}", name=f"y16_{i}") for i in range(2)]
            for ch in range(CH if stop_stage >= 4 else 0):
                sl = slice(ch * NCH, (ch + 1) * NCH)
                for i in range(2):
                    yps = psStat.tile([128, NCH], F32, tag="st")
                    for p in range(PAIRS):
                        nc.tensor.matmul(yps[:],
                                         lhsT=woutT[p][:, 128 * i:128 * (i + 1)],
                                         rhs=ao16[p][:, sl], start=(p == 0),
                                         stop=(p == 3))
                    nc.scalar.copy(y16[i][:, sl], yps[:])
            for ch in range(CH if stop_stage >= 4 else 0):
                sl = slice(ch * NCH, (ch + 1) * NCH)
                y2 = [finp.tile([128, NCH], BF16, tag="y2", name="y2", bufs=2) for _ in range(2)]
                for i in range(2):
                    nc.vector.tensor_mul(y2[i][:], y16[i][:, sl], y16[i][:, sl])
                S_ps = psStat.tile([128, NCH], F32, tag="st")
                nc.tensor.matmul(S_ps[:], lhsT=ones128_16[:], rhs=y16[0][:, sl],
                                 start=True, stop=False)
                nc.tensor.matmul(S_ps[:], lhsT=ones128_16[:], rhs=y16[1][:, sl],
                                 start=False, stop=True)
                Q_ps = psStat.tile([128, NCH], F32, tag="st")
                nc.tensor.matmul(Q_ps[:], lhsT=ones128_16[:], rhs=y2[0][:],
                                 start=True, stop=False)
                nc.tensor.matmul(Q_ps[:], lhsT=ones128_16[:], rhs=y2[1][:],
                                 start=False, stop=True)
                t_mu = finp.tile([128, NCH], F32, tag="ft_mu")
                nc.vector.tensor_scalar(t_mu[:], S_ps[:], 1.0 / C,
                                        scalar2=None, op0=OP.mult)
                t1 = finp.tile([128, NCH], F32, tag="fse", name="ft1")
                nc.scalar.activation(t1[:], t_mu[:], AF.Square)
                varb = finp.tile([128, NCH], F32, tag="fvarb")
                nc.vector.scalar_tensor_tensor(out=varb[:], in0=Q_ps[:], scalar=1.0 / C,
                                               in1=t1[:], op0=OP.mult, op1=OP.subtract)
                se = finp.tile([128, NCH], F32, tag="fse")
                nc.scalar.activation(se[:], varb[:], AF.Sqrt, bias=eps_c[:])
                rstd_b = finp.tile([128, NCH], F32, tag="frstd")
                nc.vector.reciprocal_approx_fast(out=rstd_b[:], in_=se[:])
                mu_b = finp.tile([128, NCH], F32, tag="fmu")
                nc.vector.tensor_mul(mu_b[:], t_mu[:], rstd_b[:])
                for i in range(2):
                    qs_t = finp.tile([128, NCH], F32, tag="qs_t")
                    nc.sync.dma_start(out=qs_t[:], in_=qs_d[128 * i:128 * (i + 1), sl])
                    t = finp.tile([128, NCH], F32, tag="fabc", name="fa", bufs=2)
                    nc.vector.tensor_tensor(out=t[:], in0=y16[i][:, sl],
                                            in1=rstd_b[:], op=OP.mult)
                    t2 = finp.tile([128, NCH], F32, tag="fabc", name="fb", bufs=2)
                    nc.vector.tensor_sub(t2[:], t[:], mu_b[:])
                    t3 = finp.tile([128, NCH], F32, tag="fabc", name="fc", bufs=2)
                    nc.scalar.activation(t3[:], t2[:], AF.Copy, scale=gg[i][:])
                    ot = finp.tile([128, NCH], F32, tag="fabc", name="fd", bufs=2)
                    nc.gpsimd.tensor_add(ot[:], t3[:], qs_t[:])
                    nc.sync.dma_start(out=out_d[128 * i:128 * (i + 1), sl], in_=ot[:])

            if stop_stage < 4:
                for i in range(2):
                    dummy = finp.tile([128, N], F32, tag="dummy")
                    nc.vector.memset(dummy[:], 0.0)
                    nc.sync.dma_start(out=out_d[128 * i:128 * (i + 1), :],
                                      in_=dummy[:])
    nc.finalize()
    return nc


_CACHE = {}


def kernel(**inputs):
    qsrc = np.asarray(inputs["query_source"], np.float32)
    ctx = np.asarray(inputs["context"], np.float32)
    cn_g = np.asarray(inputs["cn_g"], np.float32).reshape(C)
    cn_b = np.asarray(inputs["cn_b"], np.float32).reshape(C)
    qn_g = np.asarray(inputs["qn_g"], np.float32).reshape(C)
    qn_b = np.asarray(inputs["qn_b"], np.float32).reshape(C)
    on_g = np.asarray(inputs["on_g"], np.float32).reshape(C)
    on_b = np.asarray(inputs["on_b"], np.float32).reshape(C)
    w_kv = np.asarray(inputs["w_kv"], np.float32)
    w_q = np.asarray(inputs["w_q"], np.float32)
    w_out = np.asarray(inputs["w_out"], np.float32)
    gamma = float(np.asarray(inputs["gamma"], np.float32).reshape(()))

    assert np.abs(cn_b).max() == 0 and np.abs(qn_b).max() == 0 and \
        np.abs(on_b).max() == 0, "nonzero LN bias not implemented"

    import ml_dtypes
    bf16 = ml_dtypes.bfloat16
    wkvT = np.ascontiguousarray((w_kv * cn_g[None, :]).T).astype(np.float16)
    wqT = np.ascontiguousarray((w_q * qn_g[None, :]).T).astype(np.float16)
    woutT = np.ascontiguousarray(w_out.T).astype(bf16)
    gg = np.ascontiguousarray((gamma * on_g).reshape(C, 1), np.float32)

    p_idx = np.arange(128)
    identc = np.zeros((128, 64), np.float16)
    identc[p_idx, p_idx % 64] = 1.0
    onehot8c = (p_idx[:, None] % 8 == np.arange(8)[None, :]).astype(np.float32)
    m8ic = (((p_idx >> 3) & 1).astype(np.int32)).reshape(128, 1)
    zsel2c = (np.arange(128)[None, :] // 64 ==
              np.arange(2)[:, None]).astype(np.float32)

    if "nc" not in _CACHE:
        _CACHE["nc"] = build_program()
    nc = _CACHE["nc"]

    B = qsrc.shape[0]
    in_maps = []
    for b in range(B):
        in_maps.append({
            "ctx": np.ascontiguousarray(ctx[b].reshape(C, N)),
            "qsrc": np.ascontiguousarray(qsrc[b].reshape(C, N)),
            "wkvT": wkvT,
            "wqT": wqT,
            "woutT": woutT,
            "gg": gg,
            "identc": identc,
            "onehot8c": onehot8c,
            "m8ic": m8ic,
            "m8fc": m8ic.astype(np.float32),
            "zsel2c": zsel2c,
        })
    res = run_bass_kernel_spmd(nc, in_maps, core_ids=list(range(8)))
    outs = [np.asarray(r["out"], np.float32).reshape(1, C, 64, 64)
            for r in res.results]
    return np.concatenate(outs, axis=0)



# revision 31
# speedup vs baseline: 1.0857x; 1.0138x over previous
"""DPCA block (dual-pruned cross-attention) Trainium2 kernel.

Sharding: data-parallel over batch. B=8 -> 8 NeuronCores, one batch per core,
weights replicated, zero collectives.

Per-core dataflow (channel-major: channels on partitions, positions free):
 - chan-LN: gains folded into weights on host; per-position mu/rstd from PE
   ones-matmul broadcast-sums; x'' = (x-mu)*rstd stored bf16.
 - projections bf16 (f32 PSUM accumulate).
 - l2norm factors per head row; khat = k*rstd_b interleaved with v into a
   (khat,v) bf16 pair tensor so one gpsimd.ap_gather pulls both.
 - top-8 rows/cols via vector.max + max_index on f32 scores; 64 gathered
   (row,col) positions per head.
 - attention with head PAIRS block-diag packed on 128 partitions, keys on
   partitions, softmax Z via half-ones matvec, exp needs no max-subtract
   (|sim| <= 1 since khat,qhat l2-normalized).
 - out-proj bf16 + out-LN (same stats trick) + gamma*.. + residual in f32.
"""

import numpy as np

import concourse.bass as bass
import concourse.bacc as bacc
import concourse.mybir as mybir
from concourse.tile import TileContext
from concourse.bass_utils import run_bass_kernel_spmd

F32 = mybir.dt.float32
F32R = mybir.dt.float32r
BF16 = mybir.dt.bfloat16
F16 = mybir.dt.float16
I16 = mybir.dt.int16
I32 = mybir.dt.int32
U32 = mybir.dt.uint32
AX = mybir.AxisListType
OP = mybir.AluOpType
AF = mybir.ActivationFunctionType

C = 256
N = 4096
HEADS = 8
D = 64
PAIRS = 4
INNER = HEADS * D        # 512
NCH = 512
CH = N // NCH            # 8
KEYS = 64                # 8 rows x 8 cols kept per head
EPS = 1e-5


def build_program(stop_stage=99, sub=99):
    nc = bacc.Bacc()

    ctx_d = nc.declare_dram_parameter("ctx", [C, N], F32, False)
    qs_d = nc.declare_dram_parameter("qsrc", [C, N], F32, False)
    wkvT_d = nc.declare_dram_parameter("wkvT", [C, 2 * INNER], F16, False)
    wqT_d = nc.declare_dram_parameter("wqT", [C, INNER], F16, False)
    woutT_d = nc.declare_dram_parameter("woutT", [INNER, C], BF16, False)
    gg_d = nc.declare_dram_parameter("gg", [C, 1], F32, False)
    ident_d = nc.declare_dram_parameter("identc", [128, 64], F16, False)
    onehot8_d = nc.declare_dram_parameter("onehot8c", [128, 8], F32, False)
    m8i_d = nc.declare_dram_parameter("m8ic", [128, 1], I32, False)
    m8f_d = nc.declare_dram_parameter("m8fc", [128, 1], F32, False)
    zsel2_d = nc.declare_dram_parameter("zsel2c", [2, 128], F32, False)
    out_d = nc.declare_dram_parameter("out", [C, N], F32, True)

    with TileContext(nc) as tc:
        with (
            tc.tile_pool(name="const", bufs=1) as constp,
            tc.tile_pool(name="wpool", bufs=1) as wpool,
            tc.tile_pool(name="xin", bufs=2) as xin,
            tc.tile_pool(name="stat", bufs=1) as statp,
            tc.tile_pool(name="xpp", bufs=1) as xpp,
            tc.tile_pool(name="kvq", bufs=1) as kvqp,
            tc.tile_pool(name="pairs", bufs=2) as pairp,
            tc.tile_pool(name="sel", bufs=1) as selp,
            tc.tile_pool(name="attn", bufs=1) as attnp,
            tc.tile_pool(name="ptile", bufs=2) as ptp,
            tc.tile_pool(name="fin", bufs=1) as finp,
            tc.tile_pool(name="psStat", bufs=3, space="PSUM") as psStat,
            tc.tile_pool(name="psMain", bufs=4, space="PSUM") as psMain,
            tc.tile_pool(name="psSmall", bufs=1, space="PSUM") as psSmall,
        ):
            # ------------- constants -------------
            ones128 = constp.tile([128, 128], F32, tag="ones128")
            nc.vector.memset(ones128[:], 1.0)
            ones128_16 = constp.tile([128, 128], BF16, tag="ones128_16")
            nc.vector.memset(ones128_16[:], 1.0)
            ones128_f16 = constp.tile([128, 128], F16, tag="ones128_f16")
            nc.vector.memset(ones128_f16[:], 1.0)
            halves2 = constp.tile([128, 2], F32, tag="halves2")
            nc.vector.memset(halves2[:], 0.0)
            nc.vector.memset(halves2[0:64, 0:1], 1.0)
            nc.vector.memset(halves2[64:128, 1:2], 1.0)
            eps_c = constp.tile([128, 1], F32, tag="eps_c")
            nc.vector.memset(eps_c[:], EPS)
            halves2f = constp.tile([128, 2], F16, tag="halves2f")
            nc.vector.memset(halves2f[:], 0.0)
            nc.vector.memset(halves2f[0:64, 0:1], 1.0)
            nc.vector.memset(halves2f[64:128, 1:2], 1.0)
            ident16 = constp.tile([128, 64], F16, tag="ident16")
            nc.sync.dma_start(out=ident16[:], in_=ident_d[:])
            # block-diag ones: half-broadcast-sum stationary
            halvesbc16 = constp.tile([128, 128], F16, tag="halvesbc16")
            nc.vector.memset(halvesbc16[:], 0.0)
            nc.vector.memset(halvesbc16[0:64, 0:64], 1.0)
            nc.vector.memset(halvesbc16[64:128, 64:128], 1.0)
            zsel2 = constp.tile([2, 128], F32, tag="zsel2")
            nc.sync.dma_start(out=zsel2[:], in_=zsel2_d[:])
            onehot8 = constp.tile([128, 8], F32, tag="onehot8")
            nc.sync.dma_start(out=onehot8[:], in_=onehot8_d[:])
            m8f = constp.tile([128, 1], F32, tag="m8f")
            nc.sync.dma_start(out=m8f[:], in_=m8f_d[:])

            # ------------- weights -------------
            wkvT = [wpool.tile([128, 2 * INNER], F16, tag=f"wkvT{i}", name=f"wkvT{i}") for i in range(2)]
            wqT = [wpool.tile([128, INNER], F16, tag=f"wqT{i}", name=f"wqT{i}") for i in range(2)]
            for i in range(2):
                nc.sync.dma_start(out=wkvT[i][:], in_=wkvT_d[128 * i:128 * (i + 1), :])
                nc.sync.dma_start(out=wqT[i][:], in_=wqT_d[128 * i:128 * (i + 1), :])
            woutT = [wpool.tile([128, C], BF16, tag=f"woutT{i}", name=f"woutT{i}") for i in range(4)]
            for i in range(4):
                nc.sync.dma_start(out=woutT[i][:], in_=woutT_d[128 * i:128 * (i + 1), :])
            gg = [wpool.tile([128, 1], F32, tag=f"gg{i}", name=f"gg{i}") for i in range(2)]
            for i in range(2):
                nc.sync.dma_start(out=gg[i][:], in_=gg_d[128 * i:128 * (i + 1), :])

            # ------------- phase A: chan-LN -> x'' (f16) -------------
            # ctx gets the full (x-mu)*rstd (v needs it); qs gets only x-mu:
            # the q l2norm cancels any per-position scale, so rstd_qs (and the
            # whole variance pipeline) is unnecessary for the q path.
            xpp_t = {}
            for name, src in (("ctx", ctx_d), ("qs", qs_d)):
                xpp_t[name] = [xpp.tile([128, N], F16, tag=f"xpp_{name}{i}", name=f"xpp_{name}{i}")
                               for i in range(2)]
                for ch in range(CH):
                    sl = slice(ch * NCH, (ch + 1) * NCH)
                    xt = [xin.tile([128, NCH], F32, tag="xt", name="xt") for _ in range(2)]
                    for i in range(2):
                        nc.sync.dma_start(out=xt[i][:],
                                          in_=src[128 * i:128 * (i + 1), sl])
                    S_ps = psStat.tile([128, NCH], F32, tag="st")
                    nc.tensor.matmul(S_ps[:], lhsT=ones128[:], rhs=xt[0][:],
                                     start=True, stop=False)
                    nc.tensor.matmul(S_ps[:], lhsT=ones128[:], rhs=xt[1][:],
                                     start=False, stop=True)
                    t_mu = statp.tile([128, NCH], F32, tag="t_mu", bufs=2)
                    nc.vector.tensor_scalar(t_mu[:], S_ps[:], 1.0 / C,
                                            scalar2=None, op0=OP.mult)
                    if name == "qs":
                        for i in range(2):
                            nc.gpsimd.tensor_sub(xpp_t[name][i][:, sl],
                                                 xt[i][:], t_mu[:])
                        continue
                    xsq = [xin.tile([128, NCH], F16, tag="xsq", name="xsq") for _ in range(2)]
                    for i in range(2):
                        nc.scalar.activation(xsq[i][:], xt[i][:], AF.Square)
                    Q_ps = psStat.tile([128, NCH], F32, tag="st")
                    nc.tensor.matmul(Q_ps[:], lhsT=ones128_f16[:], rhs=xsq[0][:],
                                     start=True, stop=False)
                    nc.tensor.matmul(Q_ps[:], lhsT=ones128_f16[:], rhs=xsq[1][:],
                                     start=False, stop=True)
                    t1 = statp.tile([128, NCH], F32, tag="se", name="t1")
                    nc.scalar.activation(t1[:], t_mu[:], AF.Square)
                    varb = statp.tile([128, NCH], F32, tag="varb")
                    nc.vector.scalar_tensor_tensor(out=varb[:], in0=Q_ps[:],
                                                   scalar=1.0 / C, in1=t1[:],
                                                   op0=OP.mult, op1=OP.subtract)
                    se = statp.tile([128, NCH], F32, tag="se")
                    nc.scalar.activation(se[:], varb[:], AF.Sqrt, bias=eps_c[:])
                    rstd_b = statp.tile([128, NCH], F32, tag="rstd_b")
                    nc.vector.reciprocal_approx_fast(out=rstd_b[:], in_=se[:])
                    mu_b = statp.tile([128, NCH], F32, tag="mu_b")
                    nc.vector.tensor_mul(mu_b[:], t_mu[:], rstd_b[:])
                    for i in range(2):
                        tt = statp.tile([128, NCH], F32, tag="xr", bufs=2)
                        nc.vector.tensor_mul(tt[:], xt[i][:], rstd_b[:])
                        nc.gpsimd.tensor_sub(xpp_t[name][i][:, sl], tt[:], mu_b[:])

            # ------------- phase B: software-pipelined per head-pair -------
            # Emission order interleaves pairs so each pair's attention (which
            # waits on its gather) is emitted after the next pair's
            # projections: the in-order PE queue then never stalls on a
            # gather.
            ao16 = [attnp.tile([128, N], BF16, tag=f"ao{p}", name=f"ao{p}")
                    for p in range(PAIRS)]
            il_t, qh_t, ksel_t, kbd_t, vbd_t = {}, {}, {}, {}, {}

            def do_b1(p):
                il = kvqp.tile([128, 2 * N], F16, tag="il", bufs=2, name=f"il{p}")
                qh = kvqp.tile([128, N], F16, tag="qh", bufs=2, name=f"qh{p}")
                il_t[p], qh_t[p] = il, qh
                for ch in range(CH):
                    sl = slice(ch * NCH, (ch + 1) * NCH)
                    # --- projections (k, v, q) for this chunk ---
                    kps = psMain.tile([128, NCH], F32, tag="m")
                    nc.tensor.matmul(kps[:], lhsT=wkvT[0][:, 128 * p:128 * (p + 1)],
                                     rhs=xpp_t["ctx"][0][:, sl], start=True, stop=False)
                    nc.tensor.matmul(kps[:], lhsT=wkvT[1][:, 128 * p:128 * (p + 1)],
                                     rhs=xpp_t["ctx"][1][:, sl], start=False, stop=True)
                    k16c = kvqp.tile([128, NCH], F16, tag="k16c", bufs=2)
                    nc.scalar.copy(k16c[:], kps[:])
                    vps = psMain.tile([128, NCH], F32, tag="m")
                    vo = INNER + 128 * p
                    nc.tensor.matmul(vps[:], lhsT=wkvT[0][:, vo:vo + 128],
                                     rhs=xpp_t["ctx"][0][:, sl], start=True, stop=False)
                    nc.tensor.matmul(vps[:], lhsT=wkvT[1][:, vo:vo + 128],
                                     rhs=xpp_t["ctx"][1][:, sl], start=False, stop=True)
                    nc.scalar.copy(il[:, 2 * sl.start + 1:2 * sl.stop:2], vps[:])
                    qps = psMain.tile([128, NCH], F32, tag="m")
                    nc.tensor.matmul(qps[:], lhsT=wqT[0][:, 128 * p:128 * (p + 1)],
                                     rhs=xpp_t["qs"][0][:, sl], start=True, stop=False)
                    nc.tensor.matmul(qps[:], lhsT=wqT[1][:, 128 * p:128 * (p + 1)],
                                     rhs=xpp_t["qs"][1][:, sl], start=False, stop=True)
                    q16c = kvqp.tile([128, NCH], F16, tag="q16c", bufs=2)
                    nc.scalar.copy(q16c[:], qps[:])
                    # --- l2 factors + khat/qhat ---
                    k2c = kvqp.tile([128, NCH], F16, tag="k2c", bufs=2)
                    nc.vector.tensor_mul(k2c[:], k16c[:], k16c[:])
                    q2c = kvqp.tile([128, NCH], F16, tag="q2c", bufs=2)
                    nc.vector.tensor_mul(q2c[:], q16c[:], q16c[:])
                    rkps = psMain.tile([128, NCH], F32, tag="m")
                    nc.tensor.matmul(rkps[:], lhsT=halvesbc16[:], rhs=k2c[:],
                                     start=True, stop=True)
                    sek = statp.tile([128, NCH], F32, tag="se_", bufs=2)
                    nc.scalar.activation(sek[:], rkps[:], AF.Sqrt)
                    rbk = statp.tile([128, NCH], F32, tag="rb_", bufs=2)
                    nc.vector.reciprocal_approx_fast(out=rbk[:], in_=sek[:])
                    nc.gpsimd.tensor_tensor(out=il[:, 2 * sl.start:2 * sl.stop:2],
                                            in0=k16c[:], in1=rbk[:], op=OP.mult)
                    rqps = psMain.tile([128, NCH], F32, tag="m")
                    nc.tensor.matmul(rqps[:], lhsT=halvesbc16[:], rhs=q2c[:],
                                     start=True, stop=True)
                    seq2 = statp.tile([128, NCH], F32, tag="se_", bufs=2)
                    nc.scalar.activation(seq2[:], rqps[:], AF.Sqrt)
                    rbq = statp.tile([128, NCH], F32, tag="rb_", bufs=2)
                    nc.vector.reciprocal_approx_fast(out=rbq[:], in_=seq2[:])
                    nc.vector.tensor_tensor(out=qh[:, sl], in0=q16c[:],
                                            in1=rbq[:], op=OP.mult)

            def do_b2(p):
                il, qh = il_t[p], qh_t[p]
                # --- segmented |khat| sums + q_probe + scores + topk ---
                il4 = il[:].rearrange("p (h w d) -> p h w d", h=64, w=64, d=2)
                kabs_r = pairp.tile([128, 64], F32, tag="kabsr")
                nc.vector.tensor_reduce(out=kabs_r[:], in_=il4[:, :, :, 0],
                                        axis=AX.X, op=OP.add, apply_absolute_value=True)
                il4c = il[:].rearrange("p (h w d) -> p w h d", h=64, w=64, d=2)
                kabs_c = pairp.tile([128, 64], F32, tag="kabsc")
                nc.vector.tensor_reduce(out=kabs_c[:], in_=il4c[:, :, :, 0],
                                        axis=AX.X, op=OP.add, apply_absolute_value=True)
                qp = pairp.tile([128, 1], F32, tag="qp")
                nc.vector.tensor_reduce(out=qp[:], in_=qh[:], axis=AX.X, op=OP.add)
                qp2 = pairp.tile([128, 2], F32, tag="qp2")
                nc.vector.memset(qp2[:], 0.0)
                nc.vector.tensor_copy(out=qp2[0:64, 0:1], in_=qp[0:64, :])
                nc.vector.tensor_copy(out=qp2[64:128, 1:2], in_=qp[64:128, :])
                sc_r = pairp.tile([2, 64], F32, tag="scr")
                sc_ps = psSmall.tile([2, 64], F32, tag="s")
                nc.tensor.matmul(sc_ps[:], lhsT=qp2[:], rhs=kabs_r[:],
                                 start=True, stop=True)
                nc.scalar.copy(sc_r[:], sc_ps[:])
                sc_c = pairp.tile([2, 64], F32, tag="scc")
                sc_ps2 = psSmall.tile([2, 64], F32, tag="s")
                nc.tensor.matmul(sc_ps2[:], lhsT=qp2[:], rhs=kabs_c[:],
                                 start=True, stop=True)
                nc.scalar.copy(sc_c[:], sc_ps2[:])
                mx = pairp.tile([2, 8], F32, tag="mx")
                idx_r = pairp.tile([2, 8], U32, tag="idxr")
                nc.vector.max(out=mx[:], in_=sc_r[:])
                nc.vector.max_index(out=idx_r[:], in_max=mx[:], in_values=sc_r[:])
                mxc = pairp.tile([2, 8], F32, tag="mxc")
                idx_c = pairp.tile([2, 8], U32, tag="idxc")
                nc.vector.max(out=mxc[:], in_=sc_c[:])
                nc.vector.max_index(out=idx_c[:], in_max=mxc[:], in_values=sc_c[:])
                idxr_f = pairp.tile([2, 8], F32, tag="idxrf")
                nc.vector.tensor_copy(out=idxr_f[:], in_=idx_r[:])
                idxc_f = pairp.tile([2, 8], F32, tag="idxcf")
                nc.vector.tensor_copy(out=idxc_f[:], in_=idx_c[:])
                # broadcast idx rows to all partitions by head half
                rbc_ps = psSmall.tile([128, 8], F32, tag="s")
                nc.tensor.matmul(rbc_ps[:], lhsT=zsel2[:], rhs=idxr_f[:],
                                 start=True, stop=True)
                rbc = pairp.tile([128, 8], F32, tag="rbc")
                nc.scalar.copy(rbc[:], rbc_ps[:])
                cbc_ps = psSmall.tile([128, 8], F32, tag="s")
                nc.tensor.matmul(cbc_ps[:], lhsT=zsel2[:], rhs=idxc_f[:],
                                 start=True, stop=True)
                cbc = pairp.tile([128, 8], F32, tag="cbc")
                nc.scalar.copy(cbc[:], cbc_ps[:])
                # Bcol[p] = idx_c[h(p), p%8]
                junk8 = pairp.tile([128, 8], F32, tag="junk8")
                nc.vector.tensor_mul(junk8[:], cbc[:], onehot8[:])
                Bcol = pairp.tile([128, 1], F32, tag="Bcol")
                nc.vector.tensor_reduce(out=Bcol[:], in_=junk8[:], axis=AX.X,
                                        op=OP.add)
                # wr[p, s] = idx_r[h(p), 2s + ((p>>3)&1)]
                wdiff = pairp.tile([128, 4], F32, tag="wdiff")
                nc.vector.tensor_sub(wdiff[:], rbc[:, 1:8:2], rbc[:, 0:8:2])
                wsel = pairp.tile([128, 4], F32, tag="wsel")
                nc.vector.tensor_scalar(wsel[:], wdiff[:], m8f[:], scalar2=None,
                                        op0=OP.mult)
                wr = pairp.tile([128, 4], F32, tag="wr")
                nc.vector.tensor_add(wr[:], wsel[:], rbc[:, 0:8:2])
                posfw = pairp.tile([128, 4], F32, tag="posfw")
                nc.vector.scalar_tensor_tensor(out=posfw[:], in0=wr[:], scalar=64.0,
                                               in1=Bcol[:].to_broadcast([128, 4]),
                                               op0=OP.mult, op1=OP.add)
                widx32 = pairp.tile([128, 4], I32, tag="widx32")
                nc.vector.tensor_copy(out=widx32[:], in_=posfw[:])
                widx = pairp.tile([128, 4], I16, tag="widx")
                nc.vector.tensor_copy(out=widx[:], in_=widx32[:])
                # --- gather ---
                ksel_il = selp.tile([128, 128], F16, tag="kselil", bufs=2,
                                    name=f"ksel{p}")
                nc.gpsimd.ap_gather(
                    out_ap=ksel_il[:].rearrange("p (k d) -> p k d", d=2),
                    in_ap=il[:].rearrange("p (n d) -> p n d", d=2),
                    idxs_ap=widx[:],
                    channels=128, num_elems=N, d=2, num_idxs=KEYS)
                ksel_t[p] = ksel_il

            def do_extract(p):
                ksel_il = ksel_t[p]
                kbd = selp.tile([128, 128], F16, tag="kbd", bufs=2, name=f"kbd{p}")
                nc.vector.memset(kbd[:], 0.0)
                nc.vector.tensor_copy(out=kbd[0:64, 0:64], in_=ksel_il[0:64, 0:128:2])
                nc.vector.tensor_copy(out=kbd[64:128, 64:128],
                                      in_=ksel_il[64:128, 0:128:2])
                vbd = selp.tile([128, 128], F16, tag="vbd", bufs=2, name=f"vbd{p}")
                nc.vector.memset(vbd[:], 0.0)
                for h in range(2):
                    o = 64 * h
                    tps = psSmall.tile([64, 64], F16, tag="s")
                    nc.tensor.transpose(out=tps[:], in_=ksel_il[o:o + 64, 1:128:2],
                                        identity=ident16[o:o + 64, :])
                    nc.scalar.copy(vbd[o:o + 64, o:o + 64], tps[:])
                kbd_t[p], vbd_t[p] = kbd, vbd

            def do_b3(p):
                kbd, vbd, qh = kbd_t[p], vbd_t[p], qh_t[p]
                # --- attention for this pair ---
                for ch in range(CH):
                    sl = slice(ch * NCH, (ch + 1) * NCH)
                    sps = psMain.tile([128, NCH], F32, tag="m")
                    nc.tensor.matmul(sps[:], lhsT=kbd[:], rhs=qh[:, sl],
                                     start=True, stop=True)
                    pt = ptp.tile([128, NCH], F16, tag="pT")
                    nc.scalar.activation(pt[:], sps[:], AF.Exp)
                    zps = psSmall.tile([2, NCH], F32, tag="s")
                    nc.tensor.matmul(zps[:], lhsT=halves2f[:], rhs=pt[:],
                                     start=True, stop=True)
                    zinv = ptp.tile([2, NCH], F32, tag="zinv")
                    nc.vector.reciprocal_approx_fast(out=zinv[:], in_=zps[:])
                    zb = psMain.tile([128, NCH], F32, tag="m")
                    nc.tensor.matmul(zb[:], lhsT=zsel2[:], rhs=zinv[:],
                                     start=True, stop=True)
                    ph16 = ptp.tile([128, NCH], F16, tag="ph16")
                    nc.vector.tensor_tensor(out=ph16[:], in0=pt[:], in1=zb[:],
                                            op=OP.mult)
                    pvs = psMain.tile([128, NCH], F32, tag="m")
                    nc.tensor.matmul(pvs[:], lhsT=vbd[:], rhs=ph16[:],
                                     start=True, stop=True)
                    nc.scalar.copy(ao16[p][:, sl], pvs[:])

            if stop_stage >= 2:
                do_b1(0); do_b2(0)
                do_b1(1); do_b2(1)
                do_extract(0)
                if stop_stage >= 3:
                    do_b3(0)
                do_b1(2); do_b2(2)
                do_extract(1)
                if stop_stage >= 3:
                    do_b3(1)
                do_b1(3); do_b2(3)
                do_extract(2)
                if stop_stage >= 3:
                    do_b3(2)
                do_extract(3)
                if stop_stage >= 3:
                    do_b3(3)

            # ------------- out-proj + out-LN + residual -------------
            y16 = [attnp.tile([128, N], BF16, tag=f"y16_{i}", name=f"y16_{i}") for i in range(2)]
            for ch in range(CH if stop_stage >= 4 else 0):
                sl = slice(ch * NCH, (ch + 1) * NCH)
                for i in range(2):
                    yps = psStat.tile([128, NCH], F32, tag="st")
                    for p in range(PAIRS):
                        nc.tensor.matmul(yps[:],
                                         lhsT=woutT[p][:, 128 * i:128 * (i + 1)],
                                         rhs=ao16[p][:, sl], start=(p == 0),
                                         stop=(p == 3))
                    nc.scalar.copy(y16[i][:, sl], yps[:])
            for ch in range(CH if stop_stage >= 4 else 0):
                sl = slice(ch * NCH, (ch + 1) * NCH)
                y2 = [finp.tile([128, NCH], BF16, tag="y2", name="y2", bufs=2) for _ in range(2)]
                for i in range(2):
                    nc.vector.tensor_mul(y2[i][:], y16[i][:, sl], y16[i][:, sl])
                S_ps = psStat.tile([128, NCH], F32, tag="st")
                nc.tensor.matmul(S_ps[:], lhsT=ones128_16[:], rhs=y16[0][:, sl],
                                 start=True, stop=False)
                nc.tensor.matmul(S_ps[:], lhsT=ones128_16[:], rhs=y16[1][:, sl],
                                 start=False, stop=True)
                Q_ps = psStat.tile([128, NCH], F32, tag="st")
                nc.tensor.matmul(Q_ps[:], lhsT=ones128_16[:], rhs=y2[0][:],
                                 start=True, stop=False)
                nc.tensor.matmul(Q_ps[:], lhsT=ones128_16[:], rhs=y2[1][:],
                                 start=False, stop=True)
                t_mu = finp.tile([128, NCH], F32, tag="ft_mu")
                nc.vector.tensor_scalar(t_mu[:], S_ps[:], 1.0 / C,
                                        scalar2=None, op0=OP.mult)
                t1 = finp.tile([128, NCH], F32, tag="fse", name="ft1")
                nc.scalar.activation(t1[:], t_mu[:], AF.Square)
                varb = finp.tile([128, NCH], F32, tag="fvarb")
                nc.vector.scalar_tensor_tensor(out=varb[:], in0=Q_ps[:], scalar=1.0 / C,
                                               in1=t1[:], op0=OP.mult, op1=OP.subtract)
                se = finp.tile([128, NCH], F32, tag="fse")
                nc.scalar.activation(se[:], varb[:], AF.Sqrt, bias=eps_c[:])
                rstd_b = finp.tile([128, NCH], F32, tag="frstd")
                nc.vector.reciprocal_approx_fast(out=rstd_b[:], in_=se[:])
                mu_b = finp.tile([128, NCH], F32, tag="fmu")
                nc.vector.tensor_mul(mu_b[:], t_mu[:], rstd_b[:])
                for i in range(2):
                    qs_t = finp.tile([128, NCH], F32, tag="qs_t")
                    nc.sync.dma_start(out=qs_t[:], in_=qs_d[128 * i:128 * (i + 1), sl])
                    t = finp.tile([128, NCH], F32, tag="fabc", name="fa", bufs=2)
                    nc.vector.tensor_tensor(out=t[:], in0=y16[i][:, sl],
                                            in1=rstd_b[:], op=OP.mult)
                    t2 = finp.tile([128, NCH], F32, tag="fabc", name="fb", bufs=2)
                    nc.vector.tensor_sub(t2[:], t[:], mu_b[:])
                    t3 = finp.tile([128, NCH], F32, tag="fabc", name="fc", bufs=2)
                    nc.scalar.activation(t3[:], t2[:], AF.Copy, scale=gg[i][:])
                    ot = finp.tile([128, NCH], F32, tag="fabc", name="fd", bufs=2)
                    nc.gpsimd.tensor_add(ot[:], t3[:], qs_t[:])
                    nc.sync.dma_start(out=out_d[128 * i:128 * (i + 1), sl], in_=ot[:])

            if stop_stage < 4:
                for i in range(2):
                    dummy = finp.tile([128, N], F32, tag="dummy")
                    nc.vector.memset(dummy[:], 0.0)
                    nc.sync.dma_start(out=out_d[128 * i:128 * (i + 1), :],
                                      in_=dummy[:])
    nc.finalize()
    return nc


_CACHE = {}


def kernel(**inputs):
    qsrc = np.asarray(inputs["query_source"], np.float32)
    ctx = np.asarray(inputs["context"], np.float32)
    cn_g = np.asarray(inputs["cn_g"], np.float32).reshape(C)
    cn_b = np.asarray(inputs["cn_b"], np.float32).reshape(C)
    qn_g = np.asarray(inputs["qn_g"], np.float32).reshape(C)
    qn_b = np.asarray(inputs["qn_b"], np.float32).reshape(C)
    on_g = np.asarray(inputs["on_g"], np.float32).reshape(C)
    on_b = np.asarray(inputs["on_b"], np.float32).reshape(C)
    w_kv = np.asarray(inputs["w_kv"], np.float32)
    w_q = np.asarray(inputs["w_q"], np.float32)
    w_out = np.asarray(inputs["w_out"], np.float32)
    gamma = float(np.asarray(inputs["gamma"], np.float32).reshape(()))

    assert np.abs(cn_b).max() == 0 and np.abs(qn_b).max() == 0 and \
        np.abs(on_b).max() == 0, "nonzero LN bias not implemented"

    import ml_dtypes
    bf16 = ml_dtypes.bfloat16
    wkvT = np.ascontiguousarray((w_kv * cn_g[None, :]).T).astype(np.float16)
    wqT = np.ascontiguousarray((w_q * qn_g[None, :]).T).astype(np.float16)
    woutT = np.ascontiguousarray(w_out.T).astype(bf16)
    gg = np.ascontiguousarray((gamma * on_g).reshape(C, 1), np.float32)

    p_idx = np.arange(128)
    identc = np.zeros((128, 64), np.float16)
    identc[p_idx, p_idx % 64] = 1.0
    onehot8c = (p_idx[:, None] % 8 == np.arange(8)[None, :]).astype(np.float32)
    m8ic = (((p_idx >> 3) & 1).astype(np.int32)).reshape(128, 1)
    zsel2c = (np.arange(128)[None, :] // 64 ==
              np.arange(2)[:, None]).astype(np.float32)

    if "nc" not in _CACHE:
        _CACHE["nc"] = build_program()
    nc = _CACHE["nc"]

    B = qsrc.shape[0]
    in_maps = []
    for b in range(B):
        in_maps.append({
            "ctx": np.ascontiguousarray(ctx[b].reshape(C, N)),
            "qsrc": np.ascontiguousarray(qsrc[b].reshape(C, N)),
            "wkvT": wkvT,
            "wqT": wqT,
            "woutT": woutT,
            "gg": gg,
            "identc": identc,
            "onehot8c": onehot8c,
            "m8ic": m8ic,
            "m8fc": m8ic.astype(np.float32),
            "zsel2c": zsel2c,
        })
    res = run_bass_kernel_spmd(nc, in_maps, core_ids=list(range(8)))
    outs = [np.asarray(r["out"], np.float32).reshape(1, C, 64, 64)
            for r in res.results]
    return np.concatenate(outs, axis=0)



# revision 33
# speedup vs baseline: 1.2562x; 1.1570x over previous
"""DPCA block (dual-pruned cross-attention) Trainium2 kernel.

Sharding: data-parallel over batch. B=8 -> 8 NeuronCores, one batch per core,
weights replicated, zero collectives.

Per-core dataflow (channel-major: channels on partitions, positions free):
 - chan-LN: gains folded into weights on host; per-position mu/rstd from PE
   ones-matmul broadcast-sums; x'' = (x-mu)*rstd stored bf16.
 - projections bf16 (f32 PSUM accumulate).
 - l2norm factors per head row; khat = k*rstd_b interleaved with v into a
   (khat,v) bf16 pair tensor so one gpsimd.ap_gather pulls both.
 - top-8 rows/cols via vector.max + max_index on f32 scores; 64 gathered
   (row,col) positions per head.
 - attention with head PAIRS block-diag packed on 128 partitions, keys on
   partitions, softmax Z via half-ones matvec, exp needs no max-subtract
   (|sim| <= 1 since khat,qhat l2-normalized).
 - out-proj bf16 + out-LN (same stats trick) + gamma*.. + residual in f32.
"""

import numpy as np

import concourse.bass as bass
import concourse.bacc as bacc
import concourse.mybir as mybir
from concourse.tile import TileContext
from concourse.bass_utils import run_bass_kernel_spmd

F32 = mybir.dt.float32
F32R = mybir.dt.float32r
BF16 = mybir.dt.bfloat16
F16 = mybir.dt.float16
I16 = mybir.dt.int16
I32 = mybir.dt.int32
U32 = mybir.dt.uint32
AX = mybir.AxisListType
OP = mybir.AluOpType
AF = mybir.ActivationFunctionType

C = 256
N = 4096
HEADS = 8
D = 64
PAIRS = 4
INNER = HEADS * D        # 512
NCH = 512
CH = N // NCH            # 8
KEYS = 64                # 8 rows x 8 cols kept per head
EPS = 1e-5


def build_program(stop_stage=99, sub=99):
    nc = bacc.Bacc()

    ctx_d = nc.declare_dram_parameter("ctx", [C, N], F32, False)
    qs_d = nc.declare_dram_parameter("qsrc", [C, N], F32, False)
    wkvT_d = nc.declare_dram_parameter("wkvT", [C, 2 * INNER], F16, False)
    wqT_d = nc.declare_dram_parameter("wqT", [C, INNER], F16, False)
    woutT_d = nc.declare_dram_parameter("woutT", [INNER, C], BF16, False)
    gg_d = nc.declare_dram_parameter("gg", [C, 1], F32, False)
    ident_d = nc.declare_dram_parameter("identc", [128, 64], F16, False)
    onehot8_d = nc.declare_dram_parameter("onehot8c", [128, 8], F32, False)
    m8i_d = nc.declare_dram_parameter("m8ic", [128, 1], I32, False)
    m8f_d = nc.declare_dram_parameter("m8fc", [128, 1], F32, False)
    zsel2_d = nc.declare_dram_parameter("zsel2c", [2, 128], F32, False)
    out_d = nc.declare_dram_parameter("out", [C, N], F32, True)

    with TileContext(nc) as tc:
        with (
            tc.tile_pool(name="const", bufs=1) as constp,
            tc.tile_pool(name="wpool", bufs=1) as wpool,
            tc.tile_pool(name="xin", bufs=2) as xin,
            tc.tile_pool(name="stat", bufs=1) as statp,
            tc.tile_pool(name="xpp", bufs=1) as xpp,
            tc.tile_pool(name="kvq", bufs=1) as kvqp,
            tc.tile_pool(name="pairs", bufs=2) as pairp,
            tc.tile_pool(name="sel", bufs=1) as selp,
            tc.tile_pool(name="attn", bufs=1) as attnp,
            tc.tile_pool(name="ptile", bufs=2) as ptp,
            tc.tile_pool(name="fin", bufs=1) as finp,
            tc.tile_pool(name="psStat", bufs=3, space="PSUM") as psStat,
            tc.tile_pool(name="psMain", bufs=4, space="PSUM") as psMain,
            tc.tile_pool(name="psSmall", bufs=1, space="PSUM") as psSmall,
        ):
            # ------------- constants -------------
            ones128 = constp.tile([128, 128], F32, tag="ones128")
            nc.vector.memset(ones128[:], 1.0)
            ones128_16 = constp.tile([128, 128], BF16, tag="ones128_16")
            nc.vector.memset(ones128_16[:], 1.0)
            ones128_f16 = constp.tile([128, 128], F16, tag="ones128_f16")
            nc.vector.memset(ones128_f16[:], 1.0)
            halves2 = constp.tile([128, 2], F32, tag="halves2")
            nc.vector.memset(halves2[:], 0.0)
            nc.vector.memset(halves2[0:64, 0:1], 1.0)
            nc.vector.memset(halves2[64:128, 1:2], 1.0)
            eps_c = constp.tile([128, 1], F32, tag="eps_c")
            nc.vector.memset(eps_c[:], EPS)
            halves2f = constp.tile([128, 2], F16, tag="halves2f")
            nc.vector.memset(halves2f[:], 0.0)
            nc.vector.memset(halves2f[0:64, 0:1], 1.0)
            nc.vector.memset(halves2f[64:128, 1:2], 1.0)
            ident16 = constp.tile([128, 64], F16, tag="ident16")
            nc.sync.dma_start(out=ident16[:], in_=ident_d[:])
            # block-diag ones: half-broadcast-sum stationary
            halvesbc16 = constp.tile([128, 128], F16, tag="halvesbc16")
            nc.vector.memset(halvesbc16[:], 0.0)
            nc.vector.memset(halvesbc16[0:64, 0:64], 1.0)
            nc.vector.memset(halvesbc16[64:128, 64:128], 1.0)
            zsel2 = constp.tile([2, 128], F32, tag="zsel2")
            nc.sync.dma_start(out=zsel2[:], in_=zsel2_d[:])
            onehot8 = constp.tile([128, 8], F32, tag="onehot8")
            nc.sync.dma_start(out=onehot8[:], in_=onehot8_d[:])
            m8f = constp.tile([128, 1], F32, tag="m8f")
            nc.sync.dma_start(out=m8f[:], in_=m8f_d[:])

            # ------------- weights -------------
            wkvT = [wpool.tile([128, 2 * INNER], F16, tag=f"wkvT{i}", name=f"wkvT{i}") for i in range(2)]
            wqT = [wpool.tile([128, INNER], F16, tag=f"wqT{i}", name=f"wqT{i}") for i in range(2)]
            for i in range(2):
                nc.sync.dma_start(out=wkvT[i][:], in_=wkvT_d[128 * i:128 * (i + 1), :])
                nc.sync.dma_start(out=wqT[i][:], in_=wqT_d[128 * i:128 * (i + 1), :])
            woutT = [wpool.tile([128, C], BF16, tag=f"woutT{i}", name=f"woutT{i}") for i in range(4)]
            for i in range(4):
                nc.sync.dma_start(out=woutT[i][:], in_=woutT_d[128 * i:128 * (i + 1), :])
            gg = [wpool.tile([128, 1], F32, tag=f"gg{i}", name=f"gg{i}") for i in range(2)]
            for i in range(2):
                nc.sync.dma_start(out=gg[i][:], in_=gg_d[128 * i:128 * (i + 1), :])

            # ------------- phase A: chan-LN -> x'' (f16) -------------
            # ctx gets the full (x-mu)*rstd (v needs it); qs gets only x-mu:
            # the q l2norm cancels any per-position scale, so rstd_qs (and the
            # whole variance pipeline) is unnecessary for the q path.
            # ctx and qs chunks are interleaved as two independent dependency
            # chains (qs stats borrow the otherwise-idle psMain banks).
            xpp_t = {}
            for name in ("ctx", "qs"):
                xpp_t[name] = [xpp.tile([128, N], F16, tag=f"xpp_{name}{i}",
                                        name=f"xpp_{name}{i}")
                               for i in range(2)]
            for ch in range(CH):
                sl = slice(ch * NCH, (ch + 1) * NCH)
                # --- ctx chunk: full LN ---
                xt = [xin.tile([128, NCH], F32, tag="xt", name="xt") for _ in range(2)]
                for i in range(2):
                    nc.sync.dma_start(out=xt[i][:],
                                      in_=ctx_d[128 * i:128 * (i + 1), sl])
                S_ps = psStat.tile([128, NCH], F32, tag="st")
                nc.tensor.matmul(S_ps[:], lhsT=ones128[:], rhs=xt[0][:],
                                 start=True, stop=False)
                nc.tensor.matmul(S_ps[:], lhsT=ones128[:], rhs=xt[1][:],
                                 start=False, stop=True)
                t_mu = statp.tile([128, NCH], F32, tag="t_mu", bufs=2)
                nc.vector.tensor_scalar(t_mu[:], S_ps[:], 1.0 / C,
                                        scalar2=None, op0=OP.mult)
                xsq = [xin.tile([128, NCH], F16, tag="xsq", name="xsq", bufs=1)
                       for _ in range(2)]
                for i in range(2):
                    nc.scalar.activation(xsq[i][:], xt[i][:], AF.Square)
                Q_ps = psStat.tile([128, NCH], F32, tag="st")
                nc.tensor.matmul(Q_ps[:], lhsT=ones128_f16[:], rhs=xsq[0][:],
                                 start=True, stop=False)
                nc.tensor.matmul(Q_ps[:], lhsT=ones128_f16[:], rhs=xsq[1][:],
                                 start=False, stop=True)
                t1 = statp.tile([128, NCH], F32, tag="se", name="t1")
                nc.scalar.activation(t1[:], t_mu[:], AF.Square)
                varb = statp.tile([128, NCH], F32, tag="varb")
                nc.vector.scalar_tensor_tensor(out=varb[:], in0=Q_ps[:],
                                               scalar=1.0 / C, in1=t1[:],
                                               op0=OP.mult, op1=OP.subtract)
                se = statp.tile([128, NCH], F32, tag="se")
                nc.scalar.activation(se[:], varb[:], AF.Sqrt, bias=eps_c[:])
                rstd_b = statp.tile([128, NCH], F32, tag="rstd_b")
                nc.vector.reciprocal_approx_fast(out=rstd_b[:], in_=se[:])
                for i in range(2):
                    xc = statp.tile([128, NCH], F32, tag="xr", bufs=2)
                    nc.gpsimd.tensor_sub(xc[:], xt[i][:], t_mu[:])
                    nc.vector.tensor_tensor(out=xpp_t["ctx"][i][:, sl],
                                            in0=xc[:], in1=rstd_b[:], op=OP.mult)
                # --- qs chunk: mean-subtract only (independent chain) ---
                xtq = [xin.tile([128, NCH], F32, tag="xtq", name="xtq")
                       for _ in range(2)]
                for i in range(2):
                    nc.sync.dma_start(out=xtq[i][:],
                                      in_=qs_d[128 * i:128 * (i + 1), sl])
                Sq_ps = psMain.tile([128, NCH], F32, tag="m")
                nc.tensor.matmul(Sq_ps[:], lhsT=ones128[:], rhs=xtq[0][:],
                                 start=True, stop=False)
                nc.tensor.matmul(Sq_ps[:], lhsT=ones128[:], rhs=xtq[1][:],
                                 start=False, stop=True)
                t_muq = statp.tile([128, NCH], F32, tag="t_muq", bufs=1)
                nc.vector.tensor_scalar(t_muq[:], Sq_ps[:], 1.0 / C,
                                        scalar2=None, op0=OP.mult)
                for i in range(2):
                    nc.gpsimd.tensor_sub(xpp_t["qs"][i][:, sl],
                                         xtq[i][:], t_muq[:])

            # ------------- phase B: software-pipelined per head-pair -------
            # Emission order interleaves pairs so each pair's attention (which
            # waits on its gather) is emitted after the next pair's
            # projections: the in-order PE queue then never stalls on a
            # gather.
            ao16 = [attnp.tile([128, N], BF16, tag=f"ao{p}", name=f"ao{p}")
                    for p in range(PAIRS)]
            il_t, qh_t, ksel_t, kbd_t, vbd_t = {}, {}, {}, {}, {}

            def do_b1(p):
                il = kvqp.tile([128, 2 * N], F16, tag="il", bufs=2, name=f"il{p}")
                qh = kvqp.tile([128, N], F16, tag="qh", bufs=2, name=f"qh{p}")
                il_t[p], qh_t[p] = il, qh
                for ch in range(CH):
                    sl = slice(ch * NCH, (ch + 1) * NCH)
                    # --- projections (k, v, q) for this chunk ---
                    kps = psMain.tile([128, NCH], F32, tag="m")
                    nc.tensor.matmul(kps[:], lhsT=wkvT[0][:, 128 * p:128 * (p + 1)],
                                     rhs=xpp_t["ctx"][0][:, sl], start=True, stop=False)
                    nc.tensor.matmul(kps[:], lhsT=wkvT[1][:, 128 * p:128 * (p + 1)],
                                     rhs=xpp_t["ctx"][1][:, sl], start=False, stop=True)
                    k16c = kvqp.tile([128, NCH], F16, tag="k16c", bufs=2)
                    nc.scalar.copy(k16c[:], kps[:])
                    vps = psMain.tile([128, NCH], F32, tag="m")
                    vo = INNER + 128 * p
                    nc.tensor.matmul(vps[:], lhsT=wkvT[0][:, vo:vo + 128],
                                     rhs=xpp_t["ctx"][0][:, sl], start=True, stop=False)
                    nc.tensor.matmul(vps[:], lhsT=wkvT[1][:, vo:vo + 128],
                                     rhs=xpp_t["ctx"][1][:, sl], start=False, stop=True)
                    nc.scalar.copy(il[:, 2 * sl.start + 1:2 * sl.stop:2], vps[:])
                    qps = psMain.tile([128, NCH], F32, tag="m")
                    nc.tensor.matmul(qps[:], lhsT=wqT[0][:, 128 * p:128 * (p + 1)],
                                     rhs=xpp_t["qs"][0][:, sl], start=True, stop=False)
                    nc.tensor.matmul(qps[:], lhsT=wqT[1][:, 128 * p:128 * (p + 1)],
                                     rhs=xpp_t["qs"][1][:, sl], start=False, stop=True)
                    q16c = kvqp.tile([128, NCH], F16, tag="q16c", bufs=2)
                    nc.scalar.copy(q16c[:], qps[:])
                    # --- l2 factors + khat/qhat ---
                    k2c = kvqp.tile([128, NCH], F16, tag="k2c", bufs=1)
                    nc.vector.tensor_mul(k2c[:], k16c[:], k16c[:])
                    q2c = kvqp.tile([128, NCH], F16, tag="q2c", bufs=2)
                    nc.vector.tensor_mul(q2c[:], q16c[:], q16c[:])
                    rkps = psMain.tile([128, NCH], F32, tag="m")
                    nc.tensor.matmul(rkps[:], lhsT=halvesbc16[:], rhs=k2c[:],
                                     start=True, stop=True)
                    sek = statp.tile([128, NCH], F32, tag="se_", bufs=2)
                    nc.scalar.activation(sek[:], rkps[:], AF.Sqrt)
                    rbk = statp.tile([128, NCH], F32, tag="rb_", bufs=2)
                    nc.vector.reciprocal_approx_fast(out=rbk[:], in_=sek[:])
                    nc.gpsimd.tensor_tensor(out=il[:, 2 * sl.start:2 * sl.stop:2],
                                            in0=k16c[:], in1=rbk[:], op=OP.mult)
                    rqps = psMain.tile([128, NCH], F32, tag="m")
                    nc.tensor.matmul(rqps[:], lhsT=halvesbc16[:], rhs=q2c[:],
                                     start=True, stop=True)
                    seq2 = statp.tile([128, NCH], F32, tag="se_", bufs=2)
                    nc.scalar.activation(seq2[:], rqps[:], AF.Sqrt)
                    rbq = statp.tile([128, NCH], F32, tag="rb_", bufs=2)
                    nc.vector.reciprocal_approx_fast(out=rbq[:], in_=seq2[:])
                    nc.vector.tensor_tensor(out=qh[:, sl], in0=q16c[:],
                                            in1=rbq[:], op=OP.mult)

            def do_b2(p):
                il, qh = il_t[p], qh_t[p]
                # --- segmented |khat| sums + q_probe + scores + topk ---
                il4 = il[:].rearrange("p (h w d) -> p h w d", h=64, w=64, d=2)
                kabs_r = pairp.tile([128, 64], F32, tag="kabsr")
                nc.vector.tensor_reduce(out=kabs_r[:], in_=il4[:, :, :, 0],
                                        axis=AX.X, op=OP.add, apply_absolute_value=True)
                il4c = il[:].rearrange("p (h w d) -> p w h d", h=64, w=64, d=2)
                kabs_c = pairp.tile([128, 64], F32, tag="kabsc")
                nc.vector.tensor_reduce(out=kabs_c[:], in_=il4c[:, :, :, 0],
                                        axis=AX.X, op=OP.add, apply_absolute_value=True)
                qp = pairp.tile([128, 1], F32, tag="qp")
                nc.vector.tensor_reduce(out=qp[:], in_=qh[:], axis=AX.X, op=OP.add)
                qp2 = pairp.tile([128, 2], F32, tag="qp2")
                nc.vector.memset(qp2[:], 0.0)
                nc.vector.tensor_copy(out=qp2[0:64, 0:1], in_=qp[0:64, :])
                nc.vector.tensor_copy(out=qp2[64:128, 1:2], in_=qp[64:128, :])
                sc_r = pairp.tile([2, 64], F32, tag="scr")
                sc_ps = psSmall.tile([2, 64], F32, tag="s")
                nc.tensor.matmul(sc_ps[:], lhsT=qp2[:], rhs=kabs_r[:],
                                 start=True, stop=True)
                nc.scalar.copy(sc_r[:], sc_ps[:])
                sc_c = pairp.tile([2, 64], F32, tag="scc")
                sc_ps2 = psSmall.tile([2, 64], F32, tag="s")
                nc.tensor.matmul(sc_ps2[:], lhsT=qp2[:], rhs=kabs_c[:],
                                 start=True, stop=True)
                nc.scalar.copy(sc_c[:], sc_ps2[:])
                mx = pairp.tile([2, 8], F32, tag="mx")
                idx_r = pairp.tile([2, 8], U32, tag="idxr")
                nc.vector.max(out=mx[:], in_=sc_r[:])
                nc.vector.max_index(out=idx_r[:], in_max=mx[:], in_values=sc_r[:])
                mxc = pairp.tile([2, 8], F32, tag="mxc")
                idx_c = pairp.tile([2, 8], U32, tag="idxc")
                nc.vector.max(out=mxc[:], in_=sc_c[:])
                nc.vector.max_index(out=idx_c[:], in_max=mxc[:], in_values=sc_c[:])
                idxr_f = pairp.tile([2, 8], F32, tag="idxrf")
                nc.vector.tensor_copy(out=idxr_f[:], in_=idx_r[:])
                idxc_f = pairp.tile([2, 8], F32, tag="idxcf")
                nc.vector.tensor_copy(out=idxc_f[:], in_=idx_c[:])
                # broadcast idx rows to all partitions by head half
                rbc_ps = psSmall.tile([128, 8], F32, tag="s")
                nc.tensor.matmul(rbc_ps[:], lhsT=zsel2[:], rhs=idxr_f[:],
                                 start=True, stop=True)
                rbc = pairp.tile([128, 8], F32, tag="rbc")
                nc.scalar.copy(rbc[:], rbc_ps[:])
                cbc_ps = psSmall.tile([128, 8], F32, tag="s")
                nc.tensor.matmul(cbc_ps[:], lhsT=zsel2[:], rhs=idxc_f[:],
                                 start=True, stop=True)
                cbc = pairp.tile([128, 8], F32, tag="cbc")
                nc.scalar.copy(cbc[:], cbc_ps[:])
                # Bcol[p] = idx_c[h(p), p%8]
                junk8 = pairp.tile([128, 8], F32, tag="junk8")
                nc.vector.tensor_mul(junk8[:], cbc[:], onehot8[:])
                Bcol = pairp.tile([128, 1], F32, tag="Bcol")
                nc.vector.tensor_reduce(out=Bcol[:], in_=junk8[:], axis=AX.X,
                                        op=OP.add)
                # wr[p, s] = idx_r[h(p), 2s + ((p>>3)&1)]
                wdiff = pairp.tile([128, 4], F32, tag="wdiff")
                nc.vector.tensor_sub(wdiff[:], rbc[:, 1:8:2], rbc[:, 0:8:2])
                wsel = pairp.tile([128, 4], F32, tag="wsel")
                nc.vector.tensor_scalar(wsel[:], wdiff[:], m8f[:], scalar2=None,
                                        op0=OP.mult)
                wr = pairp.tile([128, 4], F32, tag="wr")
                nc.vector.tensor_add(wr[:], wsel[:], rbc[:, 0:8:2])
                posfw = pairp.tile([128, 4], F32, tag="posfw")
                nc.vector.scalar_tensor_tensor(out=posfw[:], in0=wr[:], scalar=64.0,
                                               in1=Bcol[:].to_broadcast([128, 4]),
                                               op0=OP.mult, op1=OP.add)
                widx32 = pairp.tile([128, 4], I32, tag="widx32")
                nc.vector.tensor_copy(out=widx32[:], in_=posfw[:])
                widx = pairp.tile([128, 4], I16, tag="widx")
                nc.vector.tensor_copy(out=widx[:], in_=widx32[:])
                # --- gather ---
                ksel_il = selp.tile([128, 128], F16, tag="kselil", bufs=2,
                                    name=f"ksel{p}")
                nc.gpsimd.ap_gather(
                    out_ap=ksel_il[:].rearrange("p (k d) -> p k d", d=2),
                    in_ap=il[:].rearrange("p (n d) -> p n d", d=2),
                    idxs_ap=widx[:],
                    channels=128, num_elems=N, d=2, num_idxs=KEYS)
                ksel_t[p] = ksel_il

            def do_extract(p):
                ksel_il = ksel_t[p]
                kbd = selp.tile([128, 128], F16, tag="kbd", bufs=2, name=f"kbd{p}")
                nc.vector.memset(kbd[:], 0.0)
                nc.vector.tensor_copy(out=kbd[0:64, 0:64], in_=ksel_il[0:64, 0:128:2])
                nc.vector.tensor_copy(out=kbd[64:128, 64:128],
                                      in_=ksel_il[64:128, 0:128:2])
                vbd = selp.tile([128, 128], F16, tag="vbd", bufs=2, name=f"vbd{p}")
                nc.vector.memset(vbd[:], 0.0)
                for h in range(2):
                    o = 64 * h
                    tps = psSmall.tile([64, 64], F16, tag="s")
                    nc.tensor.transpose(out=tps[:], in_=ksel_il[o:o + 64, 1:128:2],
                                        identity=ident16[o:o + 64, :])
                    nc.scalar.copy(vbd[o:o + 64, o:o + 64], tps[:])
                kbd_t[p], vbd_t[p] = kbd, vbd

            def do_b3(p):
                kbd, vbd, qh = kbd_t[p], vbd_t[p], qh_t[p]
                # --- attention for this pair ---
                for ch in range(CH):
                    sl = slice(ch * NCH, (ch + 1) * NCH)
                    sps = psMain.tile([128, NCH], F32, tag="m")
                    nc.tensor.matmul(sps[:], lhsT=kbd[:], rhs=qh[:, sl],
                                     start=True, stop=True)
                    pt = ptp.tile([128, NCH], F16, tag="pT")
                    nc.scalar.activation(pt[:], sps[:], AF.Exp)
                    zps = psSmall.tile([2, NCH], F32, tag="s")
                    nc.tensor.matmul(zps[:], lhsT=halves2f[:], rhs=pt[:],
                                     start=True, stop=True)
                    zinv = ptp.tile([2, NCH], F32, tag="zinv")
                    nc.vector.reciprocal_approx_fast(out=zinv[:], in_=zps[:])
                    zb = psMain.tile([128, NCH], F32, tag="m")
                    nc.tensor.matmul(zb[:], lhsT=zsel2[:], rhs=zinv[:],
                                     start=True, stop=True)
                    ph16 = ptp.tile([128, NCH], F16, tag="ph16")
                    nc.vector.tensor_tensor(out=ph16[:], in0=pt[:], in1=zb[:],
                                            op=OP.mult)
                    pvs = psMain.tile([128, NCH], F32, tag="m")
                    nc.tensor.matmul(pvs[:], lhsT=vbd[:], rhs=ph16[:],
                                     start=True, stop=True)
                    nc.scalar.copy(ao16[p][:, sl], pvs[:])

            if stop_stage >= 2:
                do_b1(0); do_b2(0)
                do_b1(1); do_b2(1)
                do_extract(0)
                if stop_stage >= 3:
                    do_b3(0)
                do_b1(2); do_b2(2)
                do_extract(1)
                if stop_stage >= 3:
                    do_b3(1)
                do_b1(3); do_b2(3)
                do_extract(2)
                if stop_stage >= 3:
                    do_b3(2)
                do_extract(3)
                if stop_stage >= 3:
                    do_b3(3)

            # ------------- out-proj + out-LN + residual -------------
            y16 = [attnp.tile([128, N], BF16, tag=f"y16_{i}", name=f"y16_{i}") for i in range(2)]
            for ch in range(CH if stop_stage >= 4 else 0):
                sl = slice(ch * NCH, (ch + 1) * NCH)
                for i in range(2):
                    yps = psStat.tile([128, NCH], F32, tag="st")
                    for p in range(PAIRS):
                        nc.tensor.matmul(yps[:],
                                         lhsT=woutT[p][:, 128 * i:128 * (i + 1)],
                                         rhs=ao16[p][:, sl], start=(p == 0),
                                         stop=(p == 3))
                    nc.scalar.copy(y16[i][:, sl], yps[:])
            for ch in range(CH if stop_stage >= 4 else 0):
                sl = slice(ch * NCH, (ch + 1) * NCH)
                y2 = [finp.tile([128, NCH], BF16, tag="y2", name="y2", bufs=1) for _ in range(2)]
                for i in range(2):
                    nc.vector.tensor_mul(y2[i][:], y16[i][:, sl], y16[i][:, sl])
                S_ps = psStat.tile([128, NCH], F32, tag="st")
                nc.tensor.matmul(S_ps[:], lhsT=ones128_16[:], rhs=y16[0][:, sl],
                                 start=True, stop=False)
                nc.tensor.matmul(S_ps[:], lhsT=ones128_16[:], rhs=y16[1][:, sl],
                                 start=False, stop=True)
                Q_ps = psStat.tile([128, NCH], F32, tag="st")
                nc.tensor.matmul(Q_ps[:], lhsT=ones128_16[:], rhs=y2[0][:],
                                 start=True, stop=False)
                nc.tensor.matmul(Q_ps[:], lhsT=ones128_16[:], rhs=y2[1][:],
                                 start=False, stop=True)
                t_mu = finp.tile([128, NCH], F32, tag="ft_mu")
                nc.vector.tensor_scalar(t_mu[:], S_ps[:], 1.0 / C,
                                        scalar2=None, op0=OP.mult)
                t1 = finp.tile([128, NCH], F32, tag="fse", name="ft1")
                nc.scalar.activation(t1[:], t_mu[:], AF.Square)
                varb = finp.tile([128, NCH], F32, tag="fvarb")
                nc.vector.scalar_tensor_tensor(out=varb[:], in0=Q_ps[:], scalar=1.0 / C,
                                               in1=t1[:], op0=OP.mult, op1=OP.subtract)
                se = finp.tile([128, NCH], F32, tag="fse")
                nc.scalar.activation(se[:], varb[:], AF.Sqrt, bias=eps_c[:])
                rstd_b = finp.tile([128, NCH], F32, tag="frstd")
                nc.vector.reciprocal_approx_fast(out=rstd_b[:], in_=se[:])
                mu_b = finp.tile([128, NCH], F32, tag="fmu")
                nc.vector.tensor_mul(mu_b[:], t_mu[:], rstd_b[:])
                for i in range(2):
                    qs_t = finp.tile([128, NCH], F32, tag="qs_t")
                    nc.sync.dma_start(out=qs_t[:], in_=qs_d[128 * i:128 * (i + 1), sl])
                    t = finp.tile([128, NCH], F32, tag="fabc", name="fa", bufs=2)
                    nc.vector.tensor_tensor(out=t[:], in0=y16[i][:, sl],
                                            in1=rstd_b[:], op=OP.mult)
                    t2 = finp.tile([128, NCH], F32, tag="fabc", name="fb", bufs=2)
                    nc.vector.tensor_sub(t2[:], t[:], mu_b[:])
                    t3 = finp.tile([128, NCH], F32, tag="fabc", name="fc", bufs=2)
                    nc.scalar.activation(t3[:], t2[:], AF.Copy, scale=gg[i][:])
                    ot = finp.tile([128, NCH], F32, tag="fabc", name="fd", bufs=2)
                    nc.gpsimd.tensor_add(ot[:], t3[:], qs_t[:])
                    nc.sync.dma_start(out=out_d[128 * i:128 * (i + 1), sl], in_=ot[:])

            if stop_stage < 4:
                for i in range(2):
                    dummy = finp.tile([128, N], F32, tag="dummy")
                    nc.vector.memset(dummy[:], 0.0)
                    nc.sync.dma_start(out=out_d[128 * i:128 * (i + 1), :],
                                      in_=dummy[:])
    nc.finalize()
    return nc


_CACHE = {}


def kernel(**inputs):
    qsrc = np.asarray(inputs["query_source"], np.float32)
    ctx = np.asarray(inputs["context"], np.float32)
    cn_g = np.asarray(inputs["cn_g"], np.float32).reshape(C)
    cn_b = np.asarray(inputs["cn_b"], np.float32).reshape(C)
    qn_g = np.asarray(inputs["qn_g"], np.float32).reshape(C)
    qn_b = np.asarray(inputs["qn_b"], np.float32).reshape(C)
    on_g = np.asarray(inputs["on_g"], np.float32).reshape(C)
    on_b = np.asarray(inputs["on_b"], np.float32).reshape(C)
    w_kv = np.asarray(inputs["w_kv"], np.float32)
    w_q = np.asarray(inputs["w_q"], np.float32)
    w_out = np.asarray(inputs["w_out"], np.float32)
    gamma = float(np.asarray(inputs["gamma"], np.float32).reshape(()))

    assert np.abs(cn_b).max() == 0 and np.abs(qn_b).max() == 0 and \
        np.abs(on_b).max() == 0, "nonzero LN bias not implemented"

    import ml_dtypes
    bf16 = ml_dtypes.bfloat16
    wkvT = np.ascontiguousarray((w_kv * cn_g[None, :]).T).astype(np.float16)
    wqT = np.ascontiguousarray((w_q * qn_g[None, :]).T).astype(np.float16)
    woutT = np.ascontiguousarray(w_out.T).astype(bf16)
    gg = np.ascontiguousarray((gamma * on_g).reshape(C, 1), np.float32)

    p_idx = np.arange(128)
    identc = np.zeros((128, 64), np.float16)
    identc[p_idx, p_idx % 64] = 1.0
    onehot8c = (p_idx[:, None] % 8 == np.arange(8)[None, :]).astype(np.float32)
    m8ic = (((p_idx >> 3) & 1).astype(np.int32)).reshape(128, 1)
    zsel2c = (np.arange(128)[None, :] // 64 ==
              np.arange(2)[:, None]).astype(np.float32)

    if "nc" not in _CACHE:
        _CACHE["nc"] = build_program()
    nc = _CACHE["nc"]

    B = qsrc.shape[0]
    in_maps = []
    for b in range(B):
        in_maps.append({
            "ctx": np.ascontiguousarray(ctx[b].reshape(C, N)),
            "qsrc": np.ascontiguousarray(qsrc[b].reshape(C, N)),
            "wkvT": wkvT,
            "wqT": wqT,
            "woutT": woutT,
            "gg": gg,
            "identc": identc,
            "onehot8c": onehot8c,
            "m8ic": m8ic,
            "m8fc": m8ic.astype(np.float32),
            "zsel2c": zsel2c,
        })
    res = run_bass_kernel_spmd(nc, in_maps, core_ids=list(range(8)))
    outs = [np.asarray(r["out"], np.float32).reshape(1, C, 64, 64)
            for r in res.results]
    return np.concatenate(outs, axis=0)



# revision 34
# speedup vs baseline: 1.3141x; 1.0461x over previous
"""DPCA block (dual-pruned cross-attention) Trainium2 kernel.

Sharding: data-parallel over batch. B=8 -> 8 NeuronCores, one batch per core,
weights replicated, zero collectives.

Per-core dataflow (channel-major: channels on partitions, positions free):
 - chan-LN: gains folded into weights on host; per-position mu/rstd from PE
   ones-matmul broadcast-sums; x'' = (x-mu)*rstd stored bf16.
 - projections bf16 (f32 PSUM accumulate).
 - l2norm factors per head row; khat = k*rstd_b interleaved with v into a
   (khat,v) bf16 pair tensor so one gpsimd.ap_gather pulls both.
 - top-8 rows/cols via vector.max + max_index on f32 scores; 64 gathered
   (row,col) positions per head.
 - attention with head PAIRS block-diag packed on 128 partitions, keys on
   partitions, softmax Z via half-ones matvec, exp needs no max-subtract
   (|sim| <= 1 since khat,qhat l2-normalized).
 - out-proj bf16 + out-LN (same stats trick) + gamma*.. + residual in f32.
"""

import numpy as np

import concourse.bass as bass
import concourse.bacc as bacc
import concourse.mybir as mybir
from concourse.tile import TileContext
from concourse.bass_utils import run_bass_kernel_spmd

F32 = mybir.dt.float32
F32R = mybir.dt.float32r
BF16 = mybir.dt.bfloat16
F16 = mybir.dt.float16
I16 = mybir.dt.int16
I32 = mybir.dt.int32
U32 = mybir.dt.uint32
AX = mybir.AxisListType
OP = mybir.AluOpType
AF = mybir.ActivationFunctionType

C = 256
N = 4096
HEADS = 8
D = 64
PAIRS = 4
INNER = HEADS * D        # 512
NCH = 512
CH = N // NCH            # 8
KEYS = 64                # 8 rows x 8 cols kept per head
EPS = 1e-5


def build_program(stop_stage=99, sub=99):
    nc = bacc.Bacc()

    ctx_d = nc.declare_dram_parameter("ctx", [C, N], F32, False)
    qs_d = nc.declare_dram_parameter("qsrc", [C, N], F32, False)
    wkvT_d = nc.declare_dram_parameter("wkvT", [C, 2 * INNER], F16, False)
    wqT_d = nc.declare_dram_parameter("wqT", [C, INNER], F16, False)
    woutT_d = nc.declare_dram_parameter("woutT", [INNER, C], BF16, False)
    gg_d = nc.declare_dram_parameter("gg", [C, 1], F32, False)
    ident_d = nc.declare_dram_parameter("identc", [128, 64], F16, False)
    onehot8_d = nc.declare_dram_parameter("onehot8c", [128, 8], F32, False)
    m8i_d = nc.declare_dram_parameter("m8ic", [128, 1], I32, False)
    m8f_d = nc.declare_dram_parameter("m8fc", [128, 1], F32, False)
    zsel2_d = nc.declare_dram_parameter("zsel2c", [2, 128], F32, False)
    out_d = nc.declare_dram_parameter("out", [C, N], F32, True)

    with TileContext(nc) as tc:
        with (
            tc.tile_pool(name="const", bufs=1) as constp,
            tc.tile_pool(name="wpool", bufs=1) as wpool,
            tc.tile_pool(name="xin", bufs=2) as xin,
            tc.tile_pool(name="stat", bufs=1) as statp,
            tc.tile_pool(name="xpp", bufs=1) as xpp,
            tc.tile_pool(name="kvq", bufs=1) as kvqp,
            tc.tile_pool(name="pairs", bufs=2) as pairp,
            tc.tile_pool(name="sel", bufs=1) as selp,
            tc.tile_pool(name="attn", bufs=1) as attnp,
            tc.tile_pool(name="ptile", bufs=2) as ptp,
            tc.tile_pool(name="fin", bufs=1) as finp,
            tc.tile_pool(name="psStat", bufs=3, space="PSUM") as psStat,
            tc.tile_pool(name="psMain", bufs=4, space="PSUM") as psMain,
            tc.tile_pool(name="psSmall", bufs=1, space="PSUM") as psSmall,
        ):
            # ------------- constants -------------
            ones128 = constp.tile([128, 128], F32, tag="ones128")
            nc.vector.memset(ones128[:], 1.0)
            ones128_16 = constp.tile([128, 128], BF16, tag="ones128_16")
            nc.vector.memset(ones128_16[:], 1.0)
            ones128_f16 = constp.tile([128, 128], F16, tag="ones128_f16")
            nc.vector.memset(ones128_f16[:], 1.0)
            halves2 = constp.tile([128, 2], F32, tag="halves2")
            nc.vector.memset(halves2[:], 0.0)
            nc.vector.memset(halves2[0:64, 0:1], 1.0)
            nc.vector.memset(halves2[64:128, 1:2], 1.0)
            eps_c = constp.tile([128, 1], F32, tag="eps_c")
            nc.vector.memset(eps_c[:], EPS)
            halves2f = constp.tile([128, 2], F16, tag="halves2f")
            nc.vector.memset(halves2f[:], 0.0)
            nc.vector.memset(halves2f[0:64, 0:1], 1.0)
            nc.vector.memset(halves2f[64:128, 1:2], 1.0)
            ident16 = constp.tile([128, 64], F16, tag="ident16")
            nc.sync.dma_start(out=ident16[:], in_=ident_d[:])
            # block-diag ones: half-broadcast-sum stationary
            halvesbc16 = constp.tile([128, 128], F16, tag="halvesbc16")
            nc.vector.memset(halvesbc16[:], 0.0)
            nc.vector.memset(halvesbc16[0:64, 0:64], 1.0)
            nc.vector.memset(halvesbc16[64:128, 64:128], 1.0)
            zsel2 = constp.tile([2, 128], F32, tag="zsel2")
            nc.sync.dma_start(out=zsel2[:], in_=zsel2_d[:])
            onehot8 = constp.tile([128, 8], F32, tag="onehot8")
            nc.sync.dma_start(out=onehot8[:], in_=onehot8_d[:])
            m8f = constp.tile([128, 1], F32, tag="m8f")
            nc.sync.dma_start(out=m8f[:], in_=m8f_d[:])

            # ------------- weights -------------
            wkvT = [wpool.tile([128, 2 * INNER], F16, tag=f"wkvT{i}", name=f"wkvT{i}") for i in range(2)]
            wqT = [wpool.tile([128, INNER], F16, tag=f"wqT{i}", name=f"wqT{i}") for i in range(2)]
            for i in range(2):
                nc.sync.dma_start(out=wkvT[i][:], in_=wkvT_d[128 * i:128 * (i + 1), :])
                nc.sync.dma_start(out=wqT[i][:], in_=wqT_d[128 * i:128 * (i + 1), :])
            woutT = [wpool.tile([128, C], BF16, tag=f"woutT{i}", name=f"woutT{i}") for i in range(4)]
            for i in range(4):
                nc.sync.dma_start(out=woutT[i][:], in_=woutT_d[128 * i:128 * (i + 1), :])
            gg = [wpool.tile([128, 1], F32, tag=f"gg{i}", name=f"gg{i}") for i in range(2)]
            for i in range(2):
                nc.sync.dma_start(out=gg[i][:], in_=gg_d[128 * i:128 * (i + 1), :])

            # ------------- phase A: chan-LN -> x'' (f16) -------------
            # ctx gets the full (x-mu)*rstd (v needs it); qs gets only x-mu:
            # the q l2norm cancels any per-position scale, so rstd_qs (and the
            # whole variance pipeline) is unnecessary for the q path.
            # ctx and qs chunks are interleaved as two independent dependency
            # chains (qs stats borrow the otherwise-idle psMain banks).
            xpp_t = {}
            for name in ("ctx", "qs"):
                xpp_t[name] = [xpp.tile([128, N], F16, tag=f"xpp_{name}{i}",
                                        name=f"xpp_{name}{i}")
                               for i in range(2)]
            for ch in range(CH):
                sl = slice(ch * NCH, (ch + 1) * NCH)
                # --- ctx chunk: full LN ---
                xt = [xin.tile([128, NCH], F32, tag="xt", name="xt") for _ in range(2)]
                for i in range(2):
                    nc.sync.dma_start(out=xt[i][:],
                                      in_=ctx_d[128 * i:128 * (i + 1), sl])
                S_ps = psStat.tile([128, NCH], F32, tag="st")
                nc.tensor.matmul(S_ps[:], lhsT=ones128[:], rhs=xt[0][:],
                                 start=True, stop=False)
                nc.tensor.matmul(S_ps[:], lhsT=ones128[:], rhs=xt[1][:],
                                 start=False, stop=True)
                t_mu = statp.tile([128, NCH], F32, tag="t_mu", bufs=2)
                nc.vector.tensor_scalar(t_mu[:], S_ps[:], 1.0 / C,
                                        scalar2=None, op0=OP.mult)
                xsq = [xin.tile([128, NCH], F16, tag="xsq", name="xsq", bufs=1)
                       for _ in range(2)]
                for i in range(2):
                    nc.scalar.activation(xsq[i][:], xt[i][:], AF.Square)
                Q_ps = psStat.tile([128, NCH], F32, tag="st")
                nc.tensor.matmul(Q_ps[:], lhsT=ones128_f16[:], rhs=xsq[0][:],
                                 start=True, stop=False)
                nc.tensor.matmul(Q_ps[:], lhsT=ones128_f16[:], rhs=xsq[1][:],
                                 start=False, stop=True)
                t1 = statp.tile([128, NCH], F32, tag="se", name="t1")
                nc.scalar.activation(t1[:], t_mu[:], AF.Square)
                varb = statp.tile([128, NCH], F32, tag="varb")
                nc.vector.scalar_tensor_tensor(out=varb[:], in0=Q_ps[:],
                                               scalar=1.0 / C, in1=t1[:],
                                               op0=OP.mult, op1=OP.subtract)
                se = statp.tile([128, NCH], F32, tag="se")
                nc.scalar.activation(se[:], varb[:], AF.Sqrt, bias=eps_c[:])
                rstd_b = statp.tile([128, NCH], F32, tag="rstd_b")
                nc.vector.reciprocal_approx_fast(out=rstd_b[:], in_=se[:])
                for i in range(2):
                    xc = statp.tile([128, NCH], F32, tag="xr", bufs=2)
                    nc.gpsimd.tensor_sub(xc[:], xt[i][:], t_mu[:])
                    nc.vector.tensor_tensor(out=xpp_t["ctx"][i][:, sl],
                                            in0=xc[:], in1=rstd_b[:], op=OP.mult)
                # --- qs chunk: mean-subtract only (independent chain) ---
                xtq = [xin.tile([128, NCH], F32, tag="xtq", name="xtq")
                       for _ in range(2)]
                for i in range(2):
                    nc.sync.dma_start(out=xtq[i][:],
                                      in_=qs_d[128 * i:128 * (i + 1), sl])
                Sq_ps = psMain.tile([128, NCH], F32, tag="m")
                nc.tensor.matmul(Sq_ps[:], lhsT=ones128[:], rhs=xtq[0][:],
                                 start=True, stop=False)
                nc.tensor.matmul(Sq_ps[:], lhsT=ones128[:], rhs=xtq[1][:],
                                 start=False, stop=True)
                t_muq = statp.tile([128, NCH], F32, tag="t_muq", bufs=1)
                nc.vector.tensor_scalar(t_muq[:], Sq_ps[:], 1.0 / C,
                                        scalar2=None, op0=OP.mult)
                for i in range(2):
                    nc.gpsimd.tensor_sub(xpp_t["qs"][i][:, sl],
                                         xtq[i][:], t_muq[:])

            # ------------- phase B: software-pipelined per head-pair -------
            # Emission order interleaves pairs so each pair's attention (which
            # waits on its gather) is emitted after the next pair's
            # projections: the in-order PE queue then never stalls on a
            # gather.
            ao16 = [attnp.tile([128, N], BF16, tag=f"ao{p}", name=f"ao{p}")
                    for p in range(PAIRS)]
            il_t, qh_t, ksel_t, kbd_t, vbd_t = {}, {}, {}, {}, {}

            def do_b1(p):
                il = kvqp.tile([128, 2 * N], F16, tag="il", bufs=2, name=f"il{p}")
                qh = kvqp.tile([128, N], F16, tag="qh", bufs=2, name=f"qh{p}")
                il_t[p], qh_t[p] = il, qh
                for ch in range(CH):
                    sl = slice(ch * NCH, (ch + 1) * NCH)
                    # --- projections (k, v, q) for this chunk ---
                    kps = psMain.tile([128, NCH], F32, tag="m")
                    nc.tensor.matmul(kps[:], lhsT=wkvT[0][:, 128 * p:128 * (p + 1)],
                                     rhs=xpp_t["ctx"][0][:, sl], start=True, stop=False)
                    nc.tensor.matmul(kps[:], lhsT=wkvT[1][:, 128 * p:128 * (p + 1)],
                                     rhs=xpp_t["ctx"][1][:, sl], start=False, stop=True)
                    k16c = kvqp.tile([128, NCH], F16, tag="k16c", bufs=2)
                    nc.scalar.copy(k16c[:], kps[:])
                    vps = psMain.tile([128, NCH], F32, tag="m")
                    vo = INNER + 128 * p
                    nc.tensor.matmul(vps[:], lhsT=wkvT[0][:, vo:vo + 128],
                                     rhs=xpp_t["ctx"][0][:, sl], start=True, stop=False)
                    nc.tensor.matmul(vps[:], lhsT=wkvT[1][:, vo:vo + 128],
                                     rhs=xpp_t["ctx"][1][:, sl], start=False, stop=True)
                    nc.scalar.copy(il[:, 2 * sl.start + 1:2 * sl.stop:2], vps[:])
                    qps = psMain.tile([128, NCH], F32, tag="m")
                    nc.tensor.matmul(qps[:], lhsT=wqT[0][:, 128 * p:128 * (p + 1)],
                                     rhs=xpp_t["qs"][0][:, sl], start=True, stop=False)
                    nc.tensor.matmul(qps[:], lhsT=wqT[1][:, 128 * p:128 * (p + 1)],
                                     rhs=xpp_t["qs"][1][:, sl], start=False, stop=True)
                    q16c = kvqp.tile([128, NCH], F16, tag="q16c", bufs=2)
                    nc.scalar.copy(q16c[:], qps[:])
                    # --- l2 factors + khat/qhat ---
                    k2c = kvqp.tile([128, NCH], F16, tag="k2c", bufs=1)
                    nc.vector.tensor_mul(k2c[:], k16c[:], k16c[:])
                    q2c = kvqp.tile([128, NCH], F16, tag="q2c", bufs=2)
                    nc.vector.tensor_mul(q2c[:], q16c[:], q16c[:])
                    rkps = psMain.tile([128, NCH], F32, tag="m")
                    nc.tensor.matmul(rkps[:], lhsT=halvesbc16[:], rhs=k2c[:],
                                     start=True, stop=True)
                    sek = statp.tile([128, NCH], F32, tag="se_", bufs=2)
                    nc.scalar.activation(sek[:], rkps[:], AF.Sqrt)
                    rbk = statp.tile([128, NCH], F32, tag="rb_", bufs=2)
                    nc.vector.reciprocal_approx_fast(out=rbk[:], in_=sek[:])
                    nc.gpsimd.tensor_tensor(out=il[:, 2 * sl.start:2 * sl.stop:2],
                                            in0=k16c[:], in1=rbk[:], op=OP.mult)
                    rqps = psMain.tile([128, NCH], F32, tag="m")
                    nc.tensor.matmul(rqps[:], lhsT=halvesbc16[:], rhs=q2c[:],
                                     start=True, stop=True)
                    seq2 = statp.tile([128, NCH], F32, tag="se_", bufs=2)
                    nc.scalar.activation(seq2[:], rqps[:], AF.Sqrt)
                    rbq = statp.tile([128, NCH], F32, tag="rb_", bufs=2)
                    nc.vector.reciprocal_approx_fast(out=rbq[:], in_=seq2[:])
                    nc.vector.tensor_tensor(out=qh[:, sl], in0=q16c[:],
                                            in1=rbq[:], op=OP.mult)

            def do_b2(p):
                il, qh = il_t[p], qh_t[p]
                # --- segmented |khat| sums + q_probe + scores + topk ---
                il4 = il[:].rearrange("p (h w d) -> p h w d", h=64, w=64, d=2)
                kabs_r = pairp.tile([128, 64], F32, tag="kabsr")
                nc.vector.tensor_reduce(out=kabs_r[:], in_=il4[:, :, :, 0],
                                        axis=AX.X, op=OP.add, apply_absolute_value=True)
                il4c = il[:].rearrange("p (h w d) -> p w h d", h=64, w=64, d=2)
                kabs_c = pairp.tile([128, 64], F32, tag="kabsc")
                nc.vector.tensor_reduce(out=kabs_c[:], in_=il4c[:, :, :, 0],
                                        axis=AX.X, op=OP.add, apply_absolute_value=True)
                qp = pairp.tile([128, 1], F32, tag="qp")
                nc.vector.tensor_reduce(out=qp[:], in_=qh[:], axis=AX.X, op=OP.add)
                qp2 = pairp.tile([128, 2], F32, tag="qp2")
                nc.vector.memset(qp2[:], 0.0)
                nc.vector.tensor_copy(out=qp2[0:64, 0:1], in_=qp[0:64, :])
                nc.vector.tensor_copy(out=qp2[64:128, 1:2], in_=qp[64:128, :])
                sc_r = pairp.tile([2, 64], F32, tag="scr")
                sc_ps = psSmall.tile([2, 64], F32, tag="s")
                nc.tensor.matmul(sc_ps[:], lhsT=qp2[:], rhs=kabs_r[:],
                                 start=True, stop=True)
                nc.scalar.copy(sc_r[:], sc_ps[:])
                sc_c = pairp.tile([2, 64], F32, tag="scc")
                sc_ps2 = psSmall.tile([2, 64], F32, tag="s")
                nc.tensor.matmul(sc_ps2[:], lhsT=qp2[:], rhs=kabs_c[:],
                                 start=True, stop=True)
                nc.scalar.copy(sc_c[:], sc_ps2[:])
                mx = pairp.tile([2, 8], F32, tag="mx")
                idx_r = pairp.tile([2, 8], U32, tag="idxr")
                nc.vector.max(out=mx[:], in_=sc_r[:])
                nc.vector.max_index(out=idx_r[:], in_max=mx[:], in_values=sc_r[:])
                mxc = pairp.tile([2, 8], F32, tag="mxc")
                idx_c = pairp.tile([2, 8], U32, tag="idxc")
                nc.vector.max(out=mxc[:], in_=sc_c[:])
                nc.vector.max_index(out=idx_c[:], in_max=mxc[:], in_values=sc_c[:])
                idxr_f = pairp.tile([2, 8], F32, tag="idxrf")
                nc.vector.tensor_copy(out=idxr_f[:], in_=idx_r[:])
                idxc_f = pairp.tile([2, 8], F32, tag="idxcf")
                nc.vector.tensor_copy(out=idxc_f[:], in_=idx_c[:])
                # broadcast idx rows to all partitions by head half
                rbc_ps = psSmall.tile([128, 8], F32, tag="s")
                nc.tensor.matmul(rbc_ps[:], lhsT=zsel2[:], rhs=idxr_f[:],
                                 start=True, stop=True)
                rbc = pairp.tile([128, 8], F32, tag="rbc")
                nc.scalar.copy(rbc[:], rbc_ps[:])
                cbc_ps = psSmall.tile([128, 8], F32, tag="s")
                nc.tensor.matmul(cbc_ps[:], lhsT=zsel2[:], rhs=idxc_f[:],
                                 start=True, stop=True)
                cbc = pairp.tile([128, 8], F32, tag="cbc")
                nc.scalar.copy(cbc[:], cbc_ps[:])
                # Bcol[p] = idx_c[h(p), p%8]
                junk8 = pairp.tile([128, 8], F32, tag="junk8")
                nc.vector.tensor_mul(junk8[:], cbc[:], onehot8[:])
                Bcol = pairp.tile([128, 1], F32, tag="Bcol")
                nc.vector.tensor_reduce(out=Bcol[:], in_=junk8[:], axis=AX.X,
                                        op=OP.add)
                # wr[p, s] = idx_r[h(p), 2s + ((p>>3)&1)]
                wdiff = pairp.tile([128, 4], F32, tag="wdiff")
                nc.vector.tensor_sub(wdiff[:], rbc[:, 1:8:2], rbc[:, 0:8:2])
                wsel = pairp.tile([128, 4], F32, tag="wsel")
                nc.vector.tensor_scalar(wsel[:], wdiff[:], m8f[:], scalar2=None,
                                        op0=OP.mult)
                wr = pairp.tile([128, 4], F32, tag="wr")
                nc.vector.tensor_add(wr[:], wsel[:], rbc[:, 0:8:2])
                posfw = pairp.tile([128, 4], F32, tag="posfw")
                nc.vector.scalar_tensor_tensor(out=posfw[:], in0=wr[:], scalar=64.0,
                                               in1=Bcol[:].to_broadcast([128, 4]),
                                               op0=OP.mult, op1=OP.add)
                widx32 = pairp.tile([128, 4], I32, tag="widx32")
                nc.vector.tensor_copy(out=widx32[:], in_=posfw[:])
                widx = pairp.tile([128, 4], I16, tag="widx")
                nc.vector.tensor_copy(out=widx[:], in_=widx32[:])
                # --- gather ---
                ksel_il = selp.tile([128, 128], F16, tag="kselil", bufs=2,
                                    name=f"ksel{p}")
                nc.gpsimd.ap_gather(
                    out_ap=ksel_il[:].rearrange("p (k d) -> p k d", d=2),
                    in_ap=il[:].rearrange("p (n d) -> p n d", d=2),
                    idxs_ap=widx[:],
                    channels=128, num_elems=N, d=2, num_idxs=KEYS)
                ksel_t[p] = ksel_il

            def do_extract(p):
                ksel_il = ksel_t[p]
                kbd = selp.tile([128, 128], F16, tag="kbd", bufs=2, name=f"kbd{p}")
                nc.vector.memset(kbd[:], 0.0)
                nc.vector.tensor_copy(out=kbd[0:64, 0:64], in_=ksel_il[0:64, 0:128:2])
                nc.vector.tensor_copy(out=kbd[64:128, 64:128],
                                      in_=ksel_il[64:128, 0:128:2])
                vbd = selp.tile([128, 128], F16, tag="vbd", bufs=2, name=f"vbd{p}")
                nc.vector.memset(vbd[:], 0.0)
                for h in range(2):
                    o = 64 * h
                    tps = psSmall.tile([64, 64], F16, tag="s")
                    nc.tensor.transpose(out=tps[:], in_=ksel_il[o:o + 64, 1:128:2],
                                        identity=ident16[o:o + 64, :])
                    nc.scalar.copy(vbd[o:o + 64, o:o + 64], tps[:])
                kbd_t[p], vbd_t[p] = kbd, vbd

            def do_b3(p):
                kbd, vbd, qh = kbd_t[p], vbd_t[p], qh_t[p]
                # --- attention for this pair ---
                for ch in range(CH):
                    sl = slice(ch * NCH, (ch + 1) * NCH)
                    sps = psMain.tile([128, NCH], F32, tag="m")
                    nc.tensor.matmul(sps[:], lhsT=kbd[:], rhs=qh[:, sl],
                                     start=True, stop=True)
                    pt = ptp.tile([128, NCH], F16, tag="pT")
                    nc.scalar.activation(pt[:], sps[:], AF.Exp)
                    zps = psSmall.tile([2, NCH], F32, tag="s")
                    nc.tensor.matmul(zps[:], lhsT=halves2f[:], rhs=pt[:],
                                     start=True, stop=True)
                    zinv = ptp.tile([2, NCH], F32, tag="zinv")
                    nc.vector.reciprocal_approx_fast(out=zinv[:], in_=zps[:])
                    zb = psMain.tile([128, NCH], F32, tag="m")
                    nc.tensor.matmul(zb[:], lhsT=zsel2[:], rhs=zinv[:],
                                     start=True, stop=True)
                    ph16 = ptp.tile([128, NCH], F16, tag="ph16")
                    nc.vector.tensor_tensor(out=ph16[:], in0=pt[:], in1=zb[:],
                                            op=OP.mult)
                    pvs = psMain.tile([128, NCH], F32, tag="m")
                    nc.tensor.matmul(pvs[:], lhsT=vbd[:], rhs=ph16[:],
                                     start=True, stop=True)
                    nc.scalar.copy(ao16[p][:, sl], pvs[:])

            if stop_stage >= 2:
                do_b1(0); do_b2(0)
                do_b1(1); do_b2(1)
                do_extract(0)
                if stop_stage >= 3:
                    do_b3(0)
                do_b1(2); do_b2(2)
                do_extract(1)
                if stop_stage >= 3:
                    do_b3(1)
                do_b1(3); do_b2(3)
                do_extract(2)
                if stop_stage >= 3:
                    do_b3(2)
                do_extract(3)
                if stop_stage >= 3:
                    do_b3(3)

            # ------------- out-proj + out-LN + residual -------------
            y16 = [attnp.tile([128, N], BF16, tag=f"y16_{i}", name=f"y16_{i}") for i in range(2)]
            for ch in range(CH if stop_stage >= 4 else 0):
                sl = slice(ch * NCH, (ch + 1) * NCH)
                for i in range(2):
                    yps = psStat.tile([128, NCH], F32, tag="st")
                    for p in range(PAIRS):
                        nc.tensor.matmul(yps[:],
                                         lhsT=woutT[p][:, 128 * i:128 * (i + 1)],
                                         rhs=ao16[p][:, sl], start=(p == 0),
                                         stop=(p == 3))
                    nc.scalar.copy(y16[i][:, sl], yps[:])
                y2 = [finp.tile([128, NCH], BF16, tag="y2", name="y2", bufs=1) for _ in range(2)]
                for i in range(2):
                    nc.vector.tensor_mul(y2[i][:], y16[i][:, sl], y16[i][:, sl])
                S_ps = psStat.tile([128, NCH], F32, tag="st")
                nc.tensor.matmul(S_ps[:], lhsT=ones128_16[:], rhs=y16[0][:, sl],
                                 start=True, stop=False)
                nc.tensor.matmul(S_ps[:], lhsT=ones128_16[:], rhs=y16[1][:, sl],
                                 start=False, stop=True)
                Q_ps = psStat.tile([128, NCH], F32, tag="st")
                nc.tensor.matmul(Q_ps[:], lhsT=ones128_16[:], rhs=y2[0][:],
                                 start=True, stop=False)
                nc.tensor.matmul(Q_ps[:], lhsT=ones128_16[:], rhs=y2[1][:],
                                 start=False, stop=True)
                t_mu = finp.tile([128, NCH], F32, tag="ft_mu")
                nc.vector.tensor_scalar(t_mu[:], S_ps[:], 1.0 / C,
                                        scalar2=None, op0=OP.mult)
                t1 = finp.tile([128, NCH], F32, tag="fse", name="ft1")
                nc.scalar.activation(t1[:], t_mu[:], AF.Square)
                varb = finp.tile([128, NCH], F32, tag="fvarb")
                nc.vector.scalar_tensor_tensor(out=varb[:], in0=Q_ps[:], scalar=1.0 / C,
                                               in1=t1[:], op0=OP.mult, op1=OP.subtract)
                se = finp.tile([128, NCH], F32, tag="fse")
                nc.scalar.activation(se[:], varb[:], AF.Sqrt, bias=eps_c[:])
                rstd_b = finp.tile([128, NCH], F32, tag="frstd")
                nc.vector.reciprocal_approx_fast(out=rstd_b[:], in_=se[:])
                mu_b = finp.tile([128, NCH], F32, tag="fmu")
                nc.vector.tensor_mul(mu_b[:], t_mu[:], rstd_b[:])
                for i in range(2):
                    qs_t = finp.tile([128, NCH], F32, tag="qs_t")
                    nc.sync.dma_start(out=qs_t[:], in_=qs_d[128 * i:128 * (i + 1), sl])
                    t = finp.tile([128, NCH], F32, tag="fabc", name="fa", bufs=2)
                    nc.vector.tensor_tensor(out=t[:], in0=y16[i][:, sl],
                                            in1=rstd_b[:], op=OP.mult)
                    t2 = finp.tile([128, NCH], F32, tag="fabc", name="fb", bufs=2)
                    nc.vector.tensor_sub(t2[:], t[:], mu_b[:])
                    t3 = finp.tile([128, NCH], F32, tag="fabc", name="fc", bufs=2)
                    nc.scalar.activation(t3[:], t2[:], AF.Copy, scale=gg[i][:])
                    ot = finp.tile([128, NCH], F32, tag="fabc", name="fd", bufs=2)
                    nc.gpsimd.tensor_add(ot[:], t3[:], qs_t[:])
                    nc.sync.dma_start(out=out_d[128 * i:128 * (i + 1), sl], in_=ot[:])

            if stop_stage < 4:
                for i in range(2):
                    dummy = finp.tile([128, N], F32, tag="dummy")
                    nc.vector.memset(dummy[:], 0.0)
                    nc.sync.dma_start(out=out_d[128 * i:128 * (i + 1), :],
                                      in_=dummy[:])
    nc.finalize()
    return nc


_CACHE = {}


def kernel(**inputs):
    qsrc = np.asarray(inputs["query_source"], np.float32)
    ctx = np.asarray(inputs["context"], np.float32)
    cn_g = np.asarray(inputs["cn_g"], np.float32).reshape(C)
    cn_b = np.asarray(inputs["cn_b"], np.float32).reshape(C)
    qn_g = np.asarray(inputs["qn_g"], np.float32).reshape(C)
    qn_b = np.asarray(inputs["qn_b"], np.float32).reshape(C)
    on_g = np.asarray(inputs["on_g"], np.float32).reshape(C)
    on_b = np.asarray(inputs["on_b"], np.float32).reshape(C)
    w_kv = np.asarray(inputs["w_kv"], np.float32)
    w_q = np.asarray(inputs["w_q"], np.float32)
    w_out = np.asarray(inputs["w_out"], np.float32)
    gamma = float(np.asarray(inputs["gamma"], np.float32).reshape(()))

    assert np.abs(cn_b).max() == 0 and np.abs(qn_b).max() == 0 and \
        np.abs(on_b).max() == 0, "nonzero LN bias not implemented"

    import ml_dtypes
    bf16 = ml_dtypes.bfloat16
    wkvT = np.ascontiguousarray((w_kv * cn_g[None, :]).T).astype(np.float16)
    wqT = np.ascontiguousarray((w_q * qn_g[None, :]).T).astype(np.float16)
    woutT = np.ascontiguousarray(w_out.T).astype(bf16)
    gg = np.ascontiguousarray((gamma * on_g).reshape(C, 1), np.float32)

    p_idx = np.arange(128)
    identc = np.zeros((128, 64), np.float16)
    identc[p_idx, p_idx % 64] = 1.0
    onehot8c = (p_idx[:, None] % 8 == np.arange(8)[None, :]).astype(np.float32)
    m8ic = (((p_idx >> 3) & 1).astype(np.int32)).reshape(128, 1)
    zsel2c = (np.arange(128)[None, :] // 64 ==
              np.arange(2)[:, None]).astype(np.float32)

    if "nc" not in _CACHE:
        _CACHE["nc"] = build_program()
    nc = _CACHE["nc"]

    B = qsrc.shape[0]
    in_maps = []
    for b in range(B):
        in_maps.append({
            "ctx": np.ascontiguousarray(ctx[b].reshape(C, N)),
            "qsrc": np.ascontiguousarray(qsrc[b].reshape(C, N)),
            "wkvT": wkvT,
            "wqT": wqT,
            "woutT": woutT,
            "gg": gg,
            "identc": identc,
            "onehot8c": onehot8c,
            "m8ic": m8ic,
            "m8fc": m8ic.astype(np.float32),
            "zsel2c": zsel2c,
        })
    res = run_bass_kernel_spmd(nc, in_maps, core_ids=list(range(8)))
    outs = [np.asarray(r["out"], np.float32).reshape(1, C, 64, 64)
            for r in res.results]
    return np.concatenate(outs, axis=0)



# revision 35
# speedup vs baseline: 1.3359x; 1.0167x over previous
"""DPCA block (dual-pruned cross-attention) Trainium2 kernel.

Sharding: data-parallel over batch. B=8 -> 8 NeuronCores, one batch per core,
weights replicated, zero collectives.

Per-core dataflow (channel-major: channels on partitions, positions free):
 - chan-LN: gains folded into weights on host; per-position mu/rstd from PE
   ones-matmul broadcast-sums; x'' = (x-mu)*rstd stored bf16.
 - projections bf16 (f32 PSUM accumulate).
 - l2norm factors per head row; khat = k*rstd_b interleaved with v into a
   (khat,v) bf16 pair tensor so one gpsimd.ap_gather pulls both.
 - top-8 rows/cols via vector.max + max_index on f32 scores; 64 gathered
   (row,col) positions per head.
 - attention with head PAIRS block-diag packed on 128 partitions, keys on
   partitions, softmax Z via half-ones matvec, exp needs no max-subtract
   (|sim| <= 1 since khat,qhat l2-normalized).
 - out-proj bf16 + out-LN (same stats trick) + gamma*.. + residual in f32.
"""

import numpy as np

import concourse.bass as bass
import concourse.bacc as bacc
import concourse.mybir as mybir
from concourse.tile import TileContext
from concourse.bass_utils import run_bass_kernel_spmd

F32 = mybir.dt.float32
F32R = mybir.dt.float32r
BF16 = mybir.dt.bfloat16
F16 = mybir.dt.float16
I16 = mybir.dt.int16
I32 = mybir.dt.int32
U32 = mybir.dt.uint32
AX = mybir.AxisListType
OP = mybir.AluOpType
AF = mybir.ActivationFunctionType

C = 256
N = 4096
HEADS = 8
D = 64
PAIRS = 4
INNER = HEADS * D        # 512
NCH = 512
CH = N // NCH            # 8
KEYS = 64                # 8 rows x 8 cols kept per head
EPS = 1e-5


def build_program(stop_stage=99, sub=99):
    nc = bacc.Bacc()

    ctx_d = nc.declare_dram_parameter("ctx", [C, N], F32, False)
    qs_d = nc.declare_dram_parameter("qsrc", [C, N], F32, False)
    wkvT_d = nc.declare_dram_parameter("wkvT", [C, 2 * INNER], F16, False)
    wqT_d = nc.declare_dram_parameter("wqT", [C, INNER], F16, False)
    woutT_d = nc.declare_dram_parameter("woutT", [INNER, C], BF16, False)
    gg_d = nc.declare_dram_parameter("gg", [C, 1], F32, False)
    ident_d = nc.declare_dram_parameter("identc", [128, 64], F16, False)
    onehot8_d = nc.declare_dram_parameter("onehot8c", [128, 8], F32, False)
    m8i_d = nc.declare_dram_parameter("m8ic", [128, 1], I32, False)
    m8f_d = nc.declare_dram_parameter("m8fc", [128, 1], F32, False)
    zsel2_d = nc.declare_dram_parameter("zsel2c", [2, 128], F32, False)
    out_d = nc.declare_dram_parameter("out", [C, N], F32, True)

    with TileContext(nc) as tc:
        with (
            tc.tile_pool(name="const", bufs=1) as constp,
            tc.tile_pool(name="wpool", bufs=1) as wpool,
            tc.tile_pool(name="xin", bufs=2) as xin,
            tc.tile_pool(name="stat", bufs=1) as statp,
            tc.tile_pool(name="xpp", bufs=1) as xpp,
            tc.tile_pool(name="kvq", bufs=1) as kvqp,
            tc.tile_pool(name="pairs", bufs=2) as pairp,
            tc.tile_pool(name="sel", bufs=1) as selp,
            tc.tile_pool(name="attn", bufs=1) as attnp,
            tc.tile_pool(name="ptile", bufs=2) as ptp,
            tc.tile_pool(name="fin", bufs=1) as finp,
            tc.tile_pool(name="psStat", bufs=3, space="PSUM") as psStat,
            tc.tile_pool(name="psMain", bufs=4, space="PSUM") as psMain,
            tc.tile_pool(name="psSmall", bufs=1, space="PSUM") as psSmall,
        ):
            # ------------- constants -------------
            ones128 = constp.tile([128, 128], F32, tag="ones128")
            nc.vector.memset(ones128[:], 1.0)
            ones128_16 = constp.tile([128, 128], BF16, tag="ones128_16")
            nc.vector.memset(ones128_16[:], 1.0)
            ones128_f16 = constp.tile([128, 128], F16, tag="ones128_f16")
            nc.vector.memset(ones128_f16[:], 1.0)
            halves2 = constp.tile([128, 2], F32, tag="halves2")
            nc.vector.memset(halves2[:], 0.0)
            nc.vector.memset(halves2[0:64, 0:1], 1.0)
            nc.vector.memset(halves2[64:128, 1:2], 1.0)
            eps_c = constp.tile([128, 1], F32, tag="eps_c")
            nc.vector.memset(eps_c[:], EPS)
            halves2f = constp.tile([128, 2], F16, tag="halves2f")
            nc.vector.memset(halves2f[:], 0.0)
            nc.vector.memset(halves2f[0:64, 0:1], 1.0)
            nc.vector.memset(halves2f[64:128, 1:2], 1.0)
            ident16 = constp.tile([128, 64], F16, tag="ident16")
            nc.sync.dma_start(out=ident16[:], in_=ident_d[:])
            # block-diag ones: half-broadcast-sum stationary
            halvesbc16 = constp.tile([128, 128], F16, tag="halvesbc16")
            nc.vector.memset(halvesbc16[:], 0.0)
            nc.vector.memset(halvesbc16[0:64, 0:64], 1.0)
            nc.vector.memset(halvesbc16[64:128, 64:128], 1.0)
            zsel2 = constp.tile([2, 128], F32, tag="zsel2")
            nc.sync.dma_start(out=zsel2[:], in_=zsel2_d[:])
            onehot8 = constp.tile([128, 8], F32, tag="onehot8")
            nc.sync.dma_start(out=onehot8[:], in_=onehot8_d[:])
            m8f = constp.tile([128, 1], F32, tag="m8f")
            nc.sync.dma_start(out=m8f[:], in_=m8f_d[:])

            # ------------- weights -------------
            wkvT = [wpool.tile([128, 2 * INNER], F16, tag=f"wkvT{i}", name=f"wkvT{i}") for i in range(2)]
            wqT = [wpool.tile([128, INNER], F16, tag=f"wqT{i}", name=f"wqT{i}") for i in range(2)]
            for i in range(2):
                nc.sync.dma_start(out=wkvT[i][:], in_=wkvT_d[128 * i:128 * (i + 1), :])
                nc.sync.dma_start(out=wqT[i][:], in_=wqT_d[128 * i:128 * (i + 1), :])
            woutT = [wpool.tile([128, C], BF16, tag=f"woutT{i}", name=f"woutT{i}") for i in range(4)]
            for i in range(4):
                nc.sync.dma_start(out=woutT[i][:], in_=woutT_d[128 * i:128 * (i + 1), :])
            gg = [wpool.tile([128, 1], F32, tag=f"gg{i}", name=f"gg{i}") for i in range(2)]
            for i in range(2):
                nc.sync.dma_start(out=gg[i][:], in_=gg_d[128 * i:128 * (i + 1), :])

            # ------------- phase A: chan-LN -> x'' (f16) -------------
            # ctx gets the full (x-mu)*rstd (v needs it); qs gets only x-mu:
            # the q l2norm cancels any per-position scale, so rstd_qs (and the
            # whole variance pipeline) is unnecessary for the q path.
            # ctx and qs chunks are interleaved as two independent dependency
            # chains (qs stats borrow the otherwise-idle psMain banks).
            xpp_t = {}
            for name in ("ctx", "qs"):
                xpp_t[name] = [xpp.tile([128, N], F16, tag=f"xpp_{name}{i}",
                                        name=f"xpp_{name}{i}")
                               for i in range(2)]
            for ch in range(CH):
                sl = slice(ch * NCH, (ch + 1) * NCH)
                # --- ctx chunk: full LN ---
                xt = [xin.tile([128, NCH], F32, tag="xt", name="xt") for _ in range(2)]
                for i in range(2):
                    nc.sync.dma_start(out=xt[i][:],
                                      in_=ctx_d[128 * i:128 * (i + 1), sl])
                S_ps = psStat.tile([128, NCH], F32, tag="st")
                nc.tensor.matmul(S_ps[:], lhsT=ones128[:], rhs=xt[0][:],
                                 start=True, stop=False)
                nc.tensor.matmul(S_ps[:], lhsT=ones128[:], rhs=xt[1][:],
                                 start=False, stop=True)
                t_mu = statp.tile([128, NCH], F32, tag="t_mu", bufs=2)
                nc.vector.tensor_scalar(t_mu[:], S_ps[:], 1.0 / C,
                                        scalar2=None, op0=OP.mult)
                xsq = [xin.tile([128, NCH], F16, tag="xsq", name="xsq", bufs=1)
                       for _ in range(2)]
                for i in range(2):
                    nc.scalar.activation(xsq[i][:], xt[i][:], AF.Square)
                Q_ps = psStat.tile([128, NCH], F32, tag="st")
                nc.tensor.matmul(Q_ps[:], lhsT=ones128_f16[:], rhs=xsq[0][:],
                                 start=True, stop=False)
                nc.tensor.matmul(Q_ps[:], lhsT=ones128_f16[:], rhs=xsq[1][:],
                                 start=False, stop=True)
                t1 = statp.tile([128, NCH], F32, tag="se", name="t1")
                nc.scalar.activation(t1[:], t_mu[:], AF.Square)
                varb = statp.tile([128, NCH], F32, tag="varb")
                nc.vector.scalar_tensor_tensor(out=varb[:], in0=Q_ps[:],
                                               scalar=1.0 / C, in1=t1[:],
                                               op0=OP.mult, op1=OP.subtract)
                se = statp.tile([128, NCH], F32, tag="se")
                nc.scalar.activation(se[:], varb[:], AF.Sqrt, bias=eps_c[:])
                rstd_b = statp.tile([128, NCH], F32, tag="rstd_b")
                nc.vector.reciprocal_approx_fast(out=rstd_b[:], in_=se[:])
                for i in range(2):
                    xc = statp.tile([128, NCH], F32, tag="xr", bufs=2)
                    nc.gpsimd.tensor_sub(xc[:], xt[i][:], t_mu[:])
                    nc.vector.tensor_tensor(out=xpp_t["ctx"][i][:, sl],
                                            in0=xc[:], in1=rstd_b[:], op=OP.mult)
                # --- qs chunk: mean-subtract only (independent chain) ---
                xtq = [xin.tile([128, NCH], F32, tag="xtq", name="xtq")
                       for _ in range(2)]
                for i in range(2):
                    nc.sync.dma_start(out=xtq[i][:],
                                      in_=qs_d[128 * i:128 * (i + 1), sl])
                Sq_ps = psMain.tile([128, NCH], F32, tag="m")
                nc.tensor.matmul(Sq_ps[:], lhsT=ones128[:], rhs=xtq[0][:],
                                 start=True, stop=False)
                nc.tensor.matmul(Sq_ps[:], lhsT=ones128[:], rhs=xtq[1][:],
                                 start=False, stop=True)
                t_muq = statp.tile([128, NCH], F32, tag="t_muq", bufs=1)
                nc.vector.tensor_scalar(t_muq[:], Sq_ps[:], 1.0 / C,
                                        scalar2=None, op0=OP.mult)
                for i in range(2):
                    nc.gpsimd.tensor_sub(xpp_t["qs"][i][:, sl],
                                         xtq[i][:], t_muq[:])

            # ------------- phase B: software-pipelined per head-pair -------
            # Emission order interleaves pairs so each pair's attention (which
            # waits on its gather) is emitted after the next pair's
            # projections: the in-order PE queue then never stalls on a
            # gather.
            ao16 = [attnp.tile([128, N], BF16, tag=f"ao{p}", name=f"ao{p}")
                    for p in range(PAIRS)]
            il_t, qh_t, ksel_t, kbd_t, vbd_t = {}, {}, {}, {}, {}

            def do_b1(p):
                il = kvqp.tile([128, 2 * N], F16, tag="il", bufs=2, name=f"il{p}")
                qh = kvqp.tile([128, N], F16, tag="qh", bufs=2, name=f"qh{p}")
                il_t[p], qh_t[p] = il, qh
                for ch in range(CH):
                    sl = slice(ch * NCH, (ch + 1) * NCH)
                    # --- projections (k, v, q) for this chunk ---
                    kps = psMain.tile([128, NCH], F32, tag="m")
                    nc.tensor.matmul(kps[:], lhsT=wkvT[0][:, 128 * p:128 * (p + 1)],
                                     rhs=xpp_t["ctx"][0][:, sl], start=True, stop=False)
                    nc.tensor.matmul(kps[:], lhsT=wkvT[1][:, 128 * p:128 * (p + 1)],
                                     rhs=xpp_t["ctx"][1][:, sl], start=False, stop=True)
                    k16c = kvqp.tile([128, NCH], F16, tag="k16c", bufs=2)
                    nc.scalar.copy(k16c[:], kps[:])
                    vps = psMain.tile([128, NCH], F32, tag="m")
                    vo = INNER + 128 * p
                    nc.tensor.matmul(vps[:], lhsT=wkvT[0][:, vo:vo + 128],
                                     rhs=xpp_t["ctx"][0][:, sl], start=True, stop=False)
                    nc.tensor.matmul(vps[:], lhsT=wkvT[1][:, vo:vo + 128],
                                     rhs=xpp_t["ctx"][1][:, sl], start=False, stop=True)
                    nc.scalar.copy(il[:, 2 * sl.start + 1:2 * sl.stop:2], vps[:])
                    qps = psMain.tile([128, NCH], F32, tag="m")
                    nc.tensor.matmul(qps[:], lhsT=wqT[0][:, 128 * p:128 * (p + 1)],
                                     rhs=xpp_t["qs"][0][:, sl], start=True, stop=False)
                    nc.tensor.matmul(qps[:], lhsT=wqT[1][:, 128 * p:128 * (p + 1)],
                                     rhs=xpp_t["qs"][1][:, sl], start=False, stop=True)
                    # --- l2 factors + khat/qhat ---
                    k2c = kvqp.tile([128, NCH], F16, tag="k2c", bufs=1)
                    nc.vector.tensor_mul(k2c[:], k16c[:], k16c[:])
                    q2c = kvqp.tile([128, NCH], F16, tag="q2c", bufs=2)
                    nc.scalar.activation(q2c[:], qps[:], AF.Square)
                    rkps = psStat.tile([128, NCH], F32, tag="st")
                    nc.tensor.matmul(rkps[:], lhsT=halvesbc16[:], rhs=k2c[:],
                                     start=True, stop=True)
                    sek = statp.tile([128, NCH], F32, tag="se_", bufs=2)
                    nc.scalar.activation(sek[:], rkps[:], AF.Sqrt)
                    rbk = statp.tile([128, NCH], F32, tag="rb_", bufs=2)
                    nc.vector.reciprocal_approx_fast(out=rbk[:], in_=sek[:])
                    nc.gpsimd.tensor_tensor(out=il[:, 2 * sl.start:2 * sl.stop:2],
                                            in0=k16c[:], in1=rbk[:], op=OP.mult)
                    rqps = psStat.tile([128, NCH], F32, tag="st")
                    nc.tensor.matmul(rqps[:], lhsT=halvesbc16[:], rhs=q2c[:],
                                     start=True, stop=True)
                    seq2 = statp.tile([128, NCH], F32, tag="se_", bufs=2)
                    nc.scalar.activation(seq2[:], rqps[:], AF.Sqrt)
                    rbq = statp.tile([128, NCH], F32, tag="rb_", bufs=2)
                    nc.vector.reciprocal_approx_fast(out=rbq[:], in_=seq2[:])
                    nc.vector.tensor_tensor(out=qh[:, sl], in0=qps[:],
                                            in1=rbq[:], op=OP.mult)

            def do_b2(p):
                il, qh = il_t[p], qh_t[p]
                # --- segmented |khat| sums + q_probe + scores + topk ---
                il4 = il[:].rearrange("p (h w d) -> p h w d", h=64, w=64, d=2)
                kabs_r = pairp.tile([128, 64], F32, tag="kabsr")
                nc.vector.tensor_reduce(out=kabs_r[:], in_=il4[:, :, :, 0],
                                        axis=AX.X, op=OP.add, apply_absolute_value=True)
                il4c = il[:].rearrange("p (h w d) -> p w h d", h=64, w=64, d=2)
                kabs_c = pairp.tile([128, 64], F32, tag="kabsc")
                nc.vector.tensor_reduce(out=kabs_c[:], in_=il4c[:, :, :, 0],
                                        axis=AX.X, op=OP.add, apply_absolute_value=True)
                qp = pairp.tile([128, 1], F32, tag="qp")
                nc.vector.tensor_reduce(out=qp[:], in_=qh[:], axis=AX.X, op=OP.add)
                qp2 = pairp.tile([128, 2], F32, tag="qp2")
                nc.vector.memset(qp2[:], 0.0)
                nc.vector.tensor_copy(out=qp2[0:64, 0:1], in_=qp[0:64, :])
                nc.vector.tensor_copy(out=qp2[64:128, 1:2], in_=qp[64:128, :])
                sc_r = pairp.tile([2, 64], F32, tag="scr")
                sc_ps = psSmall.tile([2, 64], F32, tag="s")
                nc.tensor.matmul(sc_ps[:], lhsT=qp2[:], rhs=kabs_r[:],
                                 start=True, stop=True)
                nc.scalar.copy(sc_r[:], sc_ps[:])
                sc_c = pairp.tile([2, 64], F32, tag="scc")
                sc_ps2 = psSmall.tile([2, 64], F32, tag="s")
                nc.tensor.matmul(sc_ps2[:], lhsT=qp2[:], rhs=kabs_c[:],
                                 start=True, stop=True)
                nc.scalar.copy(sc_c[:], sc_ps2[:])
                mx = pairp.tile([2, 8], F32, tag="mx")
                idx_r = pairp.tile([2, 8], U32, tag="idxr")
                nc.vector.max(out=mx[:], in_=sc_r[:])
                nc.vector.max_index(out=idx_r[:], in_max=mx[:], in_values=sc_r[:])
                mxc = pairp.tile([2, 8], F32, tag="mxc")
                idx_c = pairp.tile([2, 8], U32, tag="idxc")
                nc.vector.max(out=mxc[:], in_=sc_c[:])
                nc.vector.max_index(out=idx_c[:], in_max=mxc[:], in_values=sc_c[:])
                idxr_f = pairp.tile([2, 8], F32, tag="idxrf")
                nc.vector.tensor_copy(out=idxr_f[:], in_=idx_r[:])
                idxc_f = pairp.tile([2, 8], F32, tag="idxcf")
                nc.vector.tensor_copy(out=idxc_f[:], in_=idx_c[:])
                # broadcast idx rows to all partitions by head half
                rbc_ps = psSmall.tile([128, 8], F32, tag="s")
                nc.tensor.matmul(rbc_ps[:], lhsT=zsel2[:], rhs=idxr_f[:],
                                 start=True, stop=True)
                rbc = pairp.tile([128, 8], F32, tag="rbc")
                nc.scalar.copy(rbc[:], rbc_ps[:])
                cbc_ps = psSmall.tile([128, 8], F32, tag="s")
                nc.tensor.matmul(cbc_ps[:], lhsT=zsel2[:], rhs=idxc_f[:],
                                 start=True, stop=True)
                cbc = pairp.tile([128, 8], F32, tag="cbc")
                nc.scalar.copy(cbc[:], cbc_ps[:])
                # Bcol[p] = idx_c[h(p), p%8]
                junk8 = pairp.tile([128, 8], F32, tag="junk8")
                nc.vector.tensor_mul(junk8[:], cbc[:], onehot8[:])
                Bcol = pairp.tile([128, 1], F32, tag="Bcol")
                nc.vector.tensor_reduce(out=Bcol[:], in_=junk8[:], axis=AX.X,
                                        op=OP.add)
                # wr[p, s] = idx_r[h(p), 2s + ((p>>3)&1)]
                wdiff = pairp.tile([128, 4], F32, tag="wdiff")
                nc.vector.tensor_sub(wdiff[:], rbc[:, 1:8:2], rbc[:, 0:8:2])
                wsel = pairp.tile([128, 4], F32, tag="wsel")
                nc.vector.tensor_scalar(wsel[:], wdiff[:], m8f[:], scalar2=None,
                                        op0=OP.mult)
                wr = pairp.tile([128, 4], F32, tag="wr")
                nc.vector.tensor_add(wr[:], wsel[:], rbc[:, 0:8:2])
                posfw = pairp.tile([128, 4], F32, tag="posfw")
                nc.vector.scalar_tensor_tensor(out=posfw[:], in0=wr[:], scalar=64.0,
                                               in1=Bcol[:].to_broadcast([128, 4]),
                                               op0=OP.mult, op1=OP.add)
                widx32 = pairp.tile([128, 4], I32, tag="widx32")
                nc.vector.tensor_copy(out=widx32[:], in_=posfw[:])
                widx = pairp.tile([128, 4], I16, tag="widx")
                nc.vector.tensor_copy(out=widx[:], in_=widx32[:])
                # --- gather ---
                ksel_il = selp.tile([128, 128], F16, tag="kselil", bufs=2,
                                    name=f"ksel{p}")
                nc.gpsimd.ap_gather(
                    out_ap=ksel_il[:].rearrange("p (k d) -> p k d", d=2),
                    in_ap=il[:].rearrange("p (n d) -> p n d", d=2),
                    idxs_ap=widx[:],
                    channels=128, num_elems=N, d=2, num_idxs=KEYS)
                ksel_t[p] = ksel_il

            def do_extract(p):
                ksel_il = ksel_t[p]
                kbd = selp.tile([128, 128], F16, tag="kbd", bufs=2, name=f"kbd{p}")
                nc.vector.memset(kbd[:], 0.0)
                nc.vector.tensor_copy(out=kbd[0:64, 0:64], in_=ksel_il[0:64, 0:128:2])
                nc.vector.tensor_copy(out=kbd[64:128, 64:128],
                                      in_=ksel_il[64:128, 0:128:2])
                vbd = selp.tile([128, 128], F16, tag="vbd", bufs=2, name=f"vbd{p}")
                nc.vector.memset(vbd[:], 0.0)
                for h in range(2):
                    o = 64 * h
                    tps = psSmall.tile([64, 64], F16, tag="s")
                    nc.tensor.transpose(out=tps[:], in_=ksel_il[o:o + 64, 1:128:2],
                                        identity=ident16[o:o + 64, :])
                    nc.scalar.copy(vbd[o:o + 64, o:o + 64], tps[:])
                kbd_t[p], vbd_t[p] = kbd, vbd

            def do_b3(p):
                kbd, vbd, qh = kbd_t[p], vbd_t[p], qh_t[p]
                # --- attention for this pair ---
                for ch in range(CH):
                    sl = slice(ch * NCH, (ch + 1) * NCH)
                    sps = psMain.tile([128, NCH], F32, tag="m")
                    nc.tensor.matmul(sps[:], lhsT=kbd[:], rhs=qh[:, sl],
                                     start=True, stop=True)
                    pt = ptp.tile([128, NCH], F16, tag="pT")
                    nc.scalar.activation(pt[:], sps[:], AF.Exp)
                    zps = psSmall.tile([2, NCH], F32, tag="s")
                    nc.tensor.matmul(zps[:], lhsT=halves2f[:], rhs=pt[:],
                                     start=True, stop=True)
                    zinv = ptp.tile([2, NCH], F32, tag="zinv")
                    nc.vector.reciprocal_approx_fast(out=zinv[:], in_=zps[:])
                    zb = psMain.tile([128, NCH], F32, tag="m")
                    nc.tensor.matmul(zb[:], lhsT=zsel2[:], rhs=zinv[:],
                                     start=True, stop=True)
                    ph16 = ptp.tile([128, NCH], F16, tag="ph16")
                    nc.vector.tensor_tensor(out=ph16[:], in0=pt[:], in1=zb[:],
                                            op=OP.mult)
                    pvs = psMain.tile([128, NCH], F32, tag="m")
                    nc.tensor.matmul(pvs[:], lhsT=vbd[:], rhs=ph16[:],
                                     start=True, stop=True)
                    nc.scalar.copy(ao16[p][:, sl], pvs[:])

            if stop_stage >= 2:
                do_b1(0); do_b2(0)
                do_b1(1); do_b2(1)
                do_extract(0)
                if stop_stage >= 3:
                    do_b3(0)
                do_b1(2); do_b2(2)
                do_extract(1)
                if stop_stage >= 3:
                    do_b3(1)
                do_b1(3); do_b2(3)
                do_extract(2)
                if stop_stage >= 3:
                    do_b3(2)
                do_extract(3)
                if stop_stage >= 3:
                    do_b3(3)

            # ------------- out-proj + out-LN + residual -------------
            y16 = [attnp.tile([128, N], BF16, tag=f"y16_{i}", name=f"y16_{i}") for i in range(2)]
            for ch in range(CH if stop_stage >= 4 else 0):
                sl = slice(ch * NCH, (ch + 1) * NCH)
                for i in range(2):
                    yps = psStat.tile([128, NCH], F32, tag="st")
                    for p in range(PAIRS):
                        nc.tensor.matmul(yps[:],
                                         lhsT=woutT[p][:, 128 * i:128 * (i + 1)],
                                         rhs=ao16[p][:, sl], start=(p == 0),
                                         stop=(p == 3))
                    nc.scalar.copy(y16[i][:, sl], yps[:])
                y2 = [finp.tile([128, NCH], BF16, tag="y2", name="y2", bufs=1) for _ in range(2)]
                for i in range(2):
                    nc.vector.tensor_mul(y2[i][:], y16[i][:, sl], y16[i][:, sl])
                S_ps = psStat.tile([128, NCH], F32, tag="st")
                nc.tensor.matmul(S_ps[:], lhsT=ones128_16[:], rhs=y16[0][:, sl],
                                 start=True, stop=False)
                nc.tensor.matmul(S_ps[:], lhsT=ones128_16[:], rhs=y16[1][:, sl],
                                 start=False, stop=True)
                Q_ps = psStat.tile([128, NCH], F32, tag="st")
                nc.tensor.matmul(Q_ps[:], lhsT=ones128_16[:], rhs=y2[0][:],
                                 start=True, stop=False)
                nc.tensor.matmul(Q_ps[:], lhsT=ones128_16[:], rhs=y2[1][:],
                                 start=False, stop=True)
                t_mu = finp.tile([128, NCH], F32, tag="ft_mu")
                nc.vector.tensor_scalar(t_mu[:], S_ps[:], 1.0 / C,
                                        scalar2=None, op0=OP.mult)
                t1 = finp.tile([128, NCH], F32, tag="fse", name="ft1")
                nc.scalar.activation(t1[:], t_mu[:], AF.Square)
                varb = finp.tile([128, NCH], F32, tag="fvarb")
                nc.vector.scalar_tensor_tensor(out=varb[:], in0=Q_ps[:], scalar=1.0 / C,
                                               in1=t1[:], op0=OP.mult, op1=OP.subtract)
                se = finp.tile([128, NCH], F32, tag="fse")
                nc.scalar.activation(se[:], varb[:], AF.Sqrt, bias=eps_c[:])
                rstd_b = finp.tile([128, NCH], F32, tag="frstd")
                nc.vector.reciprocal_approx_fast(out=rstd_b[:], in_=se[:])
                mu_b = finp.tile([128, NCH], F32, tag="fmu")
                nc.vector.tensor_mul(mu_b[:], t_mu[:], rstd_b[:])
                for i in range(2):
                    qs_t = finp.tile([128, NCH], F32, tag="qs_t")
                    nc.sync.dma_start(out=qs_t[:], in_=qs_d[128 * i:128 * (i + 1), sl])
                    t = finp.tile([128, NCH], F32, tag="fabc", name="fa", bufs=2)
                    nc.vector.tensor_tensor(out=t[:], in0=y16[i][:, sl],
                                            in1=rstd_b[:], op=OP.mult)
                    t2 = finp.tile([128, NCH], F32, tag="fabc", name="fb", bufs=2)
                    nc.vector.tensor_sub(t2[:], t[:], mu_b[:])
                    t3 = finp.tile([128, NCH], F32, tag="fabc", name="fc", bufs=2)
                    nc.scalar.activation(t3[:], t2[:], AF.Copy, scale=gg[i][:])
                    ot = finp.tile([128, NCH], F32, tag="fabc", name="fd", bufs=2)
                    nc.gpsimd.tensor_add(ot[:], t3[:], qs_t[:])
                    nc.sync.dma_start(out=out_d[128 * i:128 * (i + 1), sl], in_=ot[:])

            if stop_stage < 4:
                for i in range(2):
                    dummy = finp.tile([128, N], F32, tag="dummy")
                    nc.vector.memset(dummy[:], 0.0)
                    nc.sync.dma_start(out=out_d[128 * i:128 * (i + 1), :],
                                      in_=dummy[:])
    nc.finalize()
    return nc


_CACHE = {}


def kernel(**inputs):
    qsrc = np.asarray(inputs["query_source"], np.float32)
    ctx = np.asarray(inputs["context"], np.float32)
    cn_g = np.asarray(inputs["cn_g"], np.float32).reshape(C)
    cn_b = np.asarray(inputs["cn_b"], np.float32).reshape(C)
    qn_g = np.asarray(inputs["qn_g"], np.float32).reshape(C)
    qn_b = np.asarray(inputs["qn_b"], np.float32).reshape(C)
    on_g = np.asarray(inputs["on_g"], np.float32).reshape(C)
    on_b = np.asarray(inputs["on_b"], np.float32).reshape(C)
    w_kv = np.asarray(inputs["w_kv"], np.float32)
    w_q = np.asarray(inputs["w_q"], np.float32)
    w_out = np.asarray(inputs["w_out"], np.float32)
    gamma = float(np.asarray(inputs["gamma"], np.float32).reshape(()))

    assert np.abs(cn_b).max() == 0 and np.abs(qn_b).max() == 0 and \
        np.abs(on_b).max() == 0, "nonzero LN bias not implemented"

    import ml_dtypes
    bf16 = ml_dtypes.bfloat16
    wkvT = np.ascontiguousarray((w_kv * cn_g[None, :]).T).astype(np.float16)
    wqT = np.ascontiguousarray((w_q * qn_g[None, :]).T).astype(np.float16)
    woutT = np.ascontiguousarray(w_out.T).astype(bf16)
    gg = np.ascontiguousarray((gamma * on_g).reshape(C, 1), np.float32)

    p_idx = np.arange(128)
    identc = np.zeros((128, 64), np.float16)
    identc[p_idx, p_idx % 64] = 1.0
    onehot8c = (p_idx[:, None] % 8 == np.arange(8)[None, :]).astype(np.float32)
    m8ic = (((p_idx >> 3) & 1).astype(np.int32)).reshape(128, 1)
    zsel2c = (np.arange(128)[None, :] // 64 ==
              np.arange(2)[:, None]).astype(np.float32)

    if "nc" not in _CACHE:
        _CACHE["nc"] = build_program()
    nc = _CACHE["nc"]

    B = qsrc.shape[0]
    in_maps = []
    for b in range(B):
        in_maps.append({
            "ctx": np.ascontiguousarray(ctx[b].reshape(C, N)),
            "qsrc": np.ascontiguousarray(qsrc[b].reshape(C, N)),
            "wkvT": wkvT,
            "wqT": wqT,
            "woutT": woutT,
            "gg": gg,
            "identc": identc,
            "onehot8c": onehot8c,
            "m8ic": m8ic,
            "m8fc": m8ic.astype(np.float32),
            "zsel2c": zsel2c,
        })
    res = run_bass_kernel_spmd(nc, in_maps, core_ids=list(range(8)))
    outs = [np.asarray(r["out"], np.float32).reshape(1, C, 64, 64)
            for r in res.results]
    return np.concatenate(outs, axis=0)

